# revision 14
# baseline (speedup 1.0000x reference)
"""Trainium2 Bass kernel for nn_CNN_RNN (select-GRU -> compact -> 2xGRU -> KimCNN).

Sharding: pure data-parallel, batch 64 -> 8 cores x 8.
Device NEFF1: select-gate input projection (fp32) + fp16-compensated select GRU scan
  -> per-(t,b) argmax margins.
Host: argmax bits -> stable-compaction gather indices (pure data movement) -> gathered
  embedding fold (bf16).
Device NEFF2: layer input projections (bf16), two masked GRU scans (bf16,
  weight-stationary), Kim-CNN convs as shifted matmuls, max-pool+relu, output linear.
NEFF2 is specialized (and cached) per runtime T_pad = ceil(max(nsel)/64)*64: the layer
scans, projections, and convs only run to the longest compacted sequence; skipped conv
windows are all-zero and reintroduced exactly via a max-with-0 before the relu+bias.
GRU tails use h' = n*(1-z) + z*h with (1-z) and z*h computed during the PE's n-gate
matmuls, keeping the serial post-matmul chain one op shorter.
"""
import numpy as np
import ml_dtypes

import concourse.bass as bass
import concourse.mybir as mybir
from concourse import bacc
from concourse.tile import TileContext
from contextlib import ExitStack

F32, F16, BF16 = mybir.dt.float32, mybir.dt.float16, mybir.dt.bfloat16
AF = mybir.ActivationFunctionType
ALU = mybir.AluOpType
PE, DVE, ACT = mybir.EngineType.PE, mybir.EngineType.DVE, mybir.EngineType.Activation

B, T, E, H, NF = 64, 512, 768, 512, 256
FS = (3, 4, 5)
NC = 8
BL = B // NC          # batch per core
UNROLL = 32
NITER = T // UNROLL
I4 = 64 // UNROLL   # iters per 64-t chunk


# ---------------------------------------------------------------- NEFF1 ----

def build_neff1():
    nc = bacc.Bacc("TRN2", target_bir_lowering=False, debug=False, num_devices=NC)
    embH_in = nc.dram_tensor("embH", [128, 6 * T * BL], F16, kind="ExternalInput").ap()
    embL_in = nc.dram_tensor("embL", [128, 6 * T * BL], F16, kind="ExternalInput").ap()
    WihH_in = nc.dram_tensor("WihH", [6, 128, 1536], F16, kind="ExternalInput").ap()
    WihL_in = nc.dram_tensor("WihL", [6, 128, 1536], F16, kind="ExternalInput").ap()
    WTc_in = nc.dram_tensor("WTc", [4, 128, 1536], F16, kind="ExternalInput").ap()
    wdP_in = nc.dram_tensor("wdP", [8, 128, 128], F16, kind="ExternalInput").ap()
    biasC_in = nc.dram_tensor("biasC", [128, 12], F32, kind="ExternalInput").ap()
    margins_out = nc.dram_tensor("margins", [T * BL], F32, kind="ExternalOutput").ap()

    with TileContext(nc) as tc, ExitStack() as ctx:
        wpool = ctx.enter_context(tc.tile_pool(name="w", bufs=1))
        dpool = ctx.enter_context(tc.tile_pool(name="dram", bufs=1, space="DRAM"))
        gic = dpool.tile([128, NITER * UNROLL * 96], F32, tag="gic")

        WihH, WihL = [], []
        for k in range(6):
            wh = wpool.tile([128, 1536], F16, tag=f"wihH{k}", name=f"wihH{k}")
            nc.sync.dma_start(out=wh, in_=WihH_in[k])
            WihH.append(wh)
            wl = wpool.tile([128, 1536], F16, tag=f"wihL{k}", name=f"wihL{k}")
            nc.sync.dma_start(out=wl, in_=WihL_in[k])
            WihL.append(wl)
        WTc = []
        for k in range(4):
            wt = wpool.tile([128, 1536], F16, tag=f"wtc{k}")
            nc.sync.dma_start(out=wt, in_=WTc_in[k])
            WTc.append(wt)
        wdP = []
        for i in range(8):
            wt = wpool.tile([128, 128], F16, tag=f"wdP{i}", name=f"wdP{i}")
            nc.sync.dma_start(out=wt, in_=wdP_in[i])
            wdP.append(wt)
        biasC = wpool.tile([128, 12], F32, tag="biasC")
        nc.sync.dma_start(out=biasC, in_=biasC_in)

        # --- phase A: gi_c via f16 hi/lo 3-product (≈fp32-exact, 1 cyc/row)
        epool = ctx.enter_context(tc.tile_pool(name="emb", bufs=2))
        stpool = ctx.enter_context(tc.tile_pool(name="stage", bufs=2))
        ppool = ctx.enter_context(tc.tile_pool(name="psA", bufs=2, space="PSUM"))
        for nch in range(8):
            eh = epool.tile([128, 6, 512], F16, tag="eth", name=f"eth{nch}")
            nc.sync.dma_start(out=eh, in_=embH_in.rearrange("p (k c) -> p k c", k=6)[:, :, nch * 512:(nch + 1) * 512])
            el = epool.tile([128, 6, 512], F16, tag="etl", name=f"etl{nch}")
            nc.sync.dma_start(out=el, in_=embL_in.rearrange("p (k c) -> p k c", k=6)[:, :, nch * 512:(nch + 1) * 512])
            stage = stpool.tile([128, 6144], F32, tag="stageA")
            st4 = stage.rearrange("p (i j c b) -> p i j c b", i=I4, j=UNROLL, c=12)
            for m in range(12):
                ps = ppool.tile([128, 512], F32, tag="psA")
                for pi, (W6, et) in enumerate(((WihH, eh), (WihH, el), (WihL, eh))):
                    for k in range(6):
                        nc.tensor.matmul(ps, W6[k][:, m * 128:(m + 1) * 128], et[:, k, :],
                                         start=(pi == 0 and k == 0), stop=(pi == 2 and k == 5))
                # psum col = t'*8+b, t' = i*16+j  ->  stage[p, i, j, m, b]
                nc.vector.tensor_scalar(
                    out=st4[:, :, :, m, :],
                    in0=ps.rearrange("p (i j b) -> p i j b", i=I4, j=UNROLL),
                    scalar1=biasC[:, m:m + 1], scalar2=None, op0=ALU.add)
            nc.sync.dma_start(out=gic[:, nch * 6144:(nch + 1) * 6144], in_=stage)

        # --- phase B: select scan (fp16 W, split-fp16 h, fp32 psum)
        spool = ctx.enter_context(tc.tile_pool(name="selstate", bufs=1))
        gpool = ctx.enter_context(tc.tile_pool(name="selgi", bufs=3))
        ppoolB = ctx.enter_context(tc.tile_pool(name="psB", bufs=2, space="PSUM"))
        tpool = ctx.enter_context(tc.tile_pool(name="seltmp", bufs=3))
        mpool = ctx.enter_context(tc.tile_pool(name="selmarg", bufs=3))

        hT = spool.tile([128, 32], F32, tag="selhT")
        hpk = spool.tile([128, 64], F16, tag="selhpk")
        nc.vector.memset(hT, 0.0)
        nc.vector.memset(hpk, 0.0)

        with tc.For_i(0, NITER, 1, hint_engines=(PE, DVE, ACT)) as it:
            gi = gpool.tile([128, UNROLL * 96], F32, tag="selgi")
            nc.sync.dma_start(out=gi[:, 0:384], in_=gic[:, bass.ds(it * (UNROLL * 96), 384)])
            nc.sync.dma_start(out=gi[:, 384:UNROLL * 96],
                              in_=gic[:, bass.ds(it * (UNROLL * 96) + 384, UNROLL * 96 - 384)])
            marg = mpool.tile([1, UNROLL * 8], F32, tag="selmarg")
            for j in range(UNROLL):
                ps_rz = ppoolB.tile([128, 128], F32, tag="ps_rz")
                ps_n = ppoolB.tile([128, 64], F32, tag="ps_n")
                ps_m_t = ppoolB.tile([128, 16], F32, tag="ps_m")
                ps_m = ps_m_t[0:1, :]
                for m in range(12):
                    ps = ps_rz[:, m * 16:(m + 1) * 16] if m < 8 else ps_n[:, (m - 8) * 16:(m - 7) * 16]
                    for k in range(4):
                        nc.tensor.matmul(ps, WTc[k][:, m * 128:(m + 1) * 128],
                                         hpk[:, k * 16:(k + 1) * 16],
                                         start=(k == 0), stop=(k == 3))
                gslice = gi[:, j * 96:(j + 1) * 96]
                rz_hi = ps_rz.rearrange("p (m s) -> p m s", s=16)[:, :, 0:8]
                rz_lo = ps_rz.rearrange("p (m s) -> p m s", s=16)[:, :, 8:16]
                a = tpool.tile([128, 64], F32, tag="a")
                nc.vector.tensor_add(out=a.rearrange("p (m s) -> p m s", s=8), in0=rz_hi,
                                     in1=gslice[:, 0:64].rearrange("p (m s) -> p m s", s=8))
                a2 = tpool.tile([128, 64], F32, tag="a2")
                nc.vector.tensor_add(out=a2.rearrange("p (m s) -> p m s", s=8),
                                     in0=a.rearrange("p (m s) -> p m s", s=8), in1=rz_lo)
                rz = tpool.tile([128, 64], F32, tag="rz")
                nc.scalar.activation(rz, a2, AF.Sigmoid)
                zc = tpool.tile([128, 32], F32, tag="zc")
                nc.vector.tensor_scalar(out=zc, in0=rz[:, 32:64], scalar1=-1.0,
                                        scalar2=1.0, op0=ALU.mult, op1=ALU.add)
                zh = tpool.tile([128, 32], F32, tag="zh")
                nc.vector.tensor_mul(out=zh, in0=hT, in1=rz[:, 32:64])
                n_hi = ps_n.rearrange("p (m s) -> p m s", s=16)[:, :, 0:8]
                n_lo = ps_n.rearrange("p (m s) -> p m s", s=16)[:, :, 8:16]
                t2a = tpool.tile([128, 32], F32, tag="t2a")
                nc.vector.tensor_mul(out=t2a.rearrange("p (m s) -> p m s", s=8), in0=n_hi,
                                     in1=rz[:, 0:32].rearrange("p (m s) -> p m s", s=8))
                t2b = tpool.tile([128, 32], F32, tag="t2b")
                nc.vector.tensor_mul(out=t2b.rearrange("p (m s) -> p m s", s=8), in0=n_lo,
                                     in1=rz[:, 0:32].rearrange("p (m s) -> p m s", s=8))
                u1 = tpool.tile([128, 32], F32, tag="u1")
                nc.vector.tensor_add(out=u1, in0=t2a, in1=gslice[:, 64:96])
                u = tpool.tile([128, 32], F32, tag="u")
                nc.vector.tensor_add(out=u, in0=u1, in1=t2b)
                nn_ = tpool.tile([128, 32], F32, tag="nn_")
                nc.scalar.activation(nn_, u, AF.Tanh)
                v = tpool.tile([128, 32], F32, tag="v")
                nc.vector.tensor_mul(out=v, in0=nn_, in1=zc)
                nc.vector.tensor_add(out=hT, in0=v, in1=zh)
                hpk3 = hpk.rearrange("p (k s) -> p k s", s=16)
                hT3 = hT.rearrange("p (k s) -> p k s", s=8)
                nc.vector.tensor_copy(out=hpk3[:, :, 0:8], in_=hT3)
                nc.vector.tensor_sub(out=hpk3[:, :, 8:16], in0=hT3, in1=hpk3[:, :, 0:8])
                for k in range(4):
                    # full 128-col lhsT (wd in col 0, rest zero) keeps the PE
                    # tile config identical to the gh matmuls; margin = row 0
                    nc.tensor.matmul(ps_m_t, wdP[k * 2], hpk[:, k * 16:(k + 1) * 16],
                                     start=(k == 0), stop=False)
                    nc.tensor.matmul(ps_m_t, wdP[k * 2 + 1], hpk[:, k * 16:(k + 1) * 16],
                                     start=False, stop=(k == 3))
                mc = mpool.tile([1, 8], F32, tag="mc")
                nc.vector.tensor_copy(out=mc, in_=ps_m[:, 0:8])
                nc.vector.tensor_add(out=marg[:, j * 8:(j + 1) * 8], in0=mc, in1=ps_m[:, 8:16])
            nc.sync.dma_start(out=margins_out[bass.ds(it * (UNROLL * 8), UNROLL * 8)], in_=marg)
    nc.compile()
    return nc


def build_neff1_v2():
    """Select-policy GRU scan, fully unrolled (512 steps), with the gi_c
    input projection (f16 hi/lo 3-product, ~fp32-exact) sprinkled into PE
    idle slots between scan steps. gi_rz is accumulated into the gate PSUM
    via identity matmuls; zc/zh run on GPSIMD; margins (wd . h) are 4 fp32
    matmuls per step + an ACT copy. h stays fp32 (select bits must be exact:
    ~20 flipped bits already cost 1.5e-2 rel err downstream)."""
    nc = bacc.Bacc("TRN2", target_bir_lowering=False, debug=False, num_devices=NC)
    TBL = T * BL
    embH_in = nc.dram_tensor("embH", [128, 6 * TBL], F16, kind="ExternalInput").ap()
    embL_in = nc.dram_tensor("embL", [128, 6 * TBL], F16, kind="ExternalInput").ap()
    WihH_in = nc.dram_tensor("WihH", [6, 128, 1536], F16, kind="ExternalInput").ap()
    WihL_in = nc.dram_tensor("WihL", [6, 128, 1536], F16, kind="ExternalInput").ap()
    WTc_in = nc.dram_tensor("WTc", [4, 128, 1536], F32, kind="ExternalInput").ap()
    wdT_in = nc.dram_tensor("wdT", [128, 4], F32, kind="ExternalInput").ap()
    identf_in = nc.dram_tensor("identf", [128, 128], F32, kind="ExternalInput").ap()
    biasC_in = nc.dram_tensor("biasC", [128, 12], F32, kind="ExternalInput").ap()
    margins_out = nc.dram_tensor("margins", [TBL], F32, kind="ExternalOutput").ap()

    NS = T // 32  # 16 sub-chunks

    with TileContext(nc) as tc, ExitStack() as ctx:
        wpool = ctx.enter_context(tc.tile_pool(name="w1", bufs=1))
        etpool = ctx.enter_context(tc.tile_pool(name="et1", bufs=3))
        stpool = ctx.enter_context(tc.tile_pool(name="st1", bufs=3))
        pApool = ctx.enter_context(tc.tile_pool(name="psA1", bufs=2, space="PSUM"))
        pSpool = ctx.enter_context(tc.tile_pool(name="psS1", bufs=2, space="PSUM"))
        tpool = ctx.enter_context(tc.tile_pool(name="tmp1s", bufs=3))

        WihH, WihL = [], []
        for k in range(6):
            wh = wpool.tile([128, 1536], F16, tag=f"wihH{k}", name=f"wihH{k}")
            nc.sync.dma_start(out=wh, in_=WihH_in[k])
            WihH.append(wh)
            wl = wpool.tile([128, 1536], F16, tag=f"wihL{k}", name=f"wihL{k}")
            nc.sync.dma_start(out=wl, in_=WihL_in[k])
            WihL.append(wl)
        WTc = []
        for k in range(4):
            wt = wpool.tile([128, 1536], F32, tag=f"wtc{k}", name=f"wtc{k}")
            nc.sync.dma_start(out=wt, in_=WTc_in[k])
            WTc.append(wt)
        wdT = wpool.tile([128, 4], F32, tag="wdT")
        nc.sync.dma_start(out=wdT, in_=wdT_in)
        identf = wpool.tile([128, 128], F32, tag="identf")
        nc.sync.dma_start(out=identf, in_=identf_in)
        biasC = wpool.tile([128, 12], F32, tag="biasC")
        nc.sync.dma_start(out=biasC, in_=biasC_in)

        hT = wpool.tile([128, 32], F32, tag="hT")
        nc.vector.memset(hT, 0.0)
        marg = wpool.tile([1, TBL], F32, tag="marg")

        embH = embH_in.rearrange("p (k q) -> p k q", k=6)
        embL = embL_in.rearrange("p (k q) -> p k q", k=6)
        stages = [None] * NS
        et_h = [None] * NS
        et_l = [None] * NS

        def dma_et(s):
            eh = etpool.tile([128, 6, 256], F16, tag="eth", name=f"eth{s}")
            nc.sync.dma_start(out=eh, in_=embH[:, :, s * 256:(s + 1) * 256])
            el = etpool.tile([128, 6, 256], F16, tag="etl", name=f"etl{s}")
            nc.sync.dma_start(out=el, in_=embL[:, :, s * 256:(s + 1) * 256])
            et_h[s], et_l[s] = eh, el

        def pa_thunks(s):
            st = stpool.tile([128, 32, 12, 8], F32, tag="stage", name=f"stage{s}")
            stages[s] = st
            th = []
            prods = [(WihH, et_h[s]), (WihH, et_l[s]), (WihL, et_h[s])]
            for m in range(12):
                ps_box = []
                def mk_mm(m=m, p=0, ps_box=ps_box):
                    if p == 0:
                        ps_box.append(pApool.tile([128, 256], F32, tag="psA",
                                                  name=f"psA1_{s}_{m}"))
                    W6, et = prods[p]
                    for k in range(6):
                        nc.tensor.matmul(ps_box[0], W6[k][:, m * 128:(m + 1) * 128],
                                         et[:, k, :], start=(p == 0 and k == 0),
                                         stop=(p == 2 and k == 5))
                def mk_act(m=m, ps_box=ps_box):
                    nc.scalar.activation(stages[s][:, :, m, :], ps_box[0], AF.Identity,
                                         bias=biasC[:, m:m + 1])
                for p in range(3):
                    th.append((lambda m=m, p=p, ps_box=ps_box: mk_mm(m, p, ps_box)))
                th.append(mk_act)
            return th

        def alloc_ident(t):
            s, j = divmod(t, 32)
            ps_rz = pSpool.tile([128, 112], F32, tag="ps_rz", name=f"psrz_{t}")
            nc.tensor.matmul(ps_rz[:, 0:64], identf, stages[s][:, j, 0:8, :],
                             start=True, stop=False)
            return ps_rz  # [128,112]: rz 0:64, n 64:96, margin row0 96:104

        # ---- prologue
        dma_et(0)
        dma_et(1)
        pending = []
        for th in pa_thunks(0):
            th()
        dma_et(2)
        pending.extend(pa_thunks(1))
        ps_next = alloc_ident(0)

        for t in range(T):
            s, j = divmod(t, 32)
            if j == 0 and s >= 1:
                if s + 2 < NS:
                    dma_et(s + 2)
                if s + 1 < NS:
                    pending.extend(pa_thunks(s + 1))
            ps_all = ps_next
            ps_rz = ps_all[:, 0:64]
            ps_n = ps_all[:, 64:96]
            ps_m = ps_all[0:1, 96:104]
            st = stages[s]
            for m in range(8):
                for k in range(4):
                    nc.tensor.matmul(ps_rz[:, m * 8:(m + 1) * 8],
                                     WTc[k][:, m * 128:(m + 1) * 128],
                                     hT[:, k * 8:(k + 1) * 8],
                                     start=False, stop=(k == 3))
            for m in range(4):
                for k in range(4):
                    nc.tensor.matmul(ps_n[:, m * 8:(m + 1) * 8],
                                     WTc[k][:, (8 + m) * 128:(9 + m) * 128],
                                     hT[:, k * 8:(k + 1) * 8],
                                     start=(k == 0), stop=(k == 3))
            rz = tpool.tile([128, 64], F32, tag="rz1", name=f"rz1_{t}")
            nc.scalar.activation(rz, ps_rz, AF.Sigmoid)
            zc = tpool.tile([128, 32], F32, tag="zc1", name=f"zc1_{t}")
            nc.vector.tensor_scalar(out=zc, in0=rz[:, 32:64], scalar1=-1.0,
                                    scalar2=1.0, op0=ALU.mult, op1=ALU.add)
            zh = tpool.tile([128, 32], F32, tag="zh1", name=f"zh1_{t}")
            nc.vector.tensor_mul(out=zh, in0=rz[:, 32:64], in1=hT)
            t2 = tpool.tile([128, 32], F32, tag="t2_1", name=f"t2_1_{t}")
            nc.vector.tensor_mul(out=t2, in0=ps_n, in1=rz[:, 0:32])
            u = tpool.tile([128, 32], F32, tag="u1", name=f"u1_{t}")
            nc.vector.tensor_add(out=u.rearrange("p (c b) -> p c b", c=4),
                                 in0=t2.rearrange("p (c b) -> p c b", c=4),
                                 in1=st[:, j, 8:12, :])
            nn = tpool.tile([128, 32], F32, tag="nn1", name=f"nn1_{t}")
            nc.scalar.activation(nn, u, AF.Tanh)
            v = tpool.tile([128, 32], F32, tag="v1", name=f"v1_{t}")
            nc.vector.tensor_mul(out=v, in0=nn, in1=zc)
            nc.vector.tensor_add(out=hT, in0=v, in1=zh)
            if t + 1 < T:
                ps_next = alloc_ident(t + 1)
            for k in range(4):
                nc.tensor.matmul(ps_m, wdT[:, k:k + 1], hT[:, k * 8:(k + 1) * 8],
                                 start=(k == 0), stop=(k == 3))
            nc.scalar.copy(out=marg[0:1, t * 8:(t + 1) * 8], in_=ps_m)
            nrun = -(-len(pending) // (32 - j))
            for _ in range(min(nrun, len(pending))):
                pending.pop(0)()
            if t % 64 == 63:
                nc.sync.dma_start(out=margins_out[bass.ds((t - 63) * 8, 512)],
                                  in_=marg[0:1, (t - 63) * 8:(t + 1) * 8])
    nc.compile()
    return nc


# ---------------------------------------------------------------- NEFF2 ----

def emit_layer_scan(nc, tc, ctx, name, WhT, gi_dram, mask, masku, ybuf, ycols, n_it):
    """Masked bf16 GRU scan. WhT: 4x sbuf [128,1536] bf16. gi_dram: [128, NITER*1536] bf16.
    mask: sbuf [128, T*BL] bf16 (1/0). ybuf: sbuf [128, 4*ycols] bf16 out (col c*ycols + t*8+b)."""
    spool = ctx.enter_context(tc.tile_pool(name=f"{name}st", bufs=1))
    gpool = ctx.enter_context(tc.tile_pool(name=f"{name}gi", bufs=3))
    ppool = ctx.enter_context(tc.tile_pool(name=f"{name}ps", bufs=2, space="PSUM"))
    tpool = ctx.enter_context(tc.tile_pool(name=f"{name}tmp", bufs=3))

    h16 = spool.tile([128, 32], BF16, tag=f"{name}h16")
    nc.vector.memset(h16, 0.0)
    yb4 = ybuf.rearrange("p (c q) -> p c q", c=4)

    with tc.For_i(0, n_it, 1, hint_engines=(PE, DVE, ACT)) as it:
        gi = gpool.tile([128, UNROLL * 96], BF16, tag=f"{name}gi")
        nc.sync.dma_start(out=gi, in_=gi_dram[:, bass.ds(it * (UNROLL * 96), UNROLL * 96)])
        for j in range(UNROLL):
            tcol = it * UNROLL * 8 + j * 8
            ps_rz = ppool.tile([128, 64], F32, tag=f"{name}ps_rz")
            ps_n = ppool.tile([128, 32], F32, tag=f"{name}ps_n")
            for m in range(12):
                ps = ps_rz[:, m * 8:(m + 1) * 8] if m < 8 else ps_n[:, (m - 8) * 8:(m - 7) * 8]
                for k in range(4):
                    nc.tensor.matmul(ps, WhT[k][:, m * 128:(m + 1) * 128],
                                     h16[:, k * 8:(k + 1) * 8],
                                     start=(k == 0), stop=(k == 3))
            gslice = gi[:, j * 96:(j + 1) * 96]
            a = tpool.tile([128, 64], F32, tag=f"{name}a")
            nc.vector.tensor_add(out=a, in0=ps_rz, in1=gslice[:, 0:64])
            rz = tpool.tile([128, 64], F32, tag=f"{name}rz")
            nc.scalar.activation(rz, a, AF.Sigmoid)
            zc = tpool.tile([128, 32], F32, tag=f"{name}zc")
            nc.vector.tensor_scalar(out=zc, in0=rz[:, 32:64], scalar1=-1.0,
                                    scalar2=1.0, op0=ALU.mult, op1=ALU.add)
            zh = tpool.tile([128, 32], F32, tag=f"{name}zh")
            nc.vector.tensor_mul(out=zh, in0=h16, in1=rz[:, 32:64])
            t2 = tpool.tile([128, 32], F32, tag=f"{name}t2")
            nc.vector.tensor_mul(out=t2, in0=ps_n, in1=rz[:, 0:32])
            u = tpool.tile([128, 32], F32, tag=f"{name}u")
            nc.vector.tensor_add(out=u, in0=t2, in1=gslice[:, 64:96])
            nn_ = tpool.tile([128, 32], F32, tag=f"{name}nn")
            nc.scalar.activation(nn_, u, AF.Tanh)
            v = tpool.tile([128, 32], F32, tag=f"{name}v")
            nc.vector.tensor_mul(out=v, in0=nn_, in1=zc)
            hn16 = tpool.tile([128, 32], BF16, tag=f"{name}hn16")
            nc.vector.tensor_add(out=hn16, in0=v, in1=zh)
            mview = mask[:, bass.ds(tcol, 8)].unsqueeze(1).broadcast_to([128, 4, 8])
            muview = masku[:, bass.ds(tcol, 8)].unsqueeze(1).broadcast_to([128, 4, 8])
            hn3 = hn16.rearrange("p (c b) -> p c b", c=4)
            # y = m * h'  (zero where invalid)
            nc.vector.tensor_mul(out=yb4[:, :, bass.ds(tcol, 8)], in0=hn3, in1=mview)
            # h <- m ? h' : h
            nc.vector.copy_predicated(out=h16.rearrange("p (c b) -> p c b", c=4),
                                      mask=muview, data=hn3)


def build_neff2_v2(t_pad, n_full, dump=False):
    """Fused L0+L1 GRU scans in 32-step sub-chunks with software pipelining:
    tick s: L0 chunk s | L1 chunk s-2, with gi0 proj (s+1), gi1 proj (s-1),
    and conv (s-3) matmuls sprinkled into PE idle between scan steps.
    Steps below 32*n_full skip all masking; h state lives in-place in the
    y buffer so the GRU update writes y directly.
    """
    nc = bacc.Bacc("TRN2", target_bir_lowering=False, debug=False, num_devices=NC)
    TB = t_pad * BL
    nsub = t_pad // 32
    TBP = (t_pad + 16) * BL
    nembT_in = nc.dram_tensor("nembT", [128, 6 * TB], BF16, kind="ExternalInput").ap()
    mask_in = nc.dram_tensor("maskf", [128, TB], BF16, kind="ExternalInput").ap()
    masku_in = nc.dram_tensor("masku", [128, TB], mybir.dt.uint8, kind="ExternalInput").ap()
    Wih0T_in = nc.dram_tensor("Wih0T", [6, 128, 1536], BF16, kind="ExternalInput").ap()
    WhT0_in = nc.dram_tensor("WhT0", [4, 128, 1536], BF16, kind="ExternalInput").ap()
    Wih1T_in = nc.dram_tensor("Wih1T", [4, 128, 1536], BF16, kind="ExternalInput").ap()
    WhT1_in = nc.dram_tensor("WhT1", [4, 128, 1536], BF16, kind="ExternalInput").ap()
    bias0_in = nc.dram_tensor("bias0", [128, 12], F32, kind="ExternalInput").ap()
    bias1_in = nc.dram_tensor("bias1", [128, 12], F32, kind="ExternalInput").ap()
    identb_in = nc.dram_tensor("identb", [128, 128], BF16, kind="ExternalInput").ap()
    Wconv_in = nc.dram_tensor("Wconv", [128, 12 * 4 * 256], BF16, kind="ExternalInput").ap()
    bconv_in = nc.dram_tensor("bconv", [128, 6], F32, kind="ExternalInput").ap()
    WoT_in = nc.dram_tensor("WoT", [128, 6], F32, kind="ExternalInput").ap()
    bo_in = nc.dram_tensor("bo", [1, 1], F32, kind="ExternalInput").ap()
    out_dram = nc.dram_tensor("out", [1, BL], F32, kind="ExternalOutput").ap()
    if dump:
        TBP_ = (t_pad + 16) * BL
        y0d_out = nc.dram_tensor("y0d", [128, 4 * (t_pad * BL + 8)], BF16, kind="ExternalOutput").ap()
        y1d_out = nc.dram_tensor("y1d", [128, 4 * (TBP_ + 8)], BF16, kind="ExternalOutput").ap()
        g0d_out = nc.dram_tensor("g0d", [128, 32 * 12 * 8], BF16, kind="ExternalOutput").ap()
        s0d_out = nc.dram_tensor("s0d", [128, 96 + 64 + 6 * 32], F32, kind="ExternalOutput").ap()

    with TileContext(nc) as tc, ExitStack() as ctx:
        wpool = ctx.enter_context(tc.tile_pool(name="w2", bufs=1))
        etpool = ctx.enter_context(tc.tile_pool(name="et2", bufs=3))
        g0pool = ctx.enter_context(tc.tile_pool(name="g0st", bufs=2))
        g1pool = ctx.enter_context(tc.tile_pool(name="g1st", bufs=2))
        pApool = ctx.enter_context(tc.tile_pool(name="psA2", bufs=2, space="PSUM"))
        pBpool = ctx.enter_context(tc.tile_pool(name="psB2", bufs=2, space="PSUM"))
        pC0 = ctx.enter_context(tc.tile_pool(name="psL0", bufs=2, space="PSUM"))
        pC1 = ctx.enter_context(tc.tile_pool(name="psL1", bufs=2, space="PSUM"))
        t0pool = ctx.enter_context(tc.tile_pool(name="tmp0", bufs=3))
        t1pool = ctx.enter_context(tc.tile_pool(name="tmp1", bufs=3))

        def loadw(name, src, n, dtype=BF16):
            out = []
            for k in range(n):
                wt = wpool.tile([128, 1536], dtype, tag=f"{name}{k}", name=f"{name}{k}")
                nc.sync.dma_start(out=wt, in_=src[k])
                out.append(wt)
            return out

        Wih0T = loadw("wih0", Wih0T_in, 6)
        WhT0 = loadw("wh0", WhT0_in, 4)
        Wih1T = loadw("wih1", Wih1T_in, 4)
        WhT1 = loadw("wh1", WhT1_in, 4)
        bias0 = wpool.tile([128, 12], F32, tag="bias0")
        nc.sync.dma_start(out=bias0, in_=bias0_in)
        bias1 = wpool.tile([128, 12], F32, tag="bias1")
        nc.sync.dma_start(out=bias1, in_=bias1_in)
        identb = wpool.tile([128, 128], BF16, tag="identb")
        nc.sync.dma_start(out=identb, in_=identb_in)
        Wconv_t = wpool.tile([128, 12 * 4 * 256], BF16, tag="Wconv")
        nc.sync.dma_start(out=Wconv_t, in_=Wconv_in)
        Wconv = Wconv_t.rearrange("p (d k c) -> p d k c", d=12, k=4)
        bconv = wpool.tile([128, 6], F32, tag="bconv")
        nc.sync.dma_start(out=bconv, in_=bconv_in)
        WoT = wpool.tile([128, 6], F32, tag="WoT")
        nc.sync.dma_start(out=WoT, in_=WoT_in)
        bo_sb = wpool.tile([1, 1], F32, tag="bo_sb")
        nc.sync.dma_start(out=bo_sb, in_=bo_in)
        maskf = wpool.tile([128, TB], BF16, tag="maskf")
        nc.sync.dma_start(out=maskf, in_=mask_in)
        masku = wpool.tile([128, TB], mybir.dt.uint8, tag="masku")
        nc.sync.dma_start(out=masku, in_=masku_in)

        # y buffers: col (t+1)*8+b per k-plane; slot 0 = zeroed h(-1)
        y0buf = wpool.tile([128, 4 * (TB + 8)], BF16, tag="y0buf")
        y1buf = wpool.tile([128, 4 * (TBP + 8)], BF16, tag="y1buf")
        y0 = y0buf.rearrange("p (c q) -> p c q", c=4)
        y1 = y1buf.rearrange("p (c q) -> p c q", c=4)
        for k in range(4):
            nc.vector.memset(y0buf[:, k * (TB + 8):k * (TB + 8) + 8], 0.0)
            nc.vector.memset(y1buf[:, k * (TBP + 8):k * (TBP + 8) + 8], 0.0)
            nc.vector.memset(y1buf[:, k * (TBP + 8) + 8 + TB:(k + 1) * (TBP + 8)], 0.0)
        h16_0 = wpool.tile([128, 32], BF16, tag="h16_0")
        h16_1 = wpool.tile([128, 32], BF16, tag="h16_1")

        nembT = nembT_in.rearrange("p (k q) -> p k q", k=6)
        stage0 = [None] * nsub
        stage1 = [None] * nsub
        et_tiles = [None] * nsub

        def dma_et(s):
            et = etpool.tile([128, 6, 256], BF16, tag="et", name=f"et{s}")
            nc.sync.dma_start(out=et, in_=nembT[:, :, s * 256:(s + 1) * 256])
            et_tiles[s] = et

        def gi0_thunks(s):
            st = g0pool.tile([128, 32, 12, 8], BF16, tag="g0", name=f"g0_{s}")
            stage0[s] = st
            th = []
            et = et_tiles[s]
            for m in range(12):
                def mk(m=m):
                    ps = pApool.tile([128, 256], F32, tag="psA", name=f"psA_{s}_{m}")
                    for k in range(6):
                        nc.tensor.matmul(ps, Wih0T[k][:, m * 128:(m + 1) * 128],
                                         et[:, k, :], start=(k == 0), stop=(k == 5))
                    nc.scalar.activation(stage0[s][:, :, m, :], ps, AF.Identity,
                                         bias=bias0[:, m:m + 1])
                th.append(mk)
            return th

        def gi1_thunks(s):
            st = g1pool.tile([128, 32, 12, 8], BF16, tag="g1", name=f"g1_{s}")
            stage1[s] = st
            th = []
            for m in range(12):
                def mk(m=m):
                    ps = pApool.tile([128, 256], F32, tag="psA", name=f"psA1_{s}_{m}")
                    for k in range(4):
                        nc.tensor.matmul(ps, Wih1T[k][:, m * 128:(m + 1) * 128],
                                         y0[:, k, (32 * s + 1) * 8:(32 * s + 33) * 8],
                                         start=(k == 0), stop=(k == 3))
                    nc.scalar.activation(stage1[s][:, :, m, :], ps, AF.Identity,
                                         bias=bias1[:, m:m + 1])
                th.append(mk)
            return th

        dt_base = {3: 0, 4: 3, 5: 7}
        maccs = {}
        for fi, fs in enumerate(FS):
            for mt in range(2):
                macc = wpool.tile([128, 8], F32, tag=f"macc{fi}{mt}")
                nc.vector.memset(macc, -1e30)
                maccs[(fi, mt)] = macc

        def conv_thunks(c):
            th = []
            for fi, fs in enumerate(FS):
                for mt in range(2):
                    def mk(fi=fi, fs=fs, mt=mt):
                        ps = pBpool.tile([128, 256], F32, tag="psCV", name=f"psCV_{c}_{fi}_{mt}")
                        first = True
                        for dt in range(fs):
                            for k in range(4):
                                nc.tensor.matmul(
                                    ps, Wconv[:, dt_base[fs] + dt, k, mt * 128:(mt + 1) * 128],
                                    y1[:, k, (32 * c + dt + 1) * 8:(32 * c + dt + 33) * 8],
                                    start=first, stop=(dt == fs - 1 and k == 3))
                                first = False
                        nvalid = 32 if c < nsub - 1 else 33 - fs
                        cm = t0pool.tile([128, 8], F32, tag="cvcm", name=f"cvcm_{c}_{fi}_{mt}")
                        nc.vector.tensor_reduce(
                            out=cm, in_=ps.rearrange("p (t b) -> p b t", t=32)[:, :, 0:nvalid],
                            axis=mybir.AxisListType.X, op=ALU.max)
                        nc.vector.tensor_max(out=maccs[(fi, mt)], in0=maccs[(fi, mt)], in1=cm)
                    th.append(mk)
            return th

        def l_step(l, t, stage_l, WhT, ppool, tpool, y, h16):
            masked = t >= 32 * n_full
            s, j = t // 32, t % 32
            st = stage_l[s]
            if masked and t == 32 * n_full:
                nc.vector.tensor_copy(out=h16.rearrange("p (c b) -> p c b", c=4),
                                      in_=y[:, :, t * 8:(t + 1) * 8])
            if masked:
                hp = [h16[:, k * 8:(k + 1) * 8] for k in range(4)]
                hp3 = h16.rearrange("p (c b) -> p c b", c=4)
            else:
                hp = [y[:, k, t * 8:(t + 1) * 8] for k in range(4)]
                hp3 = y[:, :, t * 8:(t + 1) * 8]
            ps_all = ppool.tile([128, 96], F32, tag=f"psg{l}", name=f"psg{l}_{t}")
            ps_rz = ps_all[:, 0:64]
            ps_n = ps_all[:, 64:96]
            # single identity matmul: psum zero-region semantics require exactly
            # one start=True writer per region before the accumulates
            nc.tensor.matmul(ps_rz, identb, st[:, j, 0:8, :], start=True, stop=False)
            for m in range(8):
                for k in range(4):
                    nc.tensor.matmul(ps_rz[:, m * 8:(m + 1) * 8],
                                     WhT[k][:, m * 128:(m + 1) * 128], hp[k],
                                     start=False, stop=(k == 3))
            for m in range(4):
                for k in range(4):
                    nc.tensor.matmul(ps_n[:, m * 8:(m + 1) * 8],
                                     WhT[k][:, (8 + m) * 128:(9 + m) * 128], hp[k],
                                     start=(k == 0), stop=(k == 3))
            yield  # stage 0: PE emitted
            rz = tpool.tile([128, 64], F32, tag=f"rz{l}", name=f"rz{l}_{t}")
            nc.scalar.activation(rz, ps_rz, AF.Sigmoid)
            yield  # stage 1: sigmoid emitted
            t2 = tpool.tile([128, 32], F32, tag=f"t2{l}", name=f"t2{l}_{t}")
            nc.vector.tensor_mul(out=t2, in0=ps_n, in1=rz[:, 0:32])
            u = tpool.tile([128, 32], F32, tag=f"u{l}", name=f"u{l}_{t}")
            nc.vector.tensor_add(out=u.rearrange("p (c b) -> p c b", c=4),
                                 in0=t2.rearrange("p (c b) -> p c b", c=4),
                                 in1=st[:, j, 8:12, :])
            zc = tpool.tile([128, 32], F32, tag=f"zc{l}", name=f"zc{l}_{t}")
            nc.vector.tensor_scalar(out=zc, in0=rz[:, 32:64], scalar1=-1.0,
                                    scalar2=1.0, op0=ALU.mult, op1=ALU.add)
            zh = tpool.tile([128, 32], F32, tag=f"zh{l}", name=f"zh{l}_{t}")
            nc.vector.tensor_mul(out=zh.rearrange("p (c b) -> p c b", c=4),
                                 in0=rz[:, 32:64].rearrange("p (c b) -> p c b", c=4),
                                 in1=hp3)
            yield  # stage 2: critical DVE (t2,u) emitted
            nn = tpool.tile([128, 32], F32, tag=f"nn{l}", name=f"nn{l}_{t}")
            nc.scalar.activation(nn, u, AF.Tanh)
            yield  # stage 3: tanh emitted
            v = tpool.tile([128, 32], F32, tag=f"v{l}", name=f"v{l}_{t}")
            nc.vector.tensor_mul(out=v, in0=nn, in1=zc)
            if dump and l == 0 and t == 0:
                dbg = wpool.tile([128, 96 + 64 + 6 * 32], F32, tag="dbgs0")
                nc.vector.tensor_copy(out=dbg[:, 0:96], in_=ps_all)
                nc.vector.tensor_copy(out=dbg[:, 96:160], in_=rz)
                for i, x in enumerate((zc, zh, t2, u, nn, v)):
                    nc.vector.tensor_copy(out=dbg[:, 160 + i * 32:160 + (i + 1) * 32], in_=x)
                nc.sync.dma_start(out=s0d_out, in_=dbg)
            ynew = y[:, :, (t + 1) * 8:(t + 2) * 8]
            if not masked:
                nc.vector.tensor_add(out=ynew, in0=v.rearrange("p (c b) -> p c b", c=4),
                                     in1=zh.rearrange("p (c b) -> p c b", c=4))
            else:
                hn16 = tpool.tile([128, 32], BF16, tag=f"hn{l}", name=f"hn{l}_{t}")
                nc.vector.tensor_add(out=hn16, in0=v, in1=zh)
                hn3 = hn16.rearrange("p (c b) -> p c b", c=4)
                mview = maskf[:, t * 8:(t + 1) * 8].unsqueeze(1).broadcast_to([128, 4, 8])
                muview = masku[:, t * 8:(t + 1) * 8].unsqueeze(1).broadcast_to([128, 4, 8])
                nc.vector.tensor_mul(out=ynew, in0=hn3, in1=mview)
                nc.vector.copy_predicated(out=h16.rearrange("p (c b) -> p c b", c=4),
                                          mask=muview, data=hn3)

        # ---- pipeline emission
        dma_et(0)
        if nsub > 1:
            dma_et(1)
        for th in gi0_thunks(0):
            th()
        if dump:
            nc.sync.dma_start(out=g0d_out, in_=stage0[0])
        pending = []
        for s in range(nsub + 3):
            if s + 2 < nsub:
                dma_et(s + 2)
            if s + 1 < nsub:
                pending.extend(gi0_thunks(s + 1))
            if 0 <= s - 1 < nsub:
                pending.extend(gi1_thunks(s - 1))
            for j in range(32):
                if j == 6 and 0 <= s - 3 < nsub:
                    # conv chunk s-3 reads y1 of chunk s-2 steps 0..4, whose
                    # writes are emitted at j=0..4 of this tick
                    pending.extend(conv_thunks(s - 3))
                # drive both layers' steps stage-interleaved so the per-engine
                # FIFO order is [PE0,PE1][sig0,sig1][dve0,dve1][tanh0,tanh1]
                # [tail0,tail1] instead of serializing the two chains
                gens = []
                if s < nsub:
                    gens.append(l_step(0, 32 * s + j, stage0, WhT0, pC0, t0pool, y0, h16_0))
                if 0 <= s - 2 < nsub:
                    gens.append(l_step(1, 32 * (s - 2) + j, stage1, WhT1, pC1, t1pool, y1, h16_1))
                for _ in range(5):
                    for g in gens:
                        next(g, None)
                nrun = -(-len(pending) // (32 - j))
                for _ in range(min(nrun, len(pending))):
                    pending.pop(0)()

        # ---- epilogue: relu+bias pool, output linear
        pooled = wpool.tile([128, 48], F32, tag="pooled")
        for fi, fs in enumerate(FS):
            for mt in range(2):
                ci = fi * 2 + mt
                macc = maccs[(fi, mt)]
                if t_pad < T:
                    nc.vector.tensor_scalar_max(out=macc, in0=macc, scalar1=0.0)
                nc.scalar.activation(pooled[:, ci * 8:(ci + 1) * 8], macc, AF.Relu,
                                     bias=bconv[:, ci:ci + 1])
        ps_o_t = pApool.tile([128, 256], F32, tag="psA", name="ps_o_t")
        ps_o = ps_o_t[0:1, 0:8]
        for ci in range(6):
            nc.tensor.matmul(ps_o, WoT[:, ci:ci + 1], pooled[:, ci * 8:(ci + 1) * 8],
                             start=(ci == 0), stop=(ci == 5))
        ov = wpool.tile([1, BL], F32, tag="ov")
        nc.vector.tensor_scalar(out=ov, in0=ps_o, scalar1=bo_sb[0:1, 0:1], scalar2=None, op0=ALU.add)
        nc.sync.dma_start(out=out_dram, in_=ov)
        if dump:
            nc.sync.dma_start(out=y0d_out, in_=y0buf)
            nc.sync.dma_start(out=y1d_out, in_=y1buf)
    nc.compile()
    return nc


def build_neff2(t_pad):
    nc = bacc.Bacc("TRN2", target_bir_lowering=False, debug=False, num_devices=NC)
    TB = t_pad * BL
    NCH = t_pad // 64
    NIT2 = t_pad // UNROLL
    nembT_in = nc.dram_tensor("nembT", [128, 6 * TB], BF16, kind="ExternalInput").ap()
    mask_in = nc.dram_tensor("maskf", [128, TB], BF16, kind="ExternalInput").ap()
    masku_in = nc.dram_tensor("masku", [128, TB], mybir.dt.uint8, kind="ExternalInput").ap()
    Wih0T_in = nc.dram_tensor("Wih0T", [6, 128, 1536], BF16, kind="ExternalInput").ap()
    WhT0_in = nc.dram_tensor("WhT0", [4, 128, 1536], BF16, kind="ExternalInput").ap()
    Wih1T_in = nc.dram_tensor("Wih1T", [4, 128, 1536], BF16, kind="ExternalInput").ap()
    WhT1_in = nc.dram_tensor("WhT1", [4, 128, 1536], BF16, kind="ExternalInput").ap()
    bias0_in = nc.dram_tensor("bias0", [128, 12], F32, kind="ExternalInput").ap()
    bias1_in = nc.dram_tensor("bias1", [128, 12], F32, kind="ExternalInput").ap()
    Wconv_in = nc.dram_tensor("Wconv", [128, 12 * 4 * 256], BF16, kind="ExternalInput").ap()
    bconv_in = nc.dram_tensor("bconv", [128, 6], F32, kind="ExternalInput").ap()
    WoT_in = nc.dram_tensor("WoT", [128, 6], F32, kind="ExternalInput").ap()
    bo_in = nc.dram_tensor("bo", [1, 1], F32, kind="ExternalInput").ap()
    out_dram = nc.dram_tensor("out", [1, BL], F32, kind="ExternalOutput").ap()

    TPAD = t_pad + 16

    with TileContext(nc) as tc, ExitStack() as ctx:
        wpool = ctx.enter_context(tc.tile_pool(name="w2", bufs=1))
        dpool = ctx.enter_context(tc.tile_pool(name="dram2", bufs=1, space="DRAM"))
        gi0d = dpool.tile([128, NIT2 * UNROLL * 96], BF16, tag="gi0d")
        gi1d = dpool.tile([128, NIT2 * UNROLL * 96], BF16, tag="gi1d")

        def load_w(name, src, n, dtype=BF16):
            out = []
            for k in range(n):
                wt = wpool.tile([128, 1536], dtype, tag=f"{name}{k}")
                nc.sync.dma_start(out=wt, in_=src[k])
                out.append(wt)
            return out

        Wih0T = load_w("wih0", Wih0T_in, 6)
        WhT0 = load_w("wh0", WhT0_in, 4)
        Wih1T = load_w("wih1", Wih1T_in, 4)
        WhT1 = load_w("wh1", WhT1_in, 4)
        bias0 = wpool.tile([128, 12], F32, tag="bias0")
        nc.sync.dma_start(out=bias0, in_=bias0_in)
        bias1 = wpool.tile([128, 12], F32, tag="bias1")
        nc.sync.dma_start(out=bias1, in_=bias1_in)
        maskf = wpool.tile([128, TB], BF16, tag="maskf")
        nc.sync.dma_start(out=maskf, in_=mask_in)
        masku = wpool.tile([128, TB], mybir.dt.uint8, tag="masku")
        nc.sync.dma_start(out=masku, in_=masku_in)

        # --- gi0 = Wih0 @ nembT + bias0  (nembT streamed per chunk)
        with tc.tile_pool(name="nemb2", bufs=2) as npool, tc.tile_pool(name="st2", bufs=2) as stpool, tc.tile_pool(name="psg0", bufs=2, space="PSUM") as ppool:
            for nch in range(NCH):
                net = npool.tile([128, 6, 512], BF16, tag="net")
                nc.sync.dma_start(out=net, in_=nembT_in.rearrange("p (k c) -> p k c", k=6)[:, :, nch * 512:(nch + 1) * 512])
                stage = stpool.tile([128, 6144], BF16, tag="stage0")
                st4 = stage.rearrange("p (i j c b) -> p i j c b", i=I4, j=UNROLL, c=12)
                for m in range(12):
                    ps = ppool.tile([128, 512], F32, tag="ps_gi0")
                    for k in range(6):
                        nc.tensor.matmul(ps, Wih0T[k][:, m * 128:(m + 1) * 128],
                                         net[:, k, :],
                                         start=(k == 0), stop=(k == 5))
                    nc.vector.tensor_scalar(
                        out=st4[:, :, :, m, :],
                        in0=ps.rearrange("p (i j b) -> p i j b", i=I4, j=UNROLL),
                        scalar1=bias0[:, m:m + 1], scalar2=None, op0=ALU.add)
                nc.sync.dma_start(out=gi0d[:, nch * 6144:(nch + 1) * 6144], in_=stage)

        with tc.tile_pool(name="y0p", bufs=1) as y0pool:
            y0buf = y0pool.tile([128, 4 * TB], BF16, tag="y0buf")
            # --- L0 scan
            with ExitStack() as c0:
                emit_layer_scan(nc, tc, c0, "L0", WhT0, gi0d, maskf, masku, y0buf, TB, NIT2)

            # --- gi1 = Wih1 @ y0 + bias1
            y04 = y0buf.rearrange("p (c q) -> p c q", c=4)
            with tc.tile_pool(name="st3", bufs=2) as stpool, tc.tile_pool(name="psg1", bufs=2, space="PSUM") as ppool:
                for nch in range(NCH):
                    stage = stpool.tile([128, 6144], BF16, tag="stage1")
                    st4 = stage.rearrange("p (i j c b) -> p i j c b", i=I4, j=UNROLL, c=12)
                    for m in range(12):
                        ps = ppool.tile([128, 512], F32, tag="ps_gi1")
                        for k in range(4):
                            nc.tensor.matmul(ps, Wih1T[k][:, m * 128:(m + 1) * 128],
                                             y04[:, k, nch * 512:(nch + 1) * 512],
                                             start=(k == 0), stop=(k == 3))
                        nc.vector.tensor_scalar(
                            out=st4[:, :, :, m, :],
                            in0=ps.rearrange("p (i j b) -> p i j b", i=I4, j=UNROLL),
                            scalar1=bias1[:, m:m + 1], scalar2=None, op0=ALU.add)
                    nc.sync.dma_start(out=gi1d[:, nch * 6144:(nch + 1) * 6144], in_=stage)

        # --- L1 scan (padded y buffer for conv reads)
        y1buf = wpool.tile([128, 4 * TPAD * BL], BF16, tag="y1buf")
        nc.vector.memset(y1buf, 0.0)
        with ExitStack() as c1:
            emit_layer_scan(nc, tc, c1, "L1", WhT1, gi1d, maskf, masku, y1buf, TPAD * BL, NIT2)

        # --- convs + maxpool + relu + output linear
        Wconv_t = wpool.tile([128, 12 * 4 * 256], BF16, tag="Wconv")
        nc.sync.dma_start(out=Wconv_t, in_=Wconv_in)
        Wconv = Wconv_t.rearrange("p (d k c) -> p d k c", d=12, k=4)
        bconv = wpool.tile([128, 6], F32, tag="bconv")
        nc.sync.dma_start(out=bconv, in_=bconv_in)
        WoT = wpool.tile([128, 6], F32, tag="WoT")
        nc.sync.dma_start(out=WoT, in_=WoT_in)
        cpool = ctx.enter_context(tc.tile_pool(name="cv", bufs=2))
        ppool = ctx.enter_context(tc.tile_pool(name="pscv", bufs=2, space="PSUM"))
        pooled = wpool.tile([128, 48], F32, tag="pooled")
        y14 = y1buf.rearrange("p (c q) -> p c q", c=4)
        dt_base = {3: 0, 4: 3, 5: 7}
        for fi, fs in enumerate(FS):
            for mt in range(2):
                ci = fi * 2 + mt
                macc = cpool.tile([128, 8], F32, tag="macc")
                nc.vector.memset(macc, -1e30)
                for nch in range(NCH):
                    ps = ppool.tile([128, 512], F32, tag="ps_cv")
                    first = True
                    for dt in range(fs):
                        for k in range(4):
                            nc.tensor.matmul(
                                ps, Wconv[:, dt_base[fs] + dt, k, mt * 128:(mt + 1) * 128],
                                y14[:, k, nch * 512 + dt * 8: nch * 512 + dt * 8 + 512],
                                start=first, stop=(dt == fs - 1 and k == 3))
                            first = False
                    nvalid = 64 if nch < NCH - 1 else 65 - fs
                    cm = cpool.tile([128, 8], F32, tag="cm")
                    nc.vector.tensor_reduce(
                        out=cm, in_=ps.rearrange("p (t b) -> p b t", t=64)[:, :, 0:nvalid],
                        axis=mybir.AxisListType.X, op=ALU.max)
                    nc.vector.tensor_max(out=macc, in0=macc, in1=cm)
                if t_pad < T:
                    # windows beyond t_pad read all-zero y -> conv value exactly 0
                    nc.vector.tensor_scalar_max(out=macc, in0=macc, scalar1=0.0)
                nc.scalar.activation(pooled[:, ci * 8:(ci + 1) * 8], macc, AF.Relu,
                                     bias=bconv[:, ci:ci + 1])
        ps_o_t = ppool.tile([128, 8], F32, tag="ps_o")
        ps_o = ps_o_t[0:1, :]
        for ci in range(6):
            nc.tensor.matmul(ps_o, WoT[:, ci:ci + 1], pooled[:, ci * 8:(ci + 1) * 8],
                             start=(ci == 0), stop=(ci == 5))
        bo_sb = wpool.tile([1, 1], F32, tag="bo_sb")
        nc.sync.dma_start(out=bo_sb, in_=bo_in)
        ov = wpool.tile([1, BL], F32, tag="ov")
        nc.vector.tensor_scalar(out=ov, in0=ps_o, scalar1=bo_sb[0:1, 0:1], scalar2=None, op0=ALU.add)
        nc.sync.dma_start(out=out_dram, in_=ov)
    nc.compile()
    return nc


def _make_runner(nc, n_cores):
    import jax
    from jax.sharding import Mesh, PartitionSpec
    from jax.experimental.shard_map import shard_map
    import concourse.bass2jax as b2j
    b2j.install_neuronx_cc_hook()
    pname = nc.partition_id_tensor.name if nc.partition_id_tensor else None
    in_names, out_names, out_avals, zero_outs = [], [], [], []
    for alloc in nc.m.functions[0].allocations:
        if not isinstance(alloc, mybir.MemoryLocationSet):
            continue
        name = alloc.memorylocations[0].name
        if alloc.kind == "ExternalInput":
            if name != pname:
                in_names.append(name)
        elif alloc.kind == "ExternalOutput":
            out_names.append(name)
            shape = tuple(alloc.tensor_shape)
            dtype = mybir.dt.np(alloc.dtype)
            out_avals.append(jax.core.ShapedArray(shape, dtype))
            zero_outs.append(np.zeros(shape, dtype))
    n_params, n_outs = len(in_names), len(out_avals)
    all_in = list(in_names) + list(out_names) + ([pname] if pname else [])
    donate = tuple(range(n_params, n_params + n_outs))

    def _body(*args):
        operands = list(args)
        if pname is not None:
            operands.append(b2j.partition_id_tensor())
        outs = b2j._bass_exec_p.bind(
            *operands, out_avals=tuple(out_avals), in_names=tuple(all_in),
            out_names=tuple(out_names), lowering_input_output_aliases=(),
            sim_require_finite=True, sim_require_nnan=True, nc=nc)
        return tuple(outs)

    mesh = Mesh(np.asarray(jax.devices()[:n_cores]), ("core",))
    fn = jax.jit(shard_map(_body, mesh=mesh,
                           in_specs=(PartitionSpec("core"),) * (n_params + n_outs),
                           out_specs=(PartitionSpec("core"),) * n_outs, check_rep=False),
                 donate_argnums=donate, keep_unused=True)

    def run(in_maps):
        import jax
        per_core = [[np.asarray(m[name]) for name in in_names] for m in in_maps]
        concat_in = [np.concatenate([per_core[c][i] for c in range(n_cores)], axis=0)
                     for i in range(n_params)]
        zeros = [np.zeros((n_cores * z.shape[0], *z.shape[1:]), z.dtype) for z in zero_outs]
        out_arrs = fn(*concat_in, *zeros)
        jax.block_until_ready(out_arrs)
        return [{name: np.asarray(out_arrs[i]).reshape(n_cores, *out_avals[i].shape)[c]
                 for i, name in enumerate(out_names)} for c in range(n_cores)]

    def bench(in_maps, iters=10, slope=True):
        """Median wall time per call with device-resident inputs (s)."""
        import jax, time
        from jax.sharding import NamedSharding, PartitionSpec
        if getattr(bench, "_key", None) is id(in_maps):
            dev_in = bench._dev_in
        else:
            per_core = [[np.asarray(m[name]) for name in in_names] for m in in_maps]
            concat_in = [np.concatenate([per_core[c][i] for c in range(n_cores)], axis=0)
                         for i in range(n_params)]
            sh = NamedSharding(mesh, PartitionSpec("core"))
            dev_in = [jax.device_put(x, sh) for x in concat_in]
            jax.block_until_ready(dev_in)
            bench._key, bench._dev_in = id(in_maps), dev_in
        zeros = [np.zeros((n_cores * z.shape[0], *z.shape[1:]), z.dtype) for z in zero_outs]
        ts = []
        for i in range(iters):
            zs = [z.copy() for z in zeros]
            t0 = time.perf_counter()
            out = fn(*dev_in, *zs)
            jax.block_until_ready(out)
            ts.append(time.perf_counter() - t0)
        ts.sort()
        sl = 0.0
        if slope:
            # async slope: queue K calls, block once; removes dispatch latency
            for K_ in (2, 6):
                zss = [[z.copy() for z in zeros] for _ in range(K_)]
                t0 = time.perf_counter()
                outs = [fn(*dev_in, *zss[k]) for k in range(K_)]
                jax.block_until_ready(outs)
                tA = (time.perf_counter() - t0) if K_ == 2 else tA
                tB = (time.perf_counter() - t0) if K_ == 6 else 0.0
            sl = (tB - tA) / 4.0
        return ts[len(ts) // 2], {"sync": ts, "slope": sl}

    run.bench = bench
    return run


# ------------------------------------------------------------- host glue ----

_cache = {}


def _get_run1():
    if "r1" not in _cache:
        _cache["r1"] = _make_runner(build_neff1(), NC)
    return _cache["r1"]


def _prep_in1(emb, Wih_c, Whh_c, bih_c, bhh_c, Ws):
    f32, f16 = np.float32, np.float16
    WihcT = _fold_gates_T(Wih_c)
    WihH = WihcT.astype(f16)
    WihL = (WihcT - WihH.astype(f32)).astype(f16)
    WTc = _fold_gates_T(Whh_c).astype(f16)
    wd = Ws[1] - Ws[0]
    wd_hi = wd.astype(f16).astype(f32)
    wd_lo = (wd - wd_hi).astype(f16)
    wdP = np.zeros((8, 128, 128), f16)
    for k in range(4):
        wdP[2 * k, :, 0] = wd_hi[k * 128:(k + 1) * 128].astype(f16)
        wdP[2 * k + 1, :, 0] = wd_lo[k * 128:(k + 1) * 128]
    biasC = np.zeros((128, 12), f32)
    bsum = bih_c + bhh_c
    for m in range(12):
        biasC[:, m] = bsum[m * 128:(m + 1) * 128] if m < 8 else bih_c[m * 128:(m + 1) * 128]
    assert np.abs(bhh_c[1024:]).max() == 0.0, "nonzero bhh_c n-gate bias unsupported"
    in1 = []
    for c in range(NC):
        es = emb[c * BL:(c + 1) * BL]                 # [8, T, E]
        embT = np.ascontiguousarray(
            es.reshape(BL, T, 6, 128).transpose(3, 2, 1, 0)).reshape(128, 6 * T * BL)
        embH = embT.astype(f16)
        embL = (embT - embH.astype(f32)).astype(f16)
        in1.append({"embH": embH, "embL": embL, "WihH": WihH, "WihL": WihL,
                    "WTc": WTc, "wdP": wdP, "biasC": biasC})
    return in1


def _get_run2(t_pad, n_full):
    key = ("r2", t_pad, n_full)
    if key not in _cache:
        _cache[key] = _make_runner(build_neff2_v2(t_pad, n_full), NC)
    return _cache[key]


def _fold_gates_T(W):
    # W: [1536, K] -> [K/128, 128, 1536] lhsT tiles (W.T folded)
    K = W.shape[1]
    return np.ascontiguousarray(W.T.reshape(K // 128, 128, 1536))


def kernel(**inputs):
    emb = np.asarray(inputs["embedded"], np.float32)
    mask = np.asarray(inputs["mask"])
    lens = mask.sum(axis=1).astype(np.int64)
    f32 = np.float32
    Wih_c, Whh_c = np.asarray(inputs["Wih_c"], f32), np.asarray(inputs["Whh_c"], f32)
    bih_c, bhh_c = np.asarray(inputs["bih_c"], f32), np.asarray(inputs["bhh_c"], f32)
    Ws, bs = np.asarray(inputs["Ws"], f32), np.asarray(inputs["bs"], f32)
    Wih0, Whh0 = np.asarray(inputs["Wih0"], f32), np.asarray(inputs["Whh0"], f32)
    bih0, bhh0 = np.asarray(inputs["bih0"], f32), np.asarray(inputs["bhh0"], f32)
    Wih1, Whh1 = np.asarray(inputs["Wih1"], f32), np.asarray(inputs["Whh1"], f32)
    bih1, bhh1 = np.asarray(inputs["bih1"], f32), np.asarray(inputs["bhh1"], f32)
    Wc = {3: np.asarray(inputs["Wc3"], f32), 4: np.asarray(inputs["Wc4"], f32),
          5: np.asarray(inputs["Wc5"], f32)}
    bc = {3: np.asarray(inputs["bc3"], f32), 4: np.asarray(inputs["bc4"], f32),
          5: np.asarray(inputs["bc5"], f32)}
    Wo, bo = np.asarray(inputs["Wo"], f32), np.asarray(inputs["bo"], f32)

    run1 = _get_run1()
    in1 = _prep_in1(emb, Wih_c, Whh_c, bih_c, bhh_c, Ws)
    _cache["in1"] = in1
    res1 = run1(in1)
    margins = np.concatenate([r["margins"].reshape(T, BL).T[None] for r in res1], 0)
    margins = margins.reshape(NC * BL, T)                   # [B, T] (b-major per core)

    # ---- host compaction (bit logic + gather, zero FLOPs)
    sel, order, valid, t_pad, n_full = _compaction(margins, bs, lens)
    run2 = _get_run2(t_pad, n_full)
    in2 = _prep_in2(emb, order, valid, t_pad,
                    Wih0, Whh0, bih0, bhh0, Wih1, Whh1, bih1, bhh1,
                    Wc, bc, Wo, bo)
    _cache["in2"], _cache["last_r2"] = in2, run2
    res2 = run2(in2)
    out = np.concatenate([r["out"].reshape(BL) for r in res2], 0)
    return out.astype(np.float32)


def _build_floor():
    nc = bacc.Bacc("TRN2", target_bir_lowering=False, debug=False, num_devices=NC)
    x_in = nc.dram_tensor("x", [128, 8], F32, kind="ExternalInput").ap()
    y_out = nc.dram_tensor("y", [128, 8], F32, kind="ExternalOutput").ap()
    with TileContext(nc) as tc, ExitStack() as ctx:
        wp = ctx.enter_context(tc.tile_pool(name="w", bufs=1))
        xt = wp.tile([128, 8], F32, tag="x")
        nc.sync.dma_start(out=xt, in_=x_in)
        nc.sync.dma_start(out=y_out, in_=xt)
    nc.compile()
    return nc


def bench_hw(rounds=5, per=9):
    """Honest device-time estimate. Alternating between different NEFFs pays a
    ~40ms program-swap per call, so each NEFF is timed in a consecutive block
    (block median ignores the first-call swap); floor blocks bracket each NEFF
    block and the per-round delta uses the adjacent floor block, cancelling
    dispatch drift. Returns ns."""
    runf = _make_runner(_build_floor(), NC)
    inf = [{"x": np.zeros((128, 8), np.float32)} for _ in range(NC)]
    runf(inf)
    d1, d2 = [], []
    for r in range(rounds):
        tf1, _ = runf.bench(inf, iters=per, slope=False)
        t1, _ = _cache["r1"].bench(_cache["in1"], iters=per, slope=False)
        tf2, _ = runf.bench(inf, iters=per, slope=False)
        t2, _ = _cache["last_r2"].bench(_cache["in2"], iters=per, slope=False)
        d1.append(t1 - min(tf1, tf2))
        d2.append(t2 - tf2)
    d1.sort(); d2.sort()
    n1 = max(0.0, d1[len(d1) // 2]) * 1e9
    n2 = max(0.0, d2[len(d2) // 2]) * 1e9
    return {"neff1_ns": n1, "neff2_ns": n2, "total_ns": n1 + n2,
            "d1_ms": [round(x * 1e3, 2) for x in d1],
            "d2_ms": [round(x * 1e3, 2) for x in d2]}


def _compaction(margins, bs, lens):
    thr = bs[0] - bs[1]
    sel = (margins > thr).astype(np.int64)
    t_idx = np.arange(T)[None, :]
    sel[:, 0] = 1
    sel[np.arange(B), lens - 1] = 1
    sel = np.where(t_idx >= lens[:, None], 0, sel)
    nsel = sel.sum(1)
    order = np.argsort(1 - sel, axis=1, kind="stable")
    valid = t_idx < nsel[:, None]
    t_pad = min(T, max(32, int(-(-int(nsel.max()) // 32) * 32)))
    n_full = min(t_pad // 32, int(nsel.min()) // 32)
    return sel, order, valid, t_pad, n_full


def _prep_in2(emb, order, valid, t_pad, Wih0, Whh0, bih0, bhh0,
              Wih1, Whh1, bih1, bhh1, Wc, bc, Wo, bo):
    f32 = np.float32
    Wih0T = _fold_gates_T(Wih0).astype(ml_dtypes.bfloat16)
    WhT0 = _fold_gates_T(Whh0).astype(ml_dtypes.bfloat16)
    Wih1T = _fold_gates_T(Wih1).astype(ml_dtypes.bfloat16)
    WhT1 = _fold_gates_T(Whh1).astype(ml_dtypes.bfloat16)
    bias0 = np.zeros((128, 12), f32)
    b0sum = bih0 + bhh0
    for m in range(12):
        bias0[:, m] = b0sum[m * 128:(m + 1) * 128] if m < 8 else bih0[m * 128:(m + 1) * 128]
    assert np.abs(bhh0[1024:]).max() == 0.0 and np.abs(bhh1[1024:]).max() == 0.0
    bias1 = np.zeros((128, 12), f32)
    b1sum = bih1 + bhh1
    for m in range(12):
        bias1[:, m] = b1sum[m * 128:(m + 1) * 128] if m < 8 else bih1[m * 128:(m + 1) * 128]
    Wconv = np.zeros((12, 4, 128, 256), f32)
    dt_base = {3: 0, 4: 3, 5: 7}
    for fs in FS:
        Wf = Wc[fs][:, 0]                                   # [NF, fs, H]
        for dt in range(fs):
            for k in range(4):
                Wconv[dt_base[fs] + dt, k] = Wf[:, dt, k * 128:(k + 1) * 128].T
    Wconv = np.ascontiguousarray(Wconv.transpose(2, 0, 1, 3)).reshape(128, -1).astype(ml_dtypes.bfloat16)
    bconv = np.zeros((128, 6), f32)
    WoT = np.zeros((128, 6), f32)
    for fi, fs in enumerate(FS):
        for mt in range(2):
            bconv[:, fi * 2 + mt] = bc[fs][mt * 128:(mt + 1) * 128]
            WoT[:, fi * 2 + mt] = Wo[0, fi * 256 + mt * 128: fi * 256 + (mt + 1) * 128]

    in2 = []
    for c in range(NC):
        bsl = slice(c * BL, (c + 1) * BL)
        new_emb = np.take_along_axis(emb[bsl], order[bsl][:, :, None], axis=1)
        new_emb = (new_emb * valid[bsl][:, :, None])[:, :t_pad]
        nembT = np.ascontiguousarray(
            new_emb.reshape(BL, t_pad, 6, 128).transpose(3, 2, 1, 0)
        ).reshape(128, 6 * t_pad * BL).astype(ml_dtypes.bfloat16)
        vs = valid[bsl][:, :t_pad]
        maskf = np.ascontiguousarray(np.broadcast_to(
            vs.T.reshape(1, t_pad * BL), (128, t_pad * BL))).astype(ml_dtypes.bfloat16)
        masku = np.ascontiguousarray(np.broadcast_to(
            vs.T.reshape(1, t_pad * BL), (128, t_pad * BL))).astype(np.uint8)
        in2.append({"nembT": nembT, "maskf": maskf, "masku": masku, "Wih0T": Wih0T, "WhT0": WhT0,
                    "Wih1T": Wih1T, "WhT1": WhT1, "bias0": bias0, "bias1": bias1,
                    "identb": np.eye(128, dtype=ml_dtypes.bfloat16),
                    "Wconv": Wconv, "bconv": bconv, "WoT": WoT,
                    "bo": bo.reshape(1, 1)})
    return in2



# revision 15
# speedup vs baseline: 1.1415x; 1.1415x over previous
"""Trainium2 Bass kernel for nn_CNN_RNN (select-GRU -> compact -> 2xGRU -> KimCNN).

Sharding: pure data-parallel, batch 64 -> 8 cores x 8.
Device NEFF1: select-gate input projection (fp32) + fp16-compensated select GRU scan
  -> per-(t,b) argmax margins.
Host: argmax bits -> stable-compaction gather indices (pure data movement) -> gathered
  embedding fold (bf16).
Device NEFF2: layer input projections (bf16), two masked GRU scans (bf16,
  weight-stationary), Kim-CNN convs as shifted matmuls, max-pool+relu, output linear.
NEFF2 is specialized (and cached) per runtime T_pad = ceil(max(nsel)/64)*64: the layer
scans, projections, and convs only run to the longest compacted sequence; skipped conv
windows are all-zero and reintroduced exactly via a max-with-0 before the relu+bias.
GRU tails use h' = n*(1-z) + z*h with (1-z) and z*h computed during the PE's n-gate
matmuls, keeping the serial post-matmul chain one op shorter.
"""
import numpy as np
import ml_dtypes

import concourse.bass as bass
import concourse.mybir as mybir
from concourse import bacc
from concourse.tile import TileContext
from contextlib import ExitStack

F32, F16, BF16 = mybir.dt.float32, mybir.dt.float16, mybir.dt.bfloat16
AF = mybir.ActivationFunctionType
ALU = mybir.AluOpType
PE, DVE, ACT = mybir.EngineType.PE, mybir.EngineType.DVE, mybir.EngineType.Activation

B, T, E, H, NF = 64, 512, 768, 512, 256
FS = (3, 4, 5)
NC = 8
BL = B // NC          # batch per core
UNROLL = 32
NITER = T // UNROLL
I4 = 64 // UNROLL   # iters per 64-t chunk


# ---------------------------------------------------------------- NEFF1 ----

def build_neff1():
    """Select scan, fully unrolled, with the f16 3-product input projection
    (phase A) sprinkled into PE idle slots between scan steps; gi stages live
    in SBUF (no DRAM roundtrip). Step math is the proven fp16-compensated
    baseline scheme (W f16, h split hi/lo f16 in the matmul free dim)."""
    nc = bacc.Bacc("TRN2", target_bir_lowering=False, debug=False, num_devices=NC)
    TBL = T * BL
    embH_in = nc.dram_tensor("embH", [128, 6 * TBL], F16, kind="ExternalInput").ap()
    embL_in = nc.dram_tensor("embL", [128, 6 * TBL], F16, kind="ExternalInput").ap()
    WihH_in = nc.dram_tensor("WihH", [6, 128, 1536], F16, kind="ExternalInput").ap()
    WihL_in = nc.dram_tensor("WihL", [6, 128, 1536], F16, kind="ExternalInput").ap()
    WTc_in = nc.dram_tensor("WTc", [4, 128, 1536], F16, kind="ExternalInput").ap()
    wdP_in = nc.dram_tensor("wdP", [8, 128, 128], F16, kind="ExternalInput").ap()
    biasC_in = nc.dram_tensor("biasC", [128, 12], F32, kind="ExternalInput").ap()
    margins_out = nc.dram_tensor("margins", [TBL], F32, kind="ExternalOutput").ap()
    NS = T // 32

    with TileContext(nc) as tc, ExitStack() as ctx:
        wpool = ctx.enter_context(tc.tile_pool(name="w1", bufs=1))
        etpool = ctx.enter_context(tc.tile_pool(name="et1", bufs=3))
        stpool = ctx.enter_context(tc.tile_pool(name="st1", bufs=3))
        pApool = ctx.enter_context(tc.tile_pool(name="psA1", bufs=2, space="PSUM"))
        ppoolB = ctx.enter_context(tc.tile_pool(name="psB", bufs=2, space="PSUM"))
        tpool = ctx.enter_context(tc.tile_pool(name="seltmp", bufs=3))

        WihH, WihL = [], []
        for k in range(6):
            wh = wpool.tile([128, 1536], F16, tag=f"wihH{k}", name=f"wihH{k}")
            nc.sync.dma_start(out=wh, in_=WihH_in[k])
            WihH.append(wh)
            wl = wpool.tile([128, 1536], F16, tag=f"wihL{k}", name=f"wihL{k}")
            nc.sync.dma_start(out=wl, in_=WihL_in[k])
            WihL.append(wl)
        WTc = []
        for k in range(4):
            wt = wpool.tile([128, 1536], F16, tag=f"wtc{k}", name=f"wtc{k}")
            nc.sync.dma_start(out=wt, in_=WTc_in[k])
            WTc.append(wt)
        wdP = []
        for i in range(8):
            wt = wpool.tile([128, 128], F16, tag=f"wdP{i}", name=f"wdP{i}")
            nc.sync.dma_start(out=wt, in_=wdP_in[i])
            wdP.append(wt)
        biasC = wpool.tile([128, 12], F32, tag="biasC")
        nc.sync.dma_start(out=biasC, in_=biasC_in)

        hT = wpool.tile([128, 32], F32, tag="selhT")
        hpk = wpool.tile([128, 64], F16, tag="selhpk")
        nc.vector.memset(hT, 0.0)
        nc.vector.memset(hpk, 0.0)
        marg = wpool.tile([1, TBL], F32, tag="marg")

        embH = embH_in.rearrange("p (k q) -> p k q", k=6)
        embL = embL_in.rearrange("p (k q) -> p k q", k=6)
        stages = [None] * NS
        et_h = [None] * NS
        et_l = [None] * NS

        def dma_et(s):
            eh = etpool.tile([128, 6, 256], F16, tag="eth", name=f"eth{s}")
            nc.sync.dma_start(out=eh, in_=embH[:, :, s * 256:(s + 1) * 256])
            el = etpool.tile([128, 6, 256], F16, tag="etl", name=f"etl{s}")
            nc.sync.dma_start(out=el, in_=embL[:, :, s * 256:(s + 1) * 256])
            et_h[s], et_l[s] = eh, el

        def pa_thunks(s):
            st = stpool.tile([128, 32, 12, 8], F32, tag="stage", name=f"stage{s}")
            stages[s] = st
            th = []
            prods = [(WihH, et_h[s]), (WihH, et_l[s]), (WihL, et_h[s])]
            for m in range(12):
                ps_box = []
                def mk_mm(m, p, ps_box):
                    if p == 0:
                        ps_box.append(pApool.tile([128, 256], F32, tag="psA",
                                                  name=f"psA1_{s}_{m}"))
                    W6, et = prods[p]
                    for k in range(6):
                        nc.tensor.matmul(ps_box[0], W6[k][:, m * 128:(m + 1) * 128],
                                         et[:, k, :], start=(p == 0 and k == 0),
                                         stop=(p == 2 and k == 5))
                def mk_act(m=m, ps_box=ps_box):
                    nc.scalar.activation(stages[s][:, :, m, :], ps_box[0], AF.Identity,
                                         bias=biasC[:, m:m + 1])
                for pi in range(3):
                    th.append((lambda m=m, p=pi, ps_box=ps_box: mk_mm(m, p, ps_box)))
                th.append(mk_act)
            return th

        dma_et(0)
        dma_et(1)
        pending = []
        for th in pa_thunks(0):
            th()
        dma_et(2)
        pending.extend(pa_thunks(1))

        for t in range(T):
            s, j = divmod(t, 32)
            if j == 0 and s >= 1:
                if s + 2 < NS:
                    dma_et(s + 2)
                if s + 1 < NS:
                    pending.extend(pa_thunks(s + 1))
            st = stages[s]
            ps_rz = ppoolB.tile([128, 128], F32, tag="ps_rz", name=f"psrz_{t}")
            ps_n = ppoolB.tile([128, 64], F32, tag="ps_n", name=f"psn_{t}")
            ps_m_t = ppoolB.tile([128, 16], F32, tag="ps_m", name=f"psm_{t}")
            for m in range(12):
                ps = ps_rz[:, m * 16:(m + 1) * 16] if m < 8 else ps_n[:, (m - 8) * 16:(m - 7) * 16]
                for k in range(4):
                    nc.tensor.matmul(ps, WTc[k][:, m * 128:(m + 1) * 128],
                                     hpk[:, k * 16:(k + 1) * 16],
                                     start=(k == 0), stop=(k == 3))
            rz_hi = ps_rz.rearrange("p (m s) -> p m s", s=16)[:, :, 0:8]
            rz_lo = ps_rz.rearrange("p (m s) -> p m s", s=16)[:, :, 8:16]
            a = tpool.tile([128, 64], F32, tag="a", name=f"a_{t}")
            nc.vector.tensor_add(out=a.rearrange("p (m s) -> p m s", s=8), in0=rz_hi,
                                 in1=st[:, j, 0:8, :])
            a2 = tpool.tile([128, 64], F32, tag="a2", name=f"a2_{t}")
            nc.vector.tensor_add(out=a2.rearrange("p (m s) -> p m s", s=8),
                                 in0=a.rearrange("p (m s) -> p m s", s=8), in1=rz_lo)
            rz = tpool.tile([128, 64], F32, tag="rz", name=f"rz_{t}")
            nc.scalar.activation(rz, a2, AF.Sigmoid)
            zc = tpool.tile([128, 32], F32, tag="zc", name=f"zc_{t}")
            nc.vector.tensor_scalar(out=zc, in0=rz[:, 32:64], scalar1=-1.0,
                                    scalar2=1.0, op0=ALU.mult, op1=ALU.add)
            zh = tpool.tile([128, 32], F32, tag="zh", name=f"zh_{t}")
            nc.vector.tensor_mul(out=zh, in0=hT, in1=rz[:, 32:64])
            n_hi = ps_n.rearrange("p (m s) -> p m s", s=16)[:, :, 0:8]
            n_lo = ps_n.rearrange("p (m s) -> p m s", s=16)[:, :, 8:16]
            t2a = tpool.tile([128, 32], F32, tag="t2a", name=f"t2a_{t}")
            nc.vector.tensor_mul(out=t2a.rearrange("p (m s) -> p m s", s=8), in0=n_hi,
                                 in1=rz[:, 0:32].rearrange("p (m s) -> p m s", s=8))
            t2b = tpool.tile([128, 32], F32, tag="t2b", name=f"t2b_{t}")
            nc.vector.tensor_mul(out=t2b.rearrange("p (m s) -> p m s", s=8), in0=n_lo,
                                 in1=rz[:, 0:32].rearrange("p (m s) -> p m s", s=8))
            u1 = tpool.tile([128, 32], F32, tag="u1", name=f"u1_{t}")
            nc.vector.tensor_add(out=u1.rearrange("p (m s) -> p m s", s=8),
                                 in0=t2a.rearrange("p (m s) -> p m s", s=8),
                                 in1=st[:, j, 8:12, :])
            u = tpool.tile([128, 32], F32, tag="u", name=f"u_{t}")
            nc.vector.tensor_add(out=u, in0=u1, in1=t2b)
            nn_ = tpool.tile([128, 32], F32, tag="nn_", name=f"nn_{t}")
            nc.scalar.activation(nn_, u, AF.Tanh)
            v = tpool.tile([128, 32], F32, tag="v", name=f"v_{t}")
            nc.vector.tensor_mul(out=v, in0=nn_, in1=zc)
            nc.vector.tensor_add(out=hT, in0=v, in1=zh)
            hpk3 = hpk.rearrange("p (k s) -> p k s", s=16)
            hT3 = hT.rearrange("p (k s) -> p k s", s=8)
            nc.vector.tensor_copy(out=hpk3[:, :, 0:8], in_=hT3)
            nc.vector.tensor_sub(out=hpk3[:, :, 8:16], in0=hT3, in1=hpk3[:, :, 0:8])
            for k in range(4):
                nc.tensor.matmul(ps_m_t, wdP[k * 2], hpk[:, k * 16:(k + 1) * 16],
                                 start=(k == 0), stop=False)
                nc.tensor.matmul(ps_m_t, wdP[k * 2 + 1], hpk[:, k * 16:(k + 1) * 16],
                                 start=False, stop=(k == 3))
            mc = tpool.tile([1, 8], F32, tag="mc", name=f"mc_{t}")
            nc.vector.tensor_copy(out=mc, in_=ps_m_t[0:1, 0:8])
            nc.vector.tensor_add(out=marg[:, t * 8:(t + 1) * 8], in0=mc,
                                 in1=ps_m_t[0:1, 8:16])
            nrun = -(-len(pending) // (32 - j))
            for _ in range(min(nrun, len(pending))):
                pending.pop(0)()
            if t % 64 == 63:
                nc.sync.dma_start(out=margins_out[bass.ds((t - 63) * 8, 512)],
                                  in_=marg[0:1, (t - 63) * 8:(t + 1) * 8])
    nc.compile()
    return nc


def build_neff1_v2():
    """Select-policy GRU scan, fully unrolled (512 steps), with the gi_c
    input projection (f16 hi/lo 3-product, ~fp32-exact) sprinkled into PE
    idle slots between scan steps. gi_rz is accumulated into the gate PSUM
    via identity matmuls; zc/zh run on GPSIMD; margins (wd . h) are 4 fp32
    matmuls per step + an ACT copy. h stays fp32 (select bits must be exact:
    ~20 flipped bits already cost 1.5e-2 rel err downstream)."""
    nc = bacc.Bacc("TRN2", target_bir_lowering=False, debug=False, num_devices=NC)
    TBL = T * BL
    embH_in = nc.dram_tensor("embH", [128, 6 * TBL], F16, kind="ExternalInput").ap()
    embL_in = nc.dram_tensor("embL", [128, 6 * TBL], F16, kind="ExternalInput").ap()
    WihH_in = nc.dram_tensor("WihH", [6, 128, 1536], F16, kind="ExternalInput").ap()
    WihL_in = nc.dram_tensor("WihL", [6, 128, 1536], F16, kind="ExternalInput").ap()
    WTc_in = nc.dram_tensor("WTc", [4, 128, 1536], F32, kind="ExternalInput").ap()
    wdT_in = nc.dram_tensor("wdT", [128, 4], F32, kind="ExternalInput").ap()
    identf_in = nc.dram_tensor("identf", [128, 128], F32, kind="ExternalInput").ap()
    biasC_in = nc.dram_tensor("biasC", [128, 12], F32, kind="ExternalInput").ap()
    margins_out = nc.dram_tensor("margins", [TBL], F32, kind="ExternalOutput").ap()

    NS = T // 32  # 16 sub-chunks

    with TileContext(nc) as tc, ExitStack() as ctx:
        wpool = ctx.enter_context(tc.tile_pool(name="w1", bufs=1))
        etpool = ctx.enter_context(tc.tile_pool(name="et1", bufs=3))
        stpool = ctx.enter_context(tc.tile_pool(name="st1", bufs=3))
        pApool = ctx.enter_context(tc.tile_pool(name="psA1", bufs=2, space="PSUM"))
        pSpool = ctx.enter_context(tc.tile_pool(name="psS1", bufs=2, space="PSUM"))
        tpool = ctx.enter_context(tc.tile_pool(name="tmp1s", bufs=3))

        WihH, WihL = [], []
        for k in range(6):
            wh = wpool.tile([128, 1536], F16, tag=f"wihH{k}", name=f"wihH{k}")
            nc.sync.dma_start(out=wh, in_=WihH_in[k])
            WihH.append(wh)
            wl = wpool.tile([128, 1536], F16, tag=f"wihL{k}", name=f"wihL{k}")
            nc.sync.dma_start(out=wl, in_=WihL_in[k])
            WihL.append(wl)
        WTc = []
        for k in range(4):
            wt = wpool.tile([128, 1536], F32, tag=f"wtc{k}", name=f"wtc{k}")
            nc.sync.dma_start(out=wt, in_=WTc_in[k])
            WTc.append(wt)
        wdT = wpool.tile([128, 4], F32, tag="wdT")
        nc.sync.dma_start(out=wdT, in_=wdT_in)
        identf = wpool.tile([128, 128], F32, tag="identf")
        nc.sync.dma_start(out=identf, in_=identf_in)
        biasC = wpool.tile([128, 12], F32, tag="biasC")
        nc.sync.dma_start(out=biasC, in_=biasC_in)

        hT = wpool.tile([128, 32], F32, tag="hT")
        nc.vector.memset(hT, 0.0)
        marg = wpool.tile([1, TBL], F32, tag="marg")

        embH = embH_in.rearrange("p (k q) -> p k q", k=6)
        embL = embL_in.rearrange("p (k q) -> p k q", k=6)
        stages = [None] * NS
        et_h = [None] * NS
        et_l = [None] * NS

        def dma_et(s):
            eh = etpool.tile([128, 6, 256], F16, tag="eth", name=f"eth{s}")
            nc.sync.dma_start(out=eh, in_=embH[:, :, s * 256:(s + 1) * 256])
            el = etpool.tile([128, 6, 256], F16, tag="etl", name=f"etl{s}")
            nc.sync.dma_start(out=el, in_=embL[:, :, s * 256:(s + 1) * 256])
            et_h[s], et_l[s] = eh, el

        def pa_thunks(s):
            st = stpool.tile([128, 32, 12, 8], F32, tag="stage", name=f"stage{s}")
            stages[s] = st
            th = []
            prods = [(WihH, et_h[s]), (WihH, et_l[s]), (WihL, et_h[s])]
            for m in range(12):
                ps_box = []
                def mk_mm(m=m, p=0, ps_box=ps_box):
                    if p == 0:
                        ps_box.append(pApool.tile([128, 256], F32, tag="psA",
                                                  name=f"psA1_{s}_{m}"))
                    W6, et = prods[p]
                    for k in range(6):
                        nc.tensor.matmul(ps_box[0], W6[k][:, m * 128:(m + 1) * 128],
                                         et[:, k, :], start=(p == 0 and k == 0),
                                         stop=(p == 2 and k == 5))
                def mk_act(m=m, ps_box=ps_box):
                    nc.scalar.activation(stages[s][:, :, m, :], ps_box[0], AF.Identity,
                                         bias=biasC[:, m:m + 1])
                for p in range(3):
                    th.append((lambda m=m, p=p, ps_box=ps_box: mk_mm(m, p, ps_box)))
                th.append(mk_act)
            return th

        def alloc_ident(t):
            s, j = divmod(t, 32)
            ps_rz = pSpool.tile([128, 112], F32, tag="ps_rz", name=f"psrz_{t}")
            nc.tensor.matmul(ps_rz[:, 0:64], identf, stages[s][:, j, 0:8, :],
                             start=True, stop=False)
            return ps_rz  # [128,112]: rz 0:64, n 64:96, margin row0 96:104

        # ---- prologue
        dma_et(0)
        dma_et(1)
        pending = []
        for th in pa_thunks(0):
            th()
        dma_et(2)
        pending.extend(pa_thunks(1))
        ps_next = alloc_ident(0)

        for t in range(T):
            s, j = divmod(t, 32)
            if j == 0 and s >= 1:
                if s + 2 < NS:
                    dma_et(s + 2)
                if s + 1 < NS:
                    pending.extend(pa_thunks(s + 1))
            ps_all = ps_next
            ps_rz = ps_all[:, 0:64]
            ps_n = ps_all[:, 64:96]
            ps_m = ps_all[0:1, 96:104]
            st = stages[s]
            for m in range(8):
                for k in range(4):
                    nc.tensor.matmul(ps_rz[:, m * 8:(m + 1) * 8],
                                     WTc[k][:, m * 128:(m + 1) * 128],
                                     hT[:, k * 8:(k + 1) * 8],
                                     start=False, stop=(k == 3))
            for m in range(4):
                for k in range(4):
                    nc.tensor.matmul(ps_n[:, m * 8:(m + 1) * 8],
                                     WTc[k][:, (8 + m) * 128:(9 + m) * 128],
                                     hT[:, k * 8:(k + 1) * 8],
                                     start=(k == 0), stop=(k == 3))
            rz = tpool.tile([128, 64], F32, tag="rz1", name=f"rz1_{t}")
            nc.scalar.activation(rz, ps_rz, AF.Sigmoid)
            zc = tpool.tile([128, 32], F32, tag="zc1", name=f"zc1_{t}")
            nc.vector.tensor_scalar(out=zc, in0=rz[:, 32:64], scalar1=-1.0,
                                    scalar2=1.0, op0=ALU.mult, op1=ALU.add)
            zh = tpool.tile([128, 32], F32, tag="zh1", name=f"zh1_{t}")
            nc.vector.tensor_mul(out=zh, in0=rz[:, 32:64], in1=hT)
            t2 = tpool.tile([128, 32], F32, tag="t2_1", name=f"t2_1_{t}")
            nc.vector.tensor_mul(out=t2, in0=ps_n, in1=rz[:, 0:32])
            u = tpool.tile([128, 32], F32, tag="u1", name=f"u1_{t}")
            nc.vector.tensor_add(out=u.rearrange("p (c b) -> p c b", c=4),
                                 in0=t2.rearrange("p (c b) -> p c b", c=4),
                                 in1=st[:, j, 8:12, :])
            nn = tpool.tile([128, 32], F32, tag="nn1", name=f"nn1_{t}")
            nc.scalar.activation(nn, u, AF.Tanh)
            v = tpool.tile([128, 32], F32, tag="v1", name=f"v1_{t}")
            nc.vector.tensor_mul(out=v, in0=nn, in1=zc)
            nc.vector.tensor_add(out=hT, in0=v, in1=zh)
            if t + 1 < T:
                ps_next = alloc_ident(t + 1)
            for k in range(4):
                nc.tensor.matmul(ps_m, wdT[:, k:k + 1], hT[:, k * 8:(k + 1) * 8],
                                 start=(k == 0), stop=(k == 3))
            nc.scalar.copy(out=marg[0:1, t * 8:(t + 1) * 8], in_=ps_m)
            nrun = -(-len(pending) // (32 - j))
            for _ in range(min(nrun, len(pending))):
                pending.pop(0)()
            if t % 64 == 63:
                nc.sync.dma_start(out=margins_out[bass.ds((t - 63) * 8, 512)],
                                  in_=marg[0:1, (t - 63) * 8:(t + 1) * 8])
    nc.compile()
    return nc


# ---------------------------------------------------------------- NEFF2 ----

def emit_layer_scan(nc, tc, ctx, name, WhT, gi_dram, mask, masku, ybuf, ycols, n_it):
    """Masked bf16 GRU scan. WhT: 4x sbuf [128,1536] bf16. gi_dram: [128, NITER*1536] bf16.
    mask: sbuf [128, T*BL] bf16 (1/0). ybuf: sbuf [128, 4*ycols] bf16 out (col c*ycols + t*8+b)."""
    spool = ctx.enter_context(tc.tile_pool(name=f"{name}st", bufs=1))
    gpool = ctx.enter_context(tc.tile_pool(name=f"{name}gi", bufs=3))
    ppool = ctx.enter_context(tc.tile_pool(name=f"{name}ps", bufs=2, space="PSUM"))
    tpool = ctx.enter_context(tc.tile_pool(name=f"{name}tmp", bufs=3))

    h16 = spool.tile([128, 32], BF16, tag=f"{name}h16")
    nc.vector.memset(h16, 0.0)
    yb4 = ybuf.rearrange("p (c q) -> p c q", c=4)

    with tc.For_i(0, n_it, 1, hint_engines=(PE, DVE, ACT)) as it:
        gi = gpool.tile([128, UNROLL * 96], BF16, tag=f"{name}gi")
        nc.sync.dma_start(out=gi, in_=gi_dram[:, bass.ds(it * (UNROLL * 96), UNROLL * 96)])
        for j in range(UNROLL):
            tcol = it * UNROLL * 8 + j * 8
            ps_rz = ppool.tile([128, 64], F32, tag=f"{name}ps_rz")
            ps_n = ppool.tile([128, 32], F32, tag=f"{name}ps_n")
            for m in range(12):
                ps = ps_rz[:, m * 8:(m + 1) * 8] if m < 8 else ps_n[:, (m - 8) * 8:(m - 7) * 8]
                for k in range(4):
                    nc.tensor.matmul(ps, WhT[k][:, m * 128:(m + 1) * 128],
                                     h16[:, k * 8:(k + 1) * 8],
                                     start=(k == 0), stop=(k == 3))
            gslice = gi[:, j * 96:(j + 1) * 96]
            a = tpool.tile([128, 64], F32, tag=f"{name}a")
            nc.vector.tensor_add(out=a, in0=ps_rz, in1=gslice[:, 0:64])
            rz = tpool.tile([128, 64], F32, tag=f"{name}rz")
            nc.scalar.activation(rz, a, AF.Sigmoid)
            zc = tpool.tile([128, 32], F32, tag=f"{name}zc")
            nc.vector.tensor_scalar(out=zc, in0=rz[:, 32:64], scalar1=-1.0,
                                    scalar2=1.0, op0=ALU.mult, op1=ALU.add)
            zh = tpool.tile([128, 32], F32, tag=f"{name}zh")
            nc.vector.tensor_mul(out=zh, in0=h16, in1=rz[:, 32:64])
            t2 = tpool.tile([128, 32], F32, tag=f"{name}t2")
            nc.vector.tensor_mul(out=t2, in0=ps_n, in1=rz[:, 0:32])
            u = tpool.tile([128, 32], F32, tag=f"{name}u")
            nc.vector.tensor_add(out=u, in0=t2, in1=gslice[:, 64:96])
            nn_ = tpool.tile([128, 32], F32, tag=f"{name}nn")
            nc.scalar.activation(nn_, u, AF.Tanh)
            v = tpool.tile([128, 32], F32, tag=f"{name}v")
            nc.vector.tensor_mul(out=v, in0=nn_, in1=zc)
            hn16 = tpool.tile([128, 32], BF16, tag=f"{name}hn16")
            nc.vector.tensor_add(out=hn16, in0=v, in1=zh)
            mview = mask[:, bass.ds(tcol, 8)].unsqueeze(1).broadcast_to([128, 4, 8])
            muview = masku[:, bass.ds(tcol, 8)].unsqueeze(1).broadcast_to([128, 4, 8])
            hn3 = hn16.rearrange("p (c b) -> p c b", c=4)
            # y = m * h'  (zero where invalid)
            nc.vector.tensor_mul(out=yb4[:, :, bass.ds(tcol, 8)], in0=hn3, in1=mview)
            # h <- m ? h' : h
            nc.vector.copy_predicated(out=h16.rearrange("p (c b) -> p c b", c=4),
                                      mask=muview, data=hn3)


def build_neff2_v2(t_pad, n_full, dump=False):
    """Fused L0+L1 GRU scans in 32-step sub-chunks with software pipelining:
    tick s: L0 chunk s | L1 chunk s-2, with gi0 proj (s+1), gi1 proj (s-1),
    and conv (s-3) matmuls sprinkled into PE idle between scan steps.
    Steps below 32*n_full skip all masking; h state lives in-place in the
    y buffer so the GRU update writes y directly.
    """
    nc = bacc.Bacc("TRN2", target_bir_lowering=False, debug=False, num_devices=NC)
    TB = t_pad * BL
    nsub = t_pad // 32
    TBP = (t_pad + 16) * BL
    nembT_in = nc.dram_tensor("nembT", [128, 6 * TB], BF16, kind="ExternalInput").ap()
    mask_in = nc.dram_tensor("maskf", [128, TB], BF16, kind="ExternalInput").ap()
    masku_in = nc.dram_tensor("masku", [128, TB], mybir.dt.uint8, kind="ExternalInput").ap()
    Wih0T_in = nc.dram_tensor("Wih0T", [6, 128, 1536], BF16, kind="ExternalInput").ap()
    WhT0_in = nc.dram_tensor("WhT0", [4, 128, 1536], BF16, kind="ExternalInput").ap()
    Wih1T_in = nc.dram_tensor("Wih1T", [4, 128, 1536], BF16, kind="ExternalInput").ap()
    WhT1_in = nc.dram_tensor("WhT1", [4, 128, 1536], BF16, kind="ExternalInput").ap()
    bias0_in = nc.dram_tensor("bias0", [128, 12], F32, kind="ExternalInput").ap()
    bias1_in = nc.dram_tensor("bias1", [128, 12], F32, kind="ExternalInput").ap()
    identb_in = nc.dram_tensor("identb", [128, 128], BF16, kind="ExternalInput").ap()
    Wconv_in = nc.dram_tensor("Wconv", [128, 12 * 4 * 256], BF16, kind="ExternalInput").ap()
    bconv_in = nc.dram_tensor("bconv", [128, 6], F32, kind="ExternalInput").ap()
    WoT_in = nc.dram_tensor("WoT", [128, 6], F32, kind="ExternalInput").ap()
    bo_in = nc.dram_tensor("bo", [1, 1], F32, kind="ExternalInput").ap()
    out_dram = nc.dram_tensor("out", [1, BL], F32, kind="ExternalOutput").ap()
    if dump:
        TBP_ = (t_pad + 16) * BL
        y0d_out = nc.dram_tensor("y0d", [128, 4 * (t_pad * BL + 8)], BF16, kind="ExternalOutput").ap()
        y1d_out = nc.dram_tensor("y1d", [128, 4 * (TBP_ + 8)], BF16, kind="ExternalOutput").ap()
        g0d_out = nc.dram_tensor("g0d", [128, 32 * 12 * 8], BF16, kind="ExternalOutput").ap()
        s0d_out = nc.dram_tensor("s0d", [128, 96 + 64 + 6 * 32], F32, kind="ExternalOutput").ap()

    with TileContext(nc) as tc, ExitStack() as ctx:
        wpool = ctx.enter_context(tc.tile_pool(name="w2", bufs=1))
        etpool = ctx.enter_context(tc.tile_pool(name="et2", bufs=3))
        g0pool = ctx.enter_context(tc.tile_pool(name="g0st", bufs=2))
        g1pool = ctx.enter_context(tc.tile_pool(name="g1st", bufs=2))
        pApool = ctx.enter_context(tc.tile_pool(name="psA2", bufs=2, space="PSUM"))
        pBpool = ctx.enter_context(tc.tile_pool(name="psB2", bufs=2, space="PSUM"))
        pC0 = ctx.enter_context(tc.tile_pool(name="psL0", bufs=2, space="PSUM"))
        pC1 = ctx.enter_context(tc.tile_pool(name="psL1", bufs=2, space="PSUM"))
        t0pool = ctx.enter_context(tc.tile_pool(name="tmp0", bufs=3))
        t1pool = ctx.enter_context(tc.tile_pool(name="tmp1", bufs=3))

        def loadw(name, src, n, dtype=BF16):
            out = []
            for k in range(n):
                wt = wpool.tile([128, 1536], dtype, tag=f"{name}{k}", name=f"{name}{k}")
                nc.sync.dma_start(out=wt, in_=src[k])
                out.append(wt)
            return out

        Wih0T = loadw("wih0", Wih0T_in, 6)
        WhT0 = loadw("wh0", WhT0_in, 4)
        Wih1T = loadw("wih1", Wih1T_in, 4)
        WhT1 = loadw("wh1", WhT1_in, 4)
        bias0 = wpool.tile([128, 12], F32, tag="bias0")
        nc.sync.dma_start(out=bias0, in_=bias0_in)
        bias1 = wpool.tile([128, 12], F32, tag="bias1")
        nc.sync.dma_start(out=bias1, in_=bias1_in)
        identb = wpool.tile([128, 128], BF16, tag="identb")
        nc.sync.dma_start(out=identb, in_=identb_in)
        Wconv_t = wpool.tile([128, 12 * 4 * 256], BF16, tag="Wconv")
        nc.sync.dma_start(out=Wconv_t, in_=Wconv_in)
        Wconv = Wconv_t.rearrange("p (d k c) -> p d k c", d=12, k=4)
        bconv = wpool.tile([128, 6], F32, tag="bconv")
        nc.sync.dma_start(out=bconv, in_=bconv_in)
        WoT = wpool.tile([128, 6], F32, tag="WoT")
        nc.sync.dma_start(out=WoT, in_=WoT_in)
        bo_sb = wpool.tile([1, 1], F32, tag="bo_sb")
        nc.sync.dma_start(out=bo_sb, in_=bo_in)
        maskf = wpool.tile([128, TB], BF16, tag="maskf")
        nc.sync.dma_start(out=maskf, in_=mask_in)
        masku = wpool.tile([128, TB], mybir.dt.uint8, tag="masku")
        nc.sync.dma_start(out=masku, in_=masku_in)

        # y buffers: col (t+1)*8+b per k-plane; slot 0 = zeroed h(-1)
        y0buf = wpool.tile([128, 4 * (TB + 8)], BF16, tag="y0buf")
        y1buf = wpool.tile([128, 4 * (TBP + 8)], BF16, tag="y1buf")
        y0 = y0buf.rearrange("p (c q) -> p c q", c=4)
        y1 = y1buf.rearrange("p (c q) -> p c q", c=4)
        for k in range(4):
            nc.vector.memset(y0buf[:, k * (TB + 8):k * (TB + 8) + 8], 0.0)
            nc.vector.memset(y1buf[:, k * (TBP + 8):k * (TBP + 8) + 8], 0.0)
            nc.vector.memset(y1buf[:, k * (TBP + 8) + 8 + TB:(k + 1) * (TBP + 8)], 0.0)
        h16_0 = wpool.tile([128, 32], BF16, tag="h16_0")
        h16_1 = wpool.tile([128, 32], BF16, tag="h16_1")

        nembT = nembT_in.rearrange("p (k q) -> p k q", k=6)
        stage0 = [None] * nsub
        stage1 = [None] * nsub
        et_tiles = [None] * nsub

        def dma_et(s):
            et = etpool.tile([128, 6, 256], BF16, tag="et", name=f"et{s}")
            nc.sync.dma_start(out=et, in_=nembT[:, :, s * 256:(s + 1) * 256])
            et_tiles[s] = et

        def gi0_thunks(s):
            st = g0pool.tile([128, 32, 12, 8], BF16, tag="g0", name=f"g0_{s}")
            stage0[s] = st
            th = []
            et = et_tiles[s]
            for m in range(12):
                def mk(m=m):
                    ps = pApool.tile([128, 256], F32, tag="psA", name=f"psA_{s}_{m}")
                    for k in range(6):
                        nc.tensor.matmul(ps, Wih0T[k][:, m * 128:(m + 1) * 128],
                                         et[:, k, :], start=(k == 0), stop=(k == 5))
                    nc.scalar.activation(stage0[s][:, :, m, :], ps, AF.Identity,
                                         bias=bias0[:, m:m + 1])
                th.append(mk)
            return th

        def gi1_thunks(s):
            st = g1pool.tile([128, 32, 12, 8], BF16, tag="g1", name=f"g1_{s}")
            stage1[s] = st
            th = []
            for m in range(12):
                def mk(m=m):
                    ps = pApool.tile([128, 256], F32, tag="psA", name=f"psA1_{s}_{m}")
                    for k in range(4):
                        nc.tensor.matmul(ps, Wih1T[k][:, m * 128:(m + 1) * 128],
                                         y0[:, k, (32 * s + 1) * 8:(32 * s + 33) * 8],
                                         start=(k == 0), stop=(k == 3))
                    nc.scalar.activation(stage1[s][:, :, m, :], ps, AF.Identity,
                                         bias=bias1[:, m:m + 1])
                th.append(mk)
            return th

        dt_base = {3: 0, 4: 3, 5: 7}
        maccs = {}
        for fi, fs in enumerate(FS):
            for mt in range(2):
                macc = wpool.tile([128, 8], F32, tag=f"macc{fi}{mt}")
                nc.vector.memset(macc, -1e30)
                maccs[(fi, mt)] = macc

        def conv_thunks(c):
            th = []
            for fi, fs in enumerate(FS):
                for mt in range(2):
                    def mk(fi=fi, fs=fs, mt=mt):
                        ps = pBpool.tile([128, 256], F32, tag="psCV", name=f"psCV_{c}_{fi}_{mt}")
                        first = True
                        for dt in range(fs):
                            for k in range(4):
                                nc.tensor.matmul(
                                    ps, Wconv[:, dt_base[fs] + dt, k, mt * 128:(mt + 1) * 128],
                                    y1[:, k, (32 * c + dt + 1) * 8:(32 * c + dt + 33) * 8],
                                    start=first, stop=(dt == fs - 1 and k == 3))
                                first = False
                        nvalid = 32 if c < nsub - 1 else 33 - fs
                        cm = t0pool.tile([128, 8], F32, tag="cvcm", name=f"cvcm_{c}_{fi}_{mt}")
                        nc.vector.tensor_reduce(
                            out=cm, in_=ps.rearrange("p (t b) -> p b t", t=32)[:, :, 0:nvalid],
                            axis=mybir.AxisListType.X, op=ALU.max)
                        nc.vector.tensor_max(out=maccs[(fi, mt)], in0=maccs[(fi, mt)], in1=cm)
                    th.append(mk)
            return th

        def l_step(l, t, stage_l, WhT, ppool, tpool, y, h16):
            masked = t >= 32 * n_full
            s, j = t // 32, t % 32
            st = stage_l[s]
            if masked and t == 32 * n_full:
                nc.vector.tensor_copy(out=h16.rearrange("p (c b) -> p c b", c=4),
                                      in_=y[:, :, t * 8:(t + 1) * 8])
            if masked:
                hp = [h16[:, k * 8:(k + 1) * 8] for k in range(4)]
                hp3 = h16.rearrange("p (c b) -> p c b", c=4)
            else:
                hp = [y[:, k, t * 8:(t + 1) * 8] for k in range(4)]
                hp3 = y[:, :, t * 8:(t + 1) * 8]
            ps_all = ppool.tile([128, 96], F32, tag=f"psg{l}", name=f"psg{l}_{t}")
            ps_rz = ps_all[:, 0:64]
            ps_n = ps_all[:, 64:96]
            # single identity matmul: psum zero-region semantics require exactly
            # one start=True writer per region before the accumulates
            nc.tensor.matmul(ps_rz, identb, st[:, j, 0:8, :], start=True, stop=False)
            for m in range(8):
                for k in range(4):
                    nc.tensor.matmul(ps_rz[:, m * 8:(m + 1) * 8],
                                     WhT[k][:, m * 128:(m + 1) * 128], hp[k],
                                     start=False, stop=(k == 3))
            for m in range(4):
                for k in range(4):
                    nc.tensor.matmul(ps_n[:, m * 8:(m + 1) * 8],
                                     WhT[k][:, (8 + m) * 128:(9 + m) * 128], hp[k],
                                     start=(k == 0), stop=(k == 3))
            yield  # stage 0: PE emitted
            rz = tpool.tile([128, 64], F32, tag=f"rz{l}", name=f"rz{l}_{t}")
            nc.scalar.activation(rz, ps_rz, AF.Sigmoid)
            yield  # stage 1: sigmoid emitted
            t2 = tpool.tile([128, 32], F32, tag=f"t2{l}", name=f"t2{l}_{t}")
            nc.vector.tensor_mul(out=t2, in0=ps_n, in1=rz[:, 0:32])
            u = tpool.tile([128, 32], F32, tag=f"u{l}", name=f"u{l}_{t}")
            nc.vector.tensor_add(out=u.rearrange("p (c b) -> p c b", c=4),
                                 in0=t2.rearrange("p (c b) -> p c b", c=4),
                                 in1=st[:, j, 8:12, :])
            zc = tpool.tile([128, 32], F32, tag=f"zc{l}", name=f"zc{l}_{t}")
            nc.vector.tensor_scalar(out=zc, in0=rz[:, 32:64], scalar1=-1.0,
                                    scalar2=1.0, op0=ALU.mult, op1=ALU.add)
            zh = tpool.tile([128, 32], F32, tag=f"zh{l}", name=f"zh{l}_{t}")
            nc.vector.tensor_mul(out=zh.rearrange("p (c b) -> p c b", c=4),
                                 in0=rz[:, 32:64].rearrange("p (c b) -> p c b", c=4),
                                 in1=hp3)
            yield  # stage 2: critical DVE (t2,u) emitted
            nn = tpool.tile([128, 32], F32, tag=f"nn{l}", name=f"nn{l}_{t}")
            nc.scalar.activation(nn, u, AF.Tanh)
            yield  # stage 3: tanh emitted
            v = tpool.tile([128, 32], F32, tag=f"v{l}", name=f"v{l}_{t}")
            nc.vector.tensor_mul(out=v, in0=nn, in1=zc)
            if dump and l == 0 and t == 0:
                dbg = wpool.tile([128, 96 + 64 + 6 * 32], F32, tag="dbgs0")
                nc.vector.tensor_copy(out=dbg[:, 0:96], in_=ps_all)
                nc.vector.tensor_copy(out=dbg[:, 96:160], in_=rz)
                for i, x in enumerate((zc, zh, t2, u, nn, v)):
                    nc.vector.tensor_copy(out=dbg[:, 160 + i * 32:160 + (i + 1) * 32], in_=x)
                nc.sync.dma_start(out=s0d_out, in_=dbg)
            ynew = y[:, :, (t + 1) * 8:(t + 2) * 8]
            if not masked:
                nc.vector.tensor_add(out=ynew, in0=v.rearrange("p (c b) -> p c b", c=4),
                                     in1=zh.rearrange("p (c b) -> p c b", c=4))
            else:
                hn16 = tpool.tile([128, 32], BF16, tag=f"hn{l}", name=f"hn{l}_{t}")
                nc.vector.tensor_add(out=hn16, in0=v, in1=zh)
                hn3 = hn16.rearrange("p (c b) -> p c b", c=4)
                mview = maskf[:, t * 8:(t + 1) * 8].unsqueeze(1).broadcast_to([128, 4, 8])
                muview = masku[:, t * 8:(t + 1) * 8].unsqueeze(1).broadcast_to([128, 4, 8])
                nc.vector.tensor_mul(out=ynew, in0=hn3, in1=mview)
                nc.vector.copy_predicated(out=h16.rearrange("p (c b) -> p c b", c=4),
                                          mask=muview, data=hn3)

        # ---- pipeline emission
        dma_et(0)
        if nsub > 1:
            dma_et(1)
        for th in gi0_thunks(0):
            th()
        if dump:
            nc.sync.dma_start(out=g0d_out, in_=stage0[0])
        pending = []
        for s in range(nsub + 3):
            if s + 2 < nsub:
                dma_et(s + 2)
            if s + 1 < nsub:
                pending.extend(gi0_thunks(s + 1))
            if 0 <= s - 1 < nsub:
                pending.extend(gi1_thunks(s - 1))
            for j in range(32):
                if j == 6 and 0 <= s - 3 < nsub:
                    # conv chunk s-3 reads y1 of chunk s-2 steps 0..4, whose
                    # writes are emitted at j=0..4 of this tick
                    pending.extend(conv_thunks(s - 3))
                # drive both layers' steps stage-interleaved so the per-engine
                # FIFO order is [PE0,PE1][sig0,sig1][dve0,dve1][tanh0,tanh1]
                # [tail0,tail1] instead of serializing the two chains
                gens = []
                if s < nsub:
                    gens.append(l_step(0, 32 * s + j, stage0, WhT0, pC0, t0pool, y0, h16_0))
                if 0 <= s - 2 < nsub:
                    gens.append(l_step(1, 32 * (s - 2) + j, stage1, WhT1, pC1, t1pool, y1, h16_1))
                for _ in range(5):
                    for g in gens:
                        next(g, None)
                nrun = -(-len(pending) // (32 - j))
                for _ in range(min(nrun, len(pending))):
                    pending.pop(0)()

        # ---- epilogue: relu+bias pool, output linear
        pooled = wpool.tile([128, 48], F32, tag="pooled")
        for fi, fs in enumerate(FS):
            for mt in range(2):
                ci = fi * 2 + mt
                macc = maccs[(fi, mt)]
                if t_pad < T:
                    nc.vector.tensor_scalar_max(out=macc, in0=macc, scalar1=0.0)
                nc.scalar.activation(pooled[:, ci * 8:(ci + 1) * 8], macc, AF.Relu,
                                     bias=bconv[:, ci:ci + 1])
        ps_o_t = pApool.tile([128, 256], F32, tag="psA", name="ps_o_t")
        ps_o = ps_o_t[0:1, 0:8]
        for ci in range(6):
            nc.tensor.matmul(ps_o, WoT[:, ci:ci + 1], pooled[:, ci * 8:(ci + 1) * 8],
                             start=(ci == 0), stop=(ci == 5))
        ov = wpool.tile([1, BL], F32, tag="ov")
        nc.vector.tensor_scalar(out=ov, in0=ps_o, scalar1=bo_sb[0:1, 0:1], scalar2=None, op0=ALU.add)
        nc.sync.dma_start(out=out_dram, in_=ov)
        if dump:
            nc.sync.dma_start(out=y0d_out, in_=y0buf)
            nc.sync.dma_start(out=y1d_out, in_=y1buf)
    nc.compile()
    return nc


def build_neff2(t_pad):
    nc = bacc.Bacc("TRN2", target_bir_lowering=False, debug=False, num_devices=NC)
    TB = t_pad * BL
    NCH = t_pad // 64
    NIT2 = t_pad // UNROLL
    nembT_in = nc.dram_tensor("nembT", [128, 6 * TB], BF16, kind="ExternalInput").ap()
    mask_in = nc.dram_tensor("maskf", [128, TB], BF16, kind="ExternalInput").ap()
    masku_in = nc.dram_tensor("masku", [128, TB], mybir.dt.uint8, kind="ExternalInput").ap()
    Wih0T_in = nc.dram_tensor("Wih0T", [6, 128, 1536], BF16, kind="ExternalInput").ap()
    WhT0_in = nc.dram_tensor("WhT0", [4, 128, 1536], BF16, kind="ExternalInput").ap()
    Wih1T_in = nc.dram_tensor("Wih1T", [4, 128, 1536], BF16, kind="ExternalInput").ap()
    WhT1_in = nc.dram_tensor("WhT1", [4, 128, 1536], BF16, kind="ExternalInput").ap()
    bias0_in = nc.dram_tensor("bias0", [128, 12], F32, kind="ExternalInput").ap()
    bias1_in = nc.dram_tensor("bias1", [128, 12], F32, kind="ExternalInput").ap()
    Wconv_in = nc.dram_tensor("Wconv", [128, 12 * 4 * 256], BF16, kind="ExternalInput").ap()
    bconv_in = nc.dram_tensor("bconv", [128, 6], F32, kind="ExternalInput").ap()
    WoT_in = nc.dram_tensor("WoT", [128, 6], F32, kind="ExternalInput").ap()
    bo_in = nc.dram_tensor("bo", [1, 1], F32, kind="ExternalInput").ap()
    out_dram = nc.dram_tensor("out", [1, BL], F32, kind="ExternalOutput").ap()

    TPAD = t_pad + 16

    with TileContext(nc) as tc, ExitStack() as ctx:
        wpool = ctx.enter_context(tc.tile_pool(name="w2", bufs=1))
        dpool = ctx.enter_context(tc.tile_pool(name="dram2", bufs=1, space="DRAM"))
        gi0d = dpool.tile([128, NIT2 * UNROLL * 96], BF16, tag="gi0d")
        gi1d = dpool.tile([128, NIT2 * UNROLL * 96], BF16, tag="gi1d")

        def load_w(name, src, n, dtype=BF16):
            out = []
            for k in range(n):
                wt = wpool.tile([128, 1536], dtype, tag=f"{name}{k}")
                nc.sync.dma_start(out=wt, in_=src[k])
                out.append(wt)
            return out

        Wih0T = load_w("wih0", Wih0T_in, 6)
        WhT0 = load_w("wh0", WhT0_in, 4)
        Wih1T = load_w("wih1", Wih1T_in, 4)
        WhT1 = load_w("wh1", WhT1_in, 4)
        bias0 = wpool.tile([128, 12], F32, tag="bias0")
        nc.sync.dma_start(out=bias0, in_=bias0_in)
        bias1 = wpool.tile([128, 12], F32, tag="bias1")
        nc.sync.dma_start(out=bias1, in_=bias1_in)
        maskf = wpool.tile([128, TB], BF16, tag="maskf")
        nc.sync.dma_start(out=maskf, in_=mask_in)
        masku = wpool.tile([128, TB], mybir.dt.uint8, tag="masku")
        nc.sync.dma_start(out=masku, in_=masku_in)

        # --- gi0 = Wih0 @ nembT + bias0  (nembT streamed per chunk)
        with tc.tile_pool(name="nemb2", bufs=2) as npool, tc.tile_pool(name="st2", bufs=2) as stpool, tc.tile_pool(name="psg0", bufs=2, space="PSUM") as ppool:
            for nch in range(NCH):
                net = npool.tile([128, 6, 512], BF16, tag="net")
                nc.sync.dma_start(out=net, in_=nembT_in.rearrange("p (k c) -> p k c", k=6)[:, :, nch * 512:(nch + 1) * 512])
                stage = stpool.tile([128, 6144], BF16, tag="stage0")
                st4 = stage.rearrange("p (i j c b) -> p i j c b", i=I4, j=UNROLL, c=12)
                for m in range(12):
                    ps = ppool.tile([128, 512], F32, tag="ps_gi0")
                    for k in range(6):
                        nc.tensor.matmul(ps, Wih0T[k][:, m * 128:(m + 1) * 128],
                                         net[:, k, :],
                                         start=(k == 0), stop=(k == 5))
                    nc.vector.tensor_scalar(
                        out=st4[:, :, :, m, :],
                        in0=ps.rearrange("p (i j b) -> p i j b", i=I4, j=UNROLL),
                        scalar1=bias0[:, m:m + 1], scalar2=None, op0=ALU.add)
                nc.sync.dma_start(out=gi0d[:, nch * 6144:(nch + 1) * 6144], in_=stage)

        with tc.tile_pool(name="y0p", bufs=1) as y0pool:
            y0buf = y0pool.tile([128, 4 * TB], BF16, tag="y0buf")
            # --- L0 scan
            with ExitStack() as c0:
                emit_layer_scan(nc, tc, c0, "L0", WhT0, gi0d, maskf, masku, y0buf, TB, NIT2)

            # --- gi1 = Wih1 @ y0 + bias1
            y04 = y0buf.rearrange("p (c q) -> p c q", c=4)
            with tc.tile_pool(name="st3", bufs=2) as stpool, tc.tile_pool(name="psg1", bufs=2, space="PSUM") as ppool:
                for nch in range(NCH):
                    stage = stpool.tile([128, 6144], BF16, tag="stage1")
                    st4 = stage.rearrange("p (i j c b) -> p i j c b", i=I4, j=UNROLL, c=12)
                    for m in range(12):
                        ps = ppool.tile([128, 512], F32, tag="ps_gi1")
                        for k in range(4):
                            nc.tensor.matmul(ps, Wih1T[k][:, m * 128:(m + 1) * 128],
                                             y04[:, k, nch * 512:(nch + 1) * 512],
                                             start=(k == 0), stop=(k == 3))
                        nc.vector.tensor_scalar(
                            out=st4[:, :, :, m, :],
                            in0=ps.rearrange("p (i j b) -> p i j b", i=I4, j=UNROLL),
                            scalar1=bias1[:, m:m + 1], scalar2=None, op0=ALU.add)
                    nc.sync.dma_start(out=gi1d[:, nch * 6144:(nch + 1) * 6144], in_=stage)

        # --- L1 scan (padded y buffer for conv reads)
        y1buf = wpool.tile([128, 4 * TPAD * BL], BF16, tag="y1buf")
        nc.vector.memset(y1buf, 0.0)
        with ExitStack() as c1:
            emit_layer_scan(nc, tc, c1, "L1", WhT1, gi1d, maskf, masku, y1buf, TPAD * BL, NIT2)

        # --- convs + maxpool + relu + output linear
        Wconv_t = wpool.tile([128, 12 * 4 * 256], BF16, tag="Wconv")
        nc.sync.dma_start(out=Wconv_t, in_=Wconv_in)
        Wconv = Wconv_t.rearrange("p (d k c) -> p d k c", d=12, k=4)
        bconv = wpool.tile([128, 6], F32, tag="bconv")
        nc.sync.dma_start(out=bconv, in_=bconv_in)
        WoT = wpool.tile([128, 6], F32, tag="WoT")
        nc.sync.dma_start(out=WoT, in_=WoT_in)
        cpool = ctx.enter_context(tc.tile_pool(name="cv", bufs=2))
        ppool = ctx.enter_context(tc.tile_pool(name="pscv", bufs=2, space="PSUM"))
        pooled = wpool.tile([128, 48], F32, tag="pooled")
        y14 = y1buf.rearrange("p (c q) -> p c q", c=4)
        dt_base = {3: 0, 4: 3, 5: 7}
        for fi, fs in enumerate(FS):
            for mt in range(2):
                ci = fi * 2 + mt
                macc = cpool.tile([128, 8], F32, tag="macc")
                nc.vector.memset(macc, -1e30)
                for nch in range(NCH):
                    ps = ppool.tile([128, 512], F32, tag="ps_cv")
                    first = True
                    for dt in range(fs):
                        for k in range(4):
                            nc.tensor.matmul(
                                ps, Wconv[:, dt_base[fs] + dt, k, mt * 128:(mt + 1) * 128],
                                y14[:, k, nch * 512 + dt * 8: nch * 512 + dt * 8 + 512],
                                start=first, stop=(dt == fs - 1 and k == 3))
                            first = False
                    nvalid = 64 if nch < NCH - 1 else 65 - fs
                    cm = cpool.tile([128, 8], F32, tag="cm")
                    nc.vector.tensor_reduce(
                        out=cm, in_=ps.rearrange("p (t b) -> p b t", t=64)[:, :, 0:nvalid],
                        axis=mybir.AxisListType.X, op=ALU.max)
                    nc.vector.tensor_max(out=macc, in0=macc, in1=cm)
                if t_pad < T:
                    # windows beyond t_pad read all-zero y -> conv value exactly 0
                    nc.vector.tensor_scalar_max(out=macc, in0=macc, scalar1=0.0)
                nc.scalar.activation(pooled[:, ci * 8:(ci + 1) * 8], macc, AF.Relu,
                                     bias=bconv[:, ci:ci + 1])
        ps_o_t = ppool.tile([128, 8], F32, tag="ps_o")
        ps_o = ps_o_t[0:1, :]
        for ci in range(6):
            nc.tensor.matmul(ps_o, WoT[:, ci:ci + 1], pooled[:, ci * 8:(ci + 1) * 8],
                             start=(ci == 0), stop=(ci == 5))
        bo_sb = wpool.tile([1, 1], F32, tag="bo_sb")
        nc.sync.dma_start(out=bo_sb, in_=bo_in)
        ov = wpool.tile([1, BL], F32, tag="ov")
        nc.vector.tensor_scalar(out=ov, in0=ps_o, scalar1=bo_sb[0:1, 0:1], scalar2=None, op0=ALU.add)
        nc.sync.dma_start(out=out_dram, in_=ov)
    nc.compile()
    return nc


def _make_runner(nc, n_cores):
    import jax
    from jax.sharding import Mesh, PartitionSpec
    from jax.experimental.shard_map import shard_map
    import concourse.bass2jax as b2j
    b2j.install_neuronx_cc_hook()
    pname = nc.partition_id_tensor.name if nc.partition_id_tensor else None
    in_names, out_names, out_avals, zero_outs = [], [], [], []
    for alloc in nc.m.functions[0].allocations:
        if not isinstance(alloc, mybir.MemoryLocationSet):
            continue
        name = alloc.memorylocations[0].name
        if alloc.kind == "ExternalInput":
            if name != pname:
                in_names.append(name)
        elif alloc.kind == "ExternalOutput":
            out_names.append(name)
            shape = tuple(alloc.tensor_shape)
            dtype = mybir.dt.np(alloc.dtype)
            out_avals.append(jax.core.ShapedArray(shape, dtype))
            zero_outs.append(np.zeros(shape, dtype))
    n_params, n_outs = len(in_names), len(out_avals)
    all_in = list(in_names) + list(out_names) + ([pname] if pname else [])
    donate = tuple(range(n_params, n_params + n_outs))

    def _body(*args):
        operands = list(args)
        if pname is not None:
            operands.append(b2j.partition_id_tensor())
        outs = b2j._bass_exec_p.bind(
            *operands, out_avals=tuple(out_avals), in_names=tuple(all_in),
            out_names=tuple(out_names), lowering_input_output_aliases=(),
            sim_require_finite=True, sim_require_nnan=True, nc=nc)
        return tuple(outs)

    mesh = Mesh(np.asarray(jax.devices()[:n_cores]), ("core",))
    fn = jax.jit(shard_map(_body, mesh=mesh,
                           in_specs=(PartitionSpec("core"),) * (n_params + n_outs),
                           out_specs=(PartitionSpec("core"),) * n_outs, check_rep=False),
                 donate_argnums=donate, keep_unused=True)

    def run(in_maps):
        import jax
        per_core = [[np.asarray(m[name]) for name in in_names] for m in in_maps]
        concat_in = [np.concatenate([per_core[c][i] for c in range(n_cores)], axis=0)
                     for i in range(n_params)]
        zeros = [np.zeros((n_cores * z.shape[0], *z.shape[1:]), z.dtype) for z in zero_outs]
        out_arrs = fn(*concat_in, *zeros)
        jax.block_until_ready(out_arrs)
        return [{name: np.asarray(out_arrs[i]).reshape(n_cores, *out_avals[i].shape)[c]
                 for i, name in enumerate(out_names)} for c in range(n_cores)]

    def bench(in_maps, iters=10, slope=True):
        """Median wall time per call with device-resident inputs (s)."""
        import jax, time
        from jax.sharding import NamedSharding, PartitionSpec
        if getattr(bench, "_key", None) is id(in_maps):
            dev_in = bench._dev_in
        else:
            per_core = [[np.asarray(m[name]) for name in in_names] for m in in_maps]
            concat_in = [np.concatenate([per_core[c][i] for c in range(n_cores)], axis=0)
                         for i in range(n_params)]
            sh = NamedSharding(mesh, PartitionSpec("core"))
            dev_in = [jax.device_put(x, sh) for x in concat_in]
            jax.block_until_ready(dev_in)
            bench._key, bench._dev_in = id(in_maps), dev_in
        zeros = [np.zeros((n_cores * z.shape[0], *z.shape[1:]), z.dtype) for z in zero_outs]
        ts = []
        for i in range(iters):
            zs = [z.copy() for z in zeros]
            t0 = time.perf_counter()
            out = fn(*dev_in, *zs)
            jax.block_until_ready(out)
            ts.append(time.perf_counter() - t0)
        ts.sort()
        sl = 0.0
        if slope:
            # async slope: queue K calls, block once; removes dispatch latency
            for K_ in (2, 6):
                zss = [[z.copy() for z in zeros] for _ in range(K_)]
                t0 = time.perf_counter()
                outs = [fn(*dev_in, *zss[k]) for k in range(K_)]
                jax.block_until_ready(outs)
                tA = (time.perf_counter() - t0) if K_ == 2 else tA
                tB = (time.perf_counter() - t0) if K_ == 6 else 0.0
            sl = (tB - tA) / 4.0
        return ts[len(ts) // 2], {"sync": ts, "slope": sl}

    run.bench = bench
    return run


# ------------------------------------------------------------- host glue ----

_cache = {}


def _get_run1():
    if "r1" not in _cache:
        _cache["r1"] = _make_runner(build_neff1(), NC)
    return _cache["r1"]


def _prep_in1(emb, Wih_c, Whh_c, bih_c, bhh_c, Ws):
    f32, f16 = np.float32, np.float16
    WihcT = _fold_gates_T(Wih_c)
    WihH = WihcT.astype(f16)
    WihL = (WihcT - WihH.astype(f32)).astype(f16)
    WTc = _fold_gates_T(Whh_c).astype(f16)
    wd = Ws[1] - Ws[0]
    wd_hi = wd.astype(f16).astype(f32)
    wd_lo = (wd - wd_hi).astype(f16)
    wdP = np.zeros((8, 128, 128), f16)
    for k in range(4):
        wdP[2 * k, :, 0] = wd_hi[k * 128:(k + 1) * 128].astype(f16)
        wdP[2 * k + 1, :, 0] = wd_lo[k * 128:(k + 1) * 128]
    biasC = np.zeros((128, 12), f32)
    bsum = bih_c + bhh_c
    for m in range(12):
        biasC[:, m] = bsum[m * 128:(m + 1) * 128] if m < 8 else bih_c[m * 128:(m + 1) * 128]
    assert np.abs(bhh_c[1024:]).max() == 0.0, "nonzero bhh_c n-gate bias unsupported"
    in1 = []
    for c in range(NC):
        es = emb[c * BL:(c + 1) * BL]                 # [8, T, E]
        embT = np.ascontiguousarray(
            es.reshape(BL, T, 6, 128).transpose(3, 2, 1, 0)).reshape(128, 6 * T * BL)
        embH = embT.astype(f16)
        embL = (embT - embH.astype(f32)).astype(f16)
        in1.append({"embH": embH, "embL": embL, "WihH": WihH, "WihL": WihL,
                    "WTc": WTc, "wdP": wdP, "biasC": biasC})
    return in1


def _get_run2(t_pad, n_full):
    key = ("r2", t_pad, n_full)
    if key not in _cache:
        _cache[key] = _make_runner(build_neff2_v2(t_pad, n_full), NC)
    return _cache[key]


def _fold_gates_T(W):
    # W: [1536, K] -> [K/128, 128, 1536] lhsT tiles (W.T folded)
    K = W.shape[1]
    return np.ascontiguousarray(W.T.reshape(K // 128, 128, 1536))


def kernel(**inputs):
    emb = np.asarray(inputs["embedded"], np.float32)
    mask = np.asarray(inputs["mask"])
    lens = mask.sum(axis=1).astype(np.int64)
    f32 = np.float32
    Wih_c, Whh_c = np.asarray(inputs["Wih_c"], f32), np.asarray(inputs["Whh_c"], f32)
    bih_c, bhh_c = np.asarray(inputs["bih_c"], f32), np.asarray(inputs["bhh_c"], f32)
    Ws, bs = np.asarray(inputs["Ws"], f32), np.asarray(inputs["bs"], f32)
    Wih0, Whh0 = np.asarray(inputs["Wih0"], f32), np.asarray(inputs["Whh0"], f32)
    bih0, bhh0 = np.asarray(inputs["bih0"], f32), np.asarray(inputs["bhh0"], f32)
    Wih1, Whh1 = np.asarray(inputs["Wih1"], f32), np.asarray(inputs["Whh1"], f32)
    bih1, bhh1 = np.asarray(inputs["bih1"], f32), np.asarray(inputs["bhh1"], f32)
    Wc = {3: np.asarray(inputs["Wc3"], f32), 4: np.asarray(inputs["Wc4"], f32),
          5: np.asarray(inputs["Wc5"], f32)}
    bc = {3: np.asarray(inputs["bc3"], f32), 4: np.asarray(inputs["bc4"], f32),
          5: np.asarray(inputs["bc5"], f32)}
    Wo, bo = np.asarray(inputs["Wo"], f32), np.asarray(inputs["bo"], f32)

    run1 = _get_run1()
    in1 = _prep_in1(emb, Wih_c, Whh_c, bih_c, bhh_c, Ws)
    _cache["in1"] = in1
    res1 = run1(in1)
    margins = np.concatenate([r["margins"].reshape(T, BL).T[None] for r in res1], 0)
    margins = margins.reshape(NC * BL, T)                   # [B, T] (b-major per core)

    # ---- host compaction (bit logic + gather, zero FLOPs)
    sel, order, valid, t_pad, n_full = _compaction(margins, bs, lens)
    run2 = _get_run2(t_pad, n_full)
    in2 = _prep_in2(emb, order, valid, t_pad,
                    Wih0, Whh0, bih0, bhh0, Wih1, Whh1, bih1, bhh1,
                    Wc, bc, Wo, bo)
    _cache["in2"], _cache["last_r2"] = in2, run2
    res2 = run2(in2)
    out = np.concatenate([r["out"].reshape(BL) for r in res2], 0)
    return out.astype(np.float32)


def _build_floor():
    nc = bacc.Bacc("TRN2", target_bir_lowering=False, debug=False, num_devices=NC)
    x_in = nc.dram_tensor("x", [128, 8], F32, kind="ExternalInput").ap()
    y_out = nc.dram_tensor("y", [128, 8], F32, kind="ExternalOutput").ap()
    with TileContext(nc) as tc, ExitStack() as ctx:
        wp = ctx.enter_context(tc.tile_pool(name="w", bufs=1))
        xt = wp.tile([128, 8], F32, tag="x")
        nc.sync.dma_start(out=xt, in_=x_in)
        nc.sync.dma_start(out=y_out, in_=xt)
    nc.compile()
    return nc


def bench_hw(rounds=5, per=9):
    """Honest device-time estimate. Alternating between different NEFFs pays a
    ~40ms program-swap per call, so each NEFF is timed in a consecutive block
    (block median ignores the first-call swap); floor blocks bracket each NEFF
    block and the per-round delta uses the adjacent floor block, cancelling
    dispatch drift. Returns ns."""
    runf = _make_runner(_build_floor(), NC)
    inf = [{"x": np.zeros((128, 8), np.float32)} for _ in range(NC)]
    runf(inf)
    d1, d2 = [], []
    for r in range(rounds):
        tf1, _ = runf.bench(inf, iters=per, slope=False)
        t1, _ = _cache["r1"].bench(_cache["in1"], iters=per, slope=False)
        tf2, _ = runf.bench(inf, iters=per, slope=False)
        t2, _ = _cache["last_r2"].bench(_cache["in2"], iters=per, slope=False)
        d1.append(t1 - min(tf1, tf2))
        d2.append(t2 - tf2)
    d1.sort(); d2.sort()
    n1 = max(0.0, d1[len(d1) // 2]) * 1e9
    n2 = max(0.0, d2[len(d2) // 2]) * 1e9
    return {"neff1_ns": n1, "neff2_ns": n2, "total_ns": n1 + n2,
            "d1_ms": [round(x * 1e3, 2) for x in d1],
            "d2_ms": [round(x * 1e3, 2) for x in d2]}


def _compaction(margins, bs, lens):
    thr = bs[0] - bs[1]
    sel = (margins > thr).astype(np.int64)
    t_idx = np.arange(T)[None, :]
    sel[:, 0] = 1
    sel[np.arange(B), lens - 1] = 1
    sel = np.where(t_idx >= lens[:, None], 0, sel)
    nsel = sel.sum(1)
    order = np.argsort(1 - sel, axis=1, kind="stable")
    valid = t_idx < nsel[:, None]
    t_pad = min(T, max(32, int(-(-int(nsel.max()) // 32) * 32)))
    n_full = min(t_pad // 32, int(nsel.min()) // 32)
    return sel, order, valid, t_pad, n_full


def _prep_in2(emb, order, valid, t_pad, Wih0, Whh0, bih0, bhh0,
              Wih1, Whh1, bih1, bhh1, Wc, bc, Wo, bo):
    f32 = np.float32
    Wih0T = _fold_gates_T(Wih0).astype(ml_dtypes.bfloat16)
    WhT0 = _fold_gates_T(Whh0).astype(ml_dtypes.bfloat16)
    Wih1T = _fold_gates_T(Wih1).astype(ml_dtypes.bfloat16)
    WhT1 = _fold_gates_T(Whh1).astype(ml_dtypes.bfloat16)
    bias0 = np.zeros((128, 12), f32)
    b0sum = bih0 + bhh0
    for m in range(12):
        bias0[:, m] = b0sum[m * 128:(m + 1) * 128] if m < 8 else bih0[m * 128:(m + 1) * 128]
    assert np.abs(bhh0[1024:]).max() == 0.0 and np.abs(bhh1[1024:]).max() == 0.0
    bias1 = np.zeros((128, 12), f32)
    b1sum = bih1 + bhh1
    for m in range(12):
        bias1[:, m] = b1sum[m * 128:(m + 1) * 128] if m < 8 else bih1[m * 128:(m + 1) * 128]
    Wconv = np.zeros((12, 4, 128, 256), f32)
    dt_base = {3: 0, 4: 3, 5: 7}
    for fs in FS:
        Wf = Wc[fs][:, 0]                                   # [NF, fs, H]
        for dt in range(fs):
            for k in range(4):
                Wconv[dt_base[fs] + dt, k] = Wf[:, dt, k * 128:(k + 1) * 128].T
    Wconv = np.ascontiguousarray(Wconv.transpose(2, 0, 1, 3)).reshape(128, -1).astype(ml_dtypes.bfloat16)
    bconv = np.zeros((128, 6), f32)
    WoT = np.zeros((128, 6), f32)
    for fi, fs in enumerate(FS):
        for mt in range(2):
            bconv[:, fi * 2 + mt] = bc[fs][mt * 128:(mt + 1) * 128]
            WoT[:, fi * 2 + mt] = Wo[0, fi * 256 + mt * 128: fi * 256 + (mt + 1) * 128]

    in2 = []
    for c in range(NC):
        bsl = slice(c * BL, (c + 1) * BL)
        new_emb = np.take_along_axis(emb[bsl], order[bsl][:, :, None], axis=1)
        new_emb = (new_emb * valid[bsl][:, :, None])[:, :t_pad]
        nembT = np.ascontiguousarray(
            new_emb.reshape(BL, t_pad, 6, 128).transpose(3, 2, 1, 0)
        ).reshape(128, 6 * t_pad * BL).astype(ml_dtypes.bfloat16)
        vs = valid[bsl][:, :t_pad]
        maskf = np.ascontiguousarray(np.broadcast_to(
            vs.T.reshape(1, t_pad * BL), (128, t_pad * BL))).astype(ml_dtypes.bfloat16)
        masku = np.ascontiguousarray(np.broadcast_to(
            vs.T.reshape(1, t_pad * BL), (128, t_pad * BL))).astype(np.uint8)
        in2.append({"nembT": nembT, "maskf": maskf, "masku": masku, "Wih0T": Wih0T, "WhT0": WhT0,
                    "Wih1T": Wih1T, "WhT1": WhT1, "bias0": bias0, "bias1": bias1,
                    "identb": np.eye(128, dtype=ml_dtypes.bfloat16),
                    "Wconv": Wconv, "bconv": bconv, "WoT": WoT,
                    "bo": bo.reshape(1, 1)})
    return in2



# revision 16
# speedup vs baseline: 1.8162x; 1.5911x over previous
"""Trainium2 Bass kernel for nn_CNN_RNN (select-GRU -> compact -> 2xGRU -> KimCNN).

Sharding: pure data-parallel, batch 64 -> 8 cores x 8.
Device NEFF1: select-gate input projection (fp32) + fp16-compensated select GRU scan
  -> per-(t,b) argmax margins.
Host: argmax bits -> stable-compaction gather indices (pure data movement) -> gathered
  embedding fold (bf16).
Device NEFF2: layer input projections (bf16), two masked GRU scans (bf16,
  weight-stationary), Kim-CNN convs as shifted matmuls, max-pool+relu, output linear.
NEFF2 is specialized (and cached) per runtime T_pad = ceil(max(nsel)/64)*64: the layer
scans, projections, and convs only run to the longest compacted sequence; skipped conv
windows are all-zero and reintroduced exactly via a max-with-0 before the relu+bias.
GRU tails use h' = n*(1-z) + z*h with (1-z) and z*h computed during the PE's n-gate
matmuls, keeping the serial post-matmul chain one op shorter.
"""
import numpy as np
import ml_dtypes

import concourse.bass as bass
import concourse.mybir as mybir
from concourse import bacc
from concourse.tile import TileContext
from contextlib import ExitStack

F32, F16, BF16 = mybir.dt.float32, mybir.dt.float16, mybir.dt.bfloat16
AF = mybir.ActivationFunctionType
ALU = mybir.AluOpType
PE, DVE, ACT = mybir.EngineType.PE, mybir.EngineType.DVE, mybir.EngineType.Activation

B, T, E, H, NF = 64, 512, 768, 512, 256
FS = (3, 4, 5)
NC = 8
BL = B // NC          # batch per core
UNROLL = 32
NITER = T // UNROLL
I4 = 64 // UNROLL   # iters per 64-t chunk


# ---------------------------------------------------------------- NEFF1 ----

def build_neff1():
    """Select scan, fully unrolled, with the f16 3-product input projection
    (phase A) sprinkled into PE idle slots between scan steps; gi stages live
    in SBUF (no DRAM roundtrip). Step math is the proven fp16-compensated
    baseline scheme (W f16, h split hi/lo f16 in the matmul free dim)."""
    nc = bacc.Bacc("TRN2", target_bir_lowering=False, debug=False, num_devices=NC)
    TBL = T * BL
    embH_in = nc.dram_tensor("embH", [128, 6 * TBL], F16, kind="ExternalInput").ap()
    embL_in = nc.dram_tensor("embL", [128, 6 * TBL], F16, kind="ExternalInput").ap()
    WihH_in = nc.dram_tensor("WihH", [6, 128, 1536], F16, kind="ExternalInput").ap()
    WihL_in = nc.dram_tensor("WihL", [6, 128, 1536], F16, kind="ExternalInput").ap()
    WTc_in = nc.dram_tensor("WTc", [4, 128, 1536], F16, kind="ExternalInput").ap()
    wdP_in = nc.dram_tensor("wdP", [8, 128, 128], F16, kind="ExternalInput").ap()
    biasC_in = nc.dram_tensor("biasC", [128, 12], F32, kind="ExternalInput").ap()
    margins_out = nc.dram_tensor("margins", [TBL], F32, kind="ExternalOutput").ap()
    NS = T // 32

    with TileContext(nc) as tc, ExitStack() as ctx:
        wpool = ctx.enter_context(tc.tile_pool(name="w1", bufs=1))
        etpool = ctx.enter_context(tc.tile_pool(name="et1", bufs=3))
        stpool = ctx.enter_context(tc.tile_pool(name="st1", bufs=3))
        pApool = ctx.enter_context(tc.tile_pool(name="psA1", bufs=2, space="PSUM"))
        ppoolB = ctx.enter_context(tc.tile_pool(name="psB", bufs=2, space="PSUM"))
        tpool = ctx.enter_context(tc.tile_pool(name="seltmp", bufs=3))

        WihH, WihL = [], []
        for k in range(6):
            wh = wpool.tile([128, 1536], F16, tag=f"wihH{k}", name=f"wihH{k}")
            nc.sync.dma_start(out=wh, in_=WihH_in[k])
            WihH.append(wh)
            wl = wpool.tile([128, 1536], F16, tag=f"wihL{k}", name=f"wihL{k}")
            nc.sync.dma_start(out=wl, in_=WihL_in[k])
            WihL.append(wl)
        WTc = []
        for k in range(4):
            wt = wpool.tile([128, 1536], F16, tag=f"wtc{k}", name=f"wtc{k}")
            nc.sync.dma_start(out=wt, in_=WTc_in[k])
            WTc.append(wt)
        wdP = []
        for i in range(8):
            wt = wpool.tile([128, 128], F16, tag=f"wdP{i}", name=f"wdP{i}")
            nc.sync.dma_start(out=wt, in_=wdP_in[i])
            wdP.append(wt)
        biasC = wpool.tile([128, 12], F32, tag="biasC")
        nc.sync.dma_start(out=biasC, in_=biasC_in)

        hT = wpool.tile([128, 32], F32, tag="selhT")
        hpk = wpool.tile([128, 64], F16, tag="selhpk")
        nc.vector.memset(hT, 0.0)
        nc.vector.memset(hpk, 0.0)
        marg = wpool.tile([1, TBL], F32, tag="marg")

        embH = embH_in.rearrange("p (k q) -> p k q", k=6)
        embL = embL_in.rearrange("p (k q) -> p k q", k=6)
        stages = [None] * NS
        et_h = [None] * NS
        et_l = [None] * NS

        def dma_et(s):
            eh = etpool.tile([128, 6, 256], F16, tag="eth", name=f"eth{s}")
            nc.sync.dma_start(out=eh, in_=embH[:, :, s * 256:(s + 1) * 256])
            el = etpool.tile([128, 6, 256], F16, tag="etl", name=f"etl{s}")
            nc.sync.dma_start(out=el, in_=embL[:, :, s * 256:(s + 1) * 256])
            et_h[s], et_l[s] = eh, el

        def pa_thunks(s):
            st = stpool.tile([128, 32, 12, 8], F32, tag="stage", name=f"stage{s}")
            stages[s] = st
            th = []
            prods = [(WihH, et_h[s]), (WihH, et_l[s]), (WihL, et_h[s])]
            for m in range(12):
                ps_box = []
                def mk_mm(m, p, ps_box):
                    if p == 0:
                        ps_box.append(pApool.tile([128, 256], F32, tag="psA",
                                                  name=f"psA1_{s}_{m}"))
                    W6, et = prods[p]
                    for k in range(6):
                        nc.tensor.matmul(ps_box[0], W6[k][:, m * 128:(m + 1) * 128],
                                         et[:, k, :], start=(p == 0 and k == 0),
                                         stop=(p == 2 and k == 5))
                def mk_act(m=m, ps_box=ps_box):
                    nc.scalar.activation(stages[s][:, :, m, :], ps_box[0], AF.Identity,
                                         bias=biasC[:, m:m + 1])
                for pi in range(3):
                    th.append((lambda m=m, p=pi, ps_box=ps_box: mk_mm(m, p, ps_box)))
                th.append(mk_act)
            return th

        dma_et(0)
        dma_et(1)
        pending = []
        for th in pa_thunks(0):
            th()
        dma_et(2)
        pending.extend(pa_thunks(1))

        for t in range(T):
            s, j = divmod(t, 32)
            if j == 0 and s >= 1:
                if s + 2 < NS:
                    dma_et(s + 2)
                if s + 1 < NS:
                    pending.extend(pa_thunks(s + 1))
            st = stages[s]
            ps_rz = ppoolB.tile([128, 128], F32, tag="ps_rz", name=f"psrz_{t}")
            ps_n = ppoolB.tile([128, 64], F32, tag="ps_n", name=f"psn_{t}")
            ps_m_t = ppoolB.tile([128, 16], F32, tag="ps_m", name=f"psm_{t}")
            for m in range(12):
                ps = ps_rz[:, m * 16:(m + 1) * 16] if m < 8 else ps_n[:, (m - 8) * 16:(m - 7) * 16]
                for k in range(4):
                    nc.tensor.matmul(ps, WTc[k][:, m * 128:(m + 1) * 128],
                                     hpk[:, k * 16:(k + 1) * 16],
                                     start=(k == 0), stop=(k == 3))
            rz_hi = ps_rz.rearrange("p (m s) -> p m s", s=16)[:, :, 0:8]
            rz_lo = ps_rz.rearrange("p (m s) -> p m s", s=16)[:, :, 8:16]
            a = tpool.tile([128, 64], F32, tag="a", name=f"a_{t}")
            nc.vector.tensor_add(out=a.rearrange("p (m s) -> p m s", s=8), in0=rz_hi,
                                 in1=st[:, j, 0:8, :])
            a2 = tpool.tile([128, 64], F32, tag="a2", name=f"a2_{t}")
            nc.vector.tensor_add(out=a2.rearrange("p (m s) -> p m s", s=8),
                                 in0=a.rearrange("p (m s) -> p m s", s=8), in1=rz_lo)
            rz = tpool.tile([128, 64], F32, tag="rz", name=f"rz_{t}")
            nc.scalar.activation(rz, a2, AF.Sigmoid)
            zc = tpool.tile([128, 32], F32, tag="zc", name=f"zc_{t}")
            nc.scalar.activation(zc, rz[:, 32:64], AF.Identity, bias=1.0, scale=-1.0)
            zh = tpool.tile([128, 32], F32, tag="zh", name=f"zh_{t}")
            nc.vector.tensor_mul(out=zh, in0=hT, in1=rz[:, 32:64])
            n_hi = ps_n.rearrange("p (m s) -> p m s", s=16)[:, :, 0:8]
            n_lo = ps_n.rearrange("p (m s) -> p m s", s=16)[:, :, 8:16]
            t2a = tpool.tile([128, 32], F32, tag="t2a", name=f"t2a_{t}")
            nc.vector.tensor_mul(out=t2a.rearrange("p (m s) -> p m s", s=8), in0=n_hi,
                                 in1=rz[:, 0:32].rearrange("p (m s) -> p m s", s=8))
            t2b = tpool.tile([128, 32], F32, tag="t2b", name=f"t2b_{t}")
            nc.vector.tensor_mul(out=t2b.rearrange("p (m s) -> p m s", s=8), in0=n_lo,
                                 in1=rz[:, 0:32].rearrange("p (m s) -> p m s", s=8))
            u1 = tpool.tile([128, 32], F32, tag="u1", name=f"u1_{t}")
            nc.vector.tensor_add(out=u1.rearrange("p (m s) -> p m s", s=8),
                                 in0=t2a.rearrange("p (m s) -> p m s", s=8),
                                 in1=st[:, j, 8:12, :])
            u = tpool.tile([128, 32], F32, tag="u", name=f"u_{t}")
            nc.vector.tensor_add(out=u, in0=u1, in1=t2b)
            nn_ = tpool.tile([128, 32], F32, tag="nn_", name=f"nn_{t}")
            nc.scalar.activation(nn_, u, AF.Tanh)
            v = tpool.tile([128, 32], F32, tag="v", name=f"v_{t}")
            nc.vector.tensor_mul(out=v, in0=nn_, in1=zc)
            nc.vector.tensor_add(out=hT, in0=v, in1=zh)
            hpk3 = hpk.rearrange("p (k s) -> p k s", s=16)
            hT3 = hT.rearrange("p (k s) -> p k s", s=8)
            nc.vector.tensor_copy(out=hpk3[:, :, 0:8], in_=hT3)
            nc.vector.tensor_sub(out=hpk3[:, :, 8:16], in0=hT3, in1=hpk3[:, :, 0:8])
            for k in range(4):
                nc.tensor.matmul(ps_m_t, wdP[k * 2], hpk[:, k * 16:(k + 1) * 16],
                                 start=(k == 0), stop=False)
                nc.tensor.matmul(ps_m_t, wdP[k * 2 + 1], hpk[:, k * 16:(k + 1) * 16],
                                 start=False, stop=(k == 3))
            mc = tpool.tile([1, 8], F32, tag="mc", name=f"mc_{t}")
            nc.scalar.copy(out=mc, in_=ps_m_t[0:1, 0:8])
            nc.vector.tensor_add(out=marg[:, t * 8:(t + 1) * 8], in0=mc,
                                 in1=ps_m_t[0:1, 8:16])
            nrun = -(-len(pending) // (32 - j))
            for _ in range(min(nrun, len(pending))):
                pending.pop(0)()
            if t % 64 == 63:
                nc.sync.dma_start(out=margins_out[bass.ds((t - 63) * 8, 512)],
                                  in_=marg[0:1, (t - 63) * 8:(t + 1) * 8])
    nc.compile()
    return nc


def build_neff1_v2():
    """Select-policy GRU scan, fully unrolled (512 steps), with the gi_c
    input projection (f16 hi/lo 3-product, ~fp32-exact) sprinkled into PE
    idle slots between scan steps. gi_rz is accumulated into the gate PSUM
    via identity matmuls; zc/zh run on GPSIMD; margins (wd . h) are 4 fp32
    matmuls per step + an ACT copy. h stays fp32 (select bits must be exact:
    ~20 flipped bits already cost 1.5e-2 rel err downstream)."""
    nc = bacc.Bacc("TRN2", target_bir_lowering=False, debug=False, num_devices=NC)
    TBL = T * BL
    embH_in = nc.dram_tensor("embH", [128, 6 * TBL], F16, kind="ExternalInput").ap()
    embL_in = nc.dram_tensor("embL", [128, 6 * TBL], F16, kind="ExternalInput").ap()
    WihH_in = nc.dram_tensor("WihH", [6, 128, 1536], F16, kind="ExternalInput").ap()
    WihL_in = nc.dram_tensor("WihL", [6, 128, 1536], F16, kind="ExternalInput").ap()
    WTc_in = nc.dram_tensor("WTc", [4, 128, 1536], F32, kind="ExternalInput").ap()
    wdT_in = nc.dram_tensor("wdT", [128, 4], F32, kind="ExternalInput").ap()
    identf_in = nc.dram_tensor("identf", [128, 128], F32, kind="ExternalInput").ap()
    biasC_in = nc.dram_tensor("biasC", [128, 12], F32, kind="ExternalInput").ap()
    margins_out = nc.dram_tensor("margins", [TBL], F32, kind="ExternalOutput").ap()

    NS = T // 32  # 16 sub-chunks

    with TileContext(nc) as tc, ExitStack() as ctx:
        wpool = ctx.enter_context(tc.tile_pool(name="w1", bufs=1))
        etpool = ctx.enter_context(tc.tile_pool(name="et1", bufs=3))
        stpool = ctx.enter_context(tc.tile_pool(name="st1", bufs=3))
        pApool = ctx.enter_context(tc.tile_pool(name="psA1", bufs=2, space="PSUM"))
        pSpool = ctx.enter_context(tc.tile_pool(name="psS1", bufs=2, space="PSUM"))
        tpool = ctx.enter_context(tc.tile_pool(name="tmp1s", bufs=3))

        WihH, WihL = [], []
        for k in range(6):
            wh = wpool.tile([128, 1536], F16, tag=f"wihH{k}", name=f"wihH{k}")
            nc.sync.dma_start(out=wh, in_=WihH_in[k])
            WihH.append(wh)
            wl = wpool.tile([128, 1536], F16, tag=f"wihL{k}", name=f"wihL{k}")
            nc.sync.dma_start(out=wl, in_=WihL_in[k])
            WihL.append(wl)
        WTc = []
        for k in range(4):
            wt = wpool.tile([128, 1536], F32, tag=f"wtc{k}", name=f"wtc{k}")
            nc.sync.dma_start(out=wt, in_=WTc_in[k])
            WTc.append(wt)
        wdT = wpool.tile([128, 4], F32, tag="wdT")
        nc.sync.dma_start(out=wdT, in_=wdT_in)
        identf = wpool.tile([128, 128], F32, tag="identf")
        nc.sync.dma_start(out=identf, in_=identf_in)
        biasC = wpool.tile([128, 12], F32, tag="biasC")
        nc.sync.dma_start(out=biasC, in_=biasC_in)

        hT = wpool.tile([128, 32], F32, tag="hT")
        nc.vector.memset(hT, 0.0)
        marg = wpool.tile([1, TBL], F32, tag="marg")

        embH = embH_in.rearrange("p (k q) -> p k q", k=6)
        embL = embL_in.rearrange("p (k q) -> p k q", k=6)
        stages = [None] * NS
        et_h = [None] * NS
        et_l = [None] * NS

        def dma_et(s):
            eh = etpool.tile([128, 6, 256], F16, tag="eth", name=f"eth{s}")
            nc.sync.dma_start(out=eh, in_=embH[:, :, s * 256:(s + 1) * 256])
            el = etpool.tile([128, 6, 256], F16, tag="etl", name=f"etl{s}")
            nc.sync.dma_start(out=el, in_=embL[:, :, s * 256:(s + 1) * 256])
            et_h[s], et_l[s] = eh, el

        def pa_thunks(s):
            st = stpool.tile([128, 32, 12, 8], F32, tag="stage", name=f"stage{s}")
            stages[s] = st
            th = []
            prods = [(WihH, et_h[s]), (WihH, et_l[s]), (WihL, et_h[s])]
            for m in range(12):
                ps_box = []
                def mk_mm(m=m, p=0, ps_box=ps_box):
                    if p == 0:
                        ps_box.append(pApool.tile([128, 256], F32, tag="psA",
                                                  name=f"psA1_{s}_{m}"))
                    W6, et = prods[p]
                    for k in range(6):
                        nc.tensor.matmul(ps_box[0], W6[k][:, m * 128:(m + 1) * 128],
                                         et[:, k, :], start=(p == 0 and k == 0),
                                         stop=(p == 2 and k == 5))
                def mk_act(m=m, ps_box=ps_box):
                    nc.scalar.activation(stages[s][:, :, m, :], ps_box[0], AF.Identity,
                                         bias=biasC[:, m:m + 1])
                for p in range(3):
                    th.append((lambda m=m, p=p, ps_box=ps_box: mk_mm(m, p, ps_box)))
                th.append(mk_act)
            return th

        def alloc_ident(t):
            s, j = divmod(t, 32)
            ps_rz = pSpool.tile([128, 112], F32, tag="ps_rz", name=f"psrz_{t}")
            nc.tensor.matmul(ps_rz[:, 0:64], identf, stages[s][:, j, 0:8, :],
                             start=True, stop=False)
            return ps_rz  # [128,112]: rz 0:64, n 64:96, margin row0 96:104

        # ---- prologue
        dma_et(0)
        dma_et(1)
        pending = []
        for th in pa_thunks(0):
            th()
        dma_et(2)
        pending.extend(pa_thunks(1))
        ps_next = alloc_ident(0)

        for t in range(T):
            s, j = divmod(t, 32)
            if j == 0 and s >= 1:
                if s + 2 < NS:
                    dma_et(s + 2)
                if s + 1 < NS:
                    pending.extend(pa_thunks(s + 1))
            ps_all = ps_next
            ps_rz = ps_all[:, 0:64]
            ps_n = ps_all[:, 64:96]
            ps_m = ps_all[0:1, 96:104]
            st = stages[s]
            for m in range(8):
                for k in range(4):
                    nc.tensor.matmul(ps_rz[:, m * 8:(m + 1) * 8],
                                     WTc[k][:, m * 128:(m + 1) * 128],
                                     hT[:, k * 8:(k + 1) * 8],
                                     start=False, stop=(k == 3))
            for m in range(4):
                for k in range(4):
                    nc.tensor.matmul(ps_n[:, m * 8:(m + 1) * 8],
                                     WTc[k][:, (8 + m) * 128:(9 + m) * 128],
                                     hT[:, k * 8:(k + 1) * 8],
                                     start=(k == 0), stop=(k == 3))
            rz = tpool.tile([128, 64], F32, tag="rz1", name=f"rz1_{t}")
            nc.scalar.activation(rz, ps_rz, AF.Sigmoid)
            zc = tpool.tile([128, 32], F32, tag="zc1", name=f"zc1_{t}")
            nc.vector.tensor_scalar(out=zc, in0=rz[:, 32:64], scalar1=-1.0,
                                    scalar2=1.0, op0=ALU.mult, op1=ALU.add)
            zh = tpool.tile([128, 32], F32, tag="zh1", name=f"zh1_{t}")
            nc.vector.tensor_mul(out=zh, in0=rz[:, 32:64], in1=hT)
            t2 = tpool.tile([128, 32], F32, tag="t2_1", name=f"t2_1_{t}")
            nc.vector.tensor_mul(out=t2, in0=ps_n, in1=rz[:, 0:32])
            u = tpool.tile([128, 32], F32, tag="u1", name=f"u1_{t}")
            nc.vector.tensor_add(out=u.rearrange("p (c b) -> p c b", c=4),
                                 in0=t2.rearrange("p (c b) -> p c b", c=4),
                                 in1=st[:, j, 8:12, :])
            nn = tpool.tile([128, 32], F32, tag="nn1", name=f"nn1_{t}")
            nc.scalar.activation(nn, u, AF.Tanh)
            v = tpool.tile([128, 32], F32, tag="v1", name=f"v1_{t}")
            nc.vector.tensor_mul(out=v, in0=nn, in1=zc)
            nc.vector.tensor_add(out=hT, in0=v, in1=zh)
            if t + 1 < T:
                ps_next = alloc_ident(t + 1)
            for k in range(4):
                nc.tensor.matmul(ps_m, wdT[:, k:k + 1], hT[:, k * 8:(k + 1) * 8],
                                 start=(k == 0), stop=(k == 3))
            nc.scalar.copy(out=marg[0:1, t * 8:(t + 1) * 8], in_=ps_m)
            nrun = -(-len(pending) // (32 - j))
            for _ in range(min(nrun, len(pending))):
                pending.pop(0)()
            if t % 64 == 63:
                nc.sync.dma_start(out=margins_out[bass.ds((t - 63) * 8, 512)],
                                  in_=marg[0:1, (t - 63) * 8:(t + 1) * 8])
    nc.compile()
    return nc


# ---------------------------------------------------------------- NEFF2 ----

def emit_layer_scan(nc, tc, ctx, name, WhT, gi_dram, mask, masku, ybuf, ycols, n_it):
    """Masked bf16 GRU scan. WhT: 4x sbuf [128,1536] bf16. gi_dram: [128, NITER*1536] bf16.
    mask: sbuf [128, T*BL] bf16 (1/0). ybuf: sbuf [128, 4*ycols] bf16 out (col c*ycols + t*8+b)."""
    spool = ctx.enter_context(tc.tile_pool(name=f"{name}st", bufs=1))
    gpool = ctx.enter_context(tc.tile_pool(name=f"{name}gi", bufs=3))
    ppool = ctx.enter_context(tc.tile_pool(name=f"{name}ps", bufs=2, space="PSUM"))
    tpool = ctx.enter_context(tc.tile_pool(name=f"{name}tmp", bufs=3))

    h16 = spool.tile([128, 32], BF16, tag=f"{name}h16")
    nc.vector.memset(h16, 0.0)
    yb4 = ybuf.rearrange("p (c q) -> p c q", c=4)

    with tc.For_i(0, n_it, 1, hint_engines=(PE, DVE, ACT)) as it:
        gi = gpool.tile([128, UNROLL * 96], BF16, tag=f"{name}gi")
        nc.sync.dma_start(out=gi, in_=gi_dram[:, bass.ds(it * (UNROLL * 96), UNROLL * 96)])
        for j in range(UNROLL):
            tcol = it * UNROLL * 8 + j * 8
            ps_rz = ppool.tile([128, 64], F32, tag=f"{name}ps_rz")
            ps_n = ppool.tile([128, 32], F32, tag=f"{name}ps_n")
            for m in range(12):
                ps = ps_rz[:, m * 8:(m + 1) * 8] if m < 8 else ps_n[:, (m - 8) * 8:(m - 7) * 8]
                for k in range(4):
                    nc.tensor.matmul(ps, WhT[k][:, m * 128:(m + 1) * 128],
                                     h16[:, k * 8:(k + 1) * 8],
                                     start=(k == 0), stop=(k == 3))
            gslice = gi[:, j * 96:(j + 1) * 96]
            a = tpool.tile([128, 64], F32, tag=f"{name}a")
            nc.vector.tensor_add(out=a, in0=ps_rz, in1=gslice[:, 0:64])
            rz = tpool.tile([128, 64], F32, tag=f"{name}rz")
            nc.scalar.activation(rz, a, AF.Sigmoid)
            zc = tpool.tile([128, 32], F32, tag=f"{name}zc")
            nc.vector.tensor_scalar(out=zc, in0=rz[:, 32:64], scalar1=-1.0,
                                    scalar2=1.0, op0=ALU.mult, op1=ALU.add)
            zh = tpool.tile([128, 32], F32, tag=f"{name}zh")
            nc.vector.tensor_mul(out=zh, in0=h16, in1=rz[:, 32:64])
            t2 = tpool.tile([128, 32], F32, tag=f"{name}t2")
            nc.vector.tensor_mul(out=t2, in0=ps_n, in1=rz[:, 0:32])
            u = tpool.tile([128, 32], F32, tag=f"{name}u")
            nc.vector.tensor_add(out=u, in0=t2, in1=gslice[:, 64:96])
            nn_ = tpool.tile([128, 32], F32, tag=f"{name}nn")
            nc.scalar.activation(nn_, u, AF.Tanh)
            v = tpool.tile([128, 32], F32, tag=f"{name}v")
            nc.vector.tensor_mul(out=v, in0=nn_, in1=zc)
            hn16 = tpool.tile([128, 32], BF16, tag=f"{name}hn16")
            nc.vector.tensor_add(out=hn16, in0=v, in1=zh)
            mview = mask[:, bass.ds(tcol, 8)].unsqueeze(1).broadcast_to([128, 4, 8])
            muview = masku[:, bass.ds(tcol, 8)].unsqueeze(1).broadcast_to([128, 4, 8])
            hn3 = hn16.rearrange("p (c b) -> p c b", c=4)
            # y = m * h'  (zero where invalid)
            nc.vector.tensor_mul(out=yb4[:, :, bass.ds(tcol, 8)], in0=hn3, in1=mview)
            # h <- m ? h' : h
            nc.vector.copy_predicated(out=h16.rearrange("p (c b) -> p c b", c=4),
                                      mask=muview, data=hn3)


def build_neff2_v2(t_pad, n_full, dump=False):
    """Fused L0+L1 GRU scans in 32-step sub-chunks with software pipelining:
    tick s: L0 chunk s | L1 chunk s-2, with gi0 proj (s+1), gi1 proj (s-1),
    and conv (s-3) matmuls sprinkled into PE idle between scan steps.
    Steps below 32*n_full skip all masking; h state lives in-place in the
    y buffer so the GRU update writes y directly.
    """
    nc = bacc.Bacc("TRN2", target_bir_lowering=False, debug=False, num_devices=NC)
    TB = t_pad * BL
    nsub = t_pad // 32
    TBP = (t_pad + 16) * BL
    nembT_in = nc.dram_tensor("nembT", [128, 6 * TB], BF16, kind="ExternalInput").ap()
    mask_in = nc.dram_tensor("maskf", [128, TB], BF16, kind="ExternalInput").ap()
    masku_in = nc.dram_tensor("masku", [128, TB], mybir.dt.uint8, kind="ExternalInput").ap()
    Wih0T_in = nc.dram_tensor("Wih0T", [6, 128, 1536], BF16, kind="ExternalInput").ap()
    WhT0_in = nc.dram_tensor("WhT0", [4, 128, 1536], BF16, kind="ExternalInput").ap()
    Wih1T_in = nc.dram_tensor("Wih1T", [4, 128, 1536], BF16, kind="ExternalInput").ap()
    WhT1_in = nc.dram_tensor("WhT1", [4, 128, 1536], BF16, kind="ExternalInput").ap()
    bias0_in = nc.dram_tensor("bias0", [128, 12], F32, kind="ExternalInput").ap()
    bias1_in = nc.dram_tensor("bias1", [128, 12], F32, kind="ExternalInput").ap()
    identb_in = nc.dram_tensor("identb", [128, 128], BF16, kind="ExternalInput").ap()
    Wconv_in = nc.dram_tensor("Wconv", [128, 12 * 4 * 256], BF16, kind="ExternalInput").ap()
    bconv_in = nc.dram_tensor("bconv", [128, 6], F32, kind="ExternalInput").ap()
    WoT_in = nc.dram_tensor("WoT", [128, 6], F32, kind="ExternalInput").ap()
    bo_in = nc.dram_tensor("bo", [1, 1], F32, kind="ExternalInput").ap()
    out_dram = nc.dram_tensor("out", [1, BL], F32, kind="ExternalOutput").ap()
    if dump:
        TBP_ = (t_pad + 16) * BL
        y0d_out = nc.dram_tensor("y0d", [128, 4 * (t_pad * BL + 8)], BF16, kind="ExternalOutput").ap()
        y1d_out = nc.dram_tensor("y1d", [128, 4 * (TBP_ + 8)], BF16, kind="ExternalOutput").ap()
        g0d_out = nc.dram_tensor("g0d", [128, 32 * 12 * 8], BF16, kind="ExternalOutput").ap()
        s0d_out = nc.dram_tensor("s0d", [128, 96 + 64 + 6 * 32], F32, kind="ExternalOutput").ap()

    with TileContext(nc) as tc, ExitStack() as ctx:
        wpool = ctx.enter_context(tc.tile_pool(name="w2", bufs=1))
        etpool = ctx.enter_context(tc.tile_pool(name="et2", bufs=3))
        g0pool = ctx.enter_context(tc.tile_pool(name="g0st", bufs=2))
        g1pool = ctx.enter_context(tc.tile_pool(name="g1st", bufs=2))
        pApool = ctx.enter_context(tc.tile_pool(name="psA2", bufs=2, space="PSUM"))
        pBpool = ctx.enter_context(tc.tile_pool(name="psB2", bufs=2, space="PSUM"))
        pC0 = ctx.enter_context(tc.tile_pool(name="psL0", bufs=2, space="PSUM"))
        pC1 = ctx.enter_context(tc.tile_pool(name="psL1", bufs=2, space="PSUM"))
        t0pool = ctx.enter_context(tc.tile_pool(name="tmp0", bufs=3))
        t1pool = ctx.enter_context(tc.tile_pool(name="tmp1", bufs=3))

        def loadw(name, src, n, dtype=BF16):
            out = []
            for k in range(n):
                wt = wpool.tile([128, 1536], dtype, tag=f"{name}{k}", name=f"{name}{k}")
                nc.sync.dma_start(out=wt, in_=src[k])
                out.append(wt)
            return out

        Wih0T = loadw("wih0", Wih0T_in, 6)
        WhT0 = loadw("wh0", WhT0_in, 4)
        Wih1T = loadw("wih1", Wih1T_in, 4)
        WhT1 = loadw("wh1", WhT1_in, 4)
        bias0 = wpool.tile([128, 12], F32, tag="bias0")
        nc.sync.dma_start(out=bias0, in_=bias0_in)
        bias1 = wpool.tile([128, 12], F32, tag="bias1")
        nc.sync.dma_start(out=bias1, in_=bias1_in)
        identb = wpool.tile([128, 128], BF16, tag="identb")
        nc.sync.dma_start(out=identb, in_=identb_in)
        Wconv_t = wpool.tile([128, 12 * 4 * 256], BF16, tag="Wconv")
        nc.sync.dma_start(out=Wconv_t, in_=Wconv_in)
        Wconv = Wconv_t.rearrange("p (d k c) -> p d k c", d=12, k=4)
        bconv = wpool.tile([128, 6], F32, tag="bconv")
        nc.sync.dma_start(out=bconv, in_=bconv_in)
        WoT = wpool.tile([128, 6], F32, tag="WoT")
        nc.sync.dma_start(out=WoT, in_=WoT_in)
        bo_sb = wpool.tile([1, 1], F32, tag="bo_sb")
        nc.sync.dma_start(out=bo_sb, in_=bo_in)
        maskf = wpool.tile([128, TB], BF16, tag="maskf")
        nc.sync.dma_start(out=maskf, in_=mask_in)
        masku = wpool.tile([128, TB], mybir.dt.uint8, tag="masku")
        nc.sync.dma_start(out=masku, in_=masku_in)

        # y buffers: col (t+1)*8+b per k-plane; slot 0 = zeroed h(-1)
        y0buf = wpool.tile([128, 4 * (TB + 8)], BF16, tag="y0buf")
        y1buf = wpool.tile([128, 4 * (TBP + 8)], BF16, tag="y1buf")
        y0 = y0buf.rearrange("p (c q) -> p c q", c=4)
        y1 = y1buf.rearrange("p (c q) -> p c q", c=4)
        for k in range(4):
            nc.vector.memset(y0buf[:, k * (TB + 8):k * (TB + 8) + 8], 0.0)
            nc.vector.memset(y1buf[:, k * (TBP + 8):k * (TBP + 8) + 8], 0.0)
            nc.vector.memset(y1buf[:, k * (TBP + 8) + 8 + TB:(k + 1) * (TBP + 8)], 0.0)
        h16_0 = wpool.tile([128, 32], BF16, tag="h16_0")
        h16_1 = wpool.tile([128, 32], BF16, tag="h16_1")

        nembT = nembT_in.rearrange("p (k q) -> p k q", k=6)
        stage0 = [None] * nsub
        stage1 = [None] * nsub
        et_tiles = [None] * nsub

        def dma_et(s):
            et = etpool.tile([128, 6, 256], BF16, tag="et", name=f"et{s}")
            nc.sync.dma_start(out=et, in_=nembT[:, :, s * 256:(s + 1) * 256])
            et_tiles[s] = et

        def gi0_thunks(s):
            st = g0pool.tile([128, 32, 12, 8], BF16, tag="g0", name=f"g0_{s}")
            stage0[s] = st
            th = []
            et = et_tiles[s]
            for m in range(12):
                def mk(m=m):
                    ps = pApool.tile([128, 256], F32, tag="psA", name=f"psA_{s}_{m}")
                    for k in range(6):
                        nc.tensor.matmul(ps, Wih0T[k][:, m * 128:(m + 1) * 128],
                                         et[:, k, :], start=(k == 0), stop=(k == 5))
                    nc.scalar.activation(stage0[s][:, :, m, :], ps, AF.Identity,
                                         bias=bias0[:, m:m + 1])
                th.append(mk)
            return th

        def gi1_thunks(s):
            st = g1pool.tile([128, 32, 12, 8], BF16, tag="g1", name=f"g1_{s}")
            stage1[s] = st
            th = []
            for m in range(12):
                def mk(m=m):
                    ps = pApool.tile([128, 256], F32, tag="psA", name=f"psA1_{s}_{m}")
                    for k in range(4):
                        nc.tensor.matmul(ps, Wih1T[k][:, m * 128:(m + 1) * 128],
                                         y0[:, k, (32 * s + 1) * 8:(32 * s + 33) * 8],
                                         start=(k == 0), stop=(k == 3))
                    nc.scalar.activation(stage1[s][:, :, m, :], ps, AF.Identity,
                                         bias=bias1[:, m:m + 1])
                th.append(mk)
            return th

        dt_base = {3: 0, 4: 3, 5: 7}
        maccs = {}
        for fi, fs in enumerate(FS):
            for mt in range(2):
                macc = wpool.tile([128, 8], F32, tag=f"macc{fi}{mt}")
                nc.vector.memset(macc, -1e30)
                maccs[(fi, mt)] = macc

        def conv_thunks(c):
            th = []
            for fi, fs in enumerate(FS):
                for mt in range(2):
                    def mk(fi=fi, fs=fs, mt=mt):
                        ps = pBpool.tile([128, 256], F32, tag="psCV", name=f"psCV_{c}_{fi}_{mt}")
                        first = True
                        for dt in range(fs):
                            for k in range(4):
                                nc.tensor.matmul(
                                    ps, Wconv[:, dt_base[fs] + dt, k, mt * 128:(mt + 1) * 128],
                                    y1[:, k, (32 * c + dt + 1) * 8:(32 * c + dt + 33) * 8],
                                    start=first, stop=(dt == fs - 1 and k == 3))
                                first = False
                        nvalid = 32 if c < nsub - 1 else 33 - fs
                        cm = t0pool.tile([128, 8], F32, tag="cvcm", name=f"cvcm_{c}_{fi}_{mt}")
                        nc.vector.tensor_reduce(
                            out=cm, in_=ps.rearrange("p (t b) -> p b t", t=32)[:, :, 0:nvalid],
                            axis=mybir.AxisListType.X, op=ALU.max)
                        nc.vector.tensor_max(out=maccs[(fi, mt)], in0=maccs[(fi, mt)], in1=cm)
                    th.append(mk)
            return th

        def l_step(l, t, stage_l, WhT, ppool, tpool, y, h16):
            masked = t >= 32 * n_full
            s, j = t // 32, t % 32
            st = stage_l[s]
            if masked and t == 32 * n_full:
                nc.vector.tensor_copy(out=h16.rearrange("p (c b) -> p c b", c=4),
                                      in_=y[:, :, t * 8:(t + 1) * 8])
            if masked:
                hp = [h16[:, k * 8:(k + 1) * 8] for k in range(4)]
                hp3 = h16.rearrange("p (c b) -> p c b", c=4)
            else:
                hp = [y[:, k, t * 8:(t + 1) * 8] for k in range(4)]
                hp3 = y[:, :, t * 8:(t + 1) * 8]
            ps_all = ppool.tile([128, 96], F32, tag=f"psg{l}", name=f"psg{l}_{t}")
            ps_rz = ps_all[:, 0:64]
            ps_n = ps_all[:, 64:96]
            # single identity matmul: psum zero-region semantics require exactly
            # one start=True writer per region before the accumulates
            nc.tensor.matmul(ps_rz, identb, st[:, j, 0:8, :], start=True, stop=False)
            for m in range(8):
                for k in range(4):
                    nc.tensor.matmul(ps_rz[:, m * 8:(m + 1) * 8],
                                     WhT[k][:, m * 128:(m + 1) * 128], hp[k],
                                     start=False, stop=(k == 3))
            for m in range(4):
                for k in range(4):
                    nc.tensor.matmul(ps_n[:, m * 8:(m + 1) * 8],
                                     WhT[k][:, (8 + m) * 128:(9 + m) * 128], hp[k],
                                     start=(k == 0), stop=(k == 3))
            yield  # stage 0: PE emitted
            rz = tpool.tile([128, 64], F32, tag=f"rz{l}", name=f"rz{l}_{t}")
            nc.scalar.activation(rz, ps_rz, AF.Sigmoid)
            yield  # stage 1: sigmoid emitted
            t2 = tpool.tile([128, 32], F32, tag=f"t2{l}", name=f"t2{l}_{t}")
            nc.vector.tensor_mul(out=t2, in0=ps_n, in1=rz[:, 0:32])
            u = tpool.tile([128, 32], F32, tag=f"u{l}", name=f"u{l}_{t}")
            nc.vector.tensor_add(out=u.rearrange("p (c b) -> p c b", c=4),
                                 in0=t2.rearrange("p (c b) -> p c b", c=4),
                                 in1=st[:, j, 8:12, :])
            zc = tpool.tile([128, 32], F32, tag=f"zc{l}", name=f"zc{l}_{t}")
            nc.vector.tensor_scalar(out=zc, in0=rz[:, 32:64], scalar1=-1.0,
                                    scalar2=1.0, op0=ALU.mult, op1=ALU.add)
            zh = tpool.tile([128, 32], F32, tag=f"zh{l}", name=f"zh{l}_{t}")
            nc.vector.tensor_mul(out=zh.rearrange("p (c b) -> p c b", c=4),
                                 in0=rz[:, 32:64].rearrange("p (c b) -> p c b", c=4),
                                 in1=hp3)
            yield  # stage 2: critical DVE (t2,u) emitted
            nn = tpool.tile([128, 32], F32, tag=f"nn{l}", name=f"nn{l}_{t}")
            nc.scalar.activation(nn, u, AF.Tanh)
            yield  # stage 3: tanh emitted
            v = tpool.tile([128, 32], F32, tag=f"v{l}", name=f"v{l}_{t}")
            nc.vector.tensor_mul(out=v, in0=nn, in1=zc)
            if dump and l == 0 and t == 0:
                dbg = wpool.tile([128, 96 + 64 + 6 * 32], F32, tag="dbgs0")
                nc.vector.tensor_copy(out=dbg[:, 0:96], in_=ps_all)
                nc.vector.tensor_copy(out=dbg[:, 96:160], in_=rz)
                for i, x in enumerate((zc, zh, t2, u, nn, v)):
                    nc.vector.tensor_copy(out=dbg[:, 160 + i * 32:160 + (i + 1) * 32], in_=x)
                nc.sync.dma_start(out=s0d_out, in_=dbg)
            ynew = y[:, :, (t + 1) * 8:(t + 2) * 8]
            if not masked:
                nc.vector.tensor_add(out=ynew, in0=v.rearrange("p (c b) -> p c b", c=4),
                                     in1=zh.rearrange("p (c b) -> p c b", c=4))
            else:
                hn16 = tpool.tile([128, 32], BF16, tag=f"hn{l}", name=f"hn{l}_{t}")
                nc.vector.tensor_add(out=hn16, in0=v, in1=zh)
                hn3 = hn16.rearrange("p (c b) -> p c b", c=4)
                mview = maskf[:, t * 8:(t + 1) * 8].unsqueeze(1).broadcast_to([128, 4, 8])
                muview = masku[:, t * 8:(t + 1) * 8].unsqueeze(1).broadcast_to([128, 4, 8])
                nc.vector.tensor_mul(out=ynew, in0=hn3, in1=mview)
                nc.vector.copy_predicated(out=h16.rearrange("p (c b) -> p c b", c=4),
                                          mask=muview, data=hn3)

        # ---- pipeline emission
        dma_et(0)
        if nsub > 1:
            dma_et(1)
        for th in gi0_thunks(0):
            th()
        if dump:
            nc.sync.dma_start(out=g0d_out, in_=stage0[0])
        pending = []
        for s in range(nsub + 3):
            if s + 2 < nsub:
                dma_et(s + 2)
            if s + 1 < nsub:
                pending.extend(gi0_thunks(s + 1))
            if 0 <= s - 1 < nsub:
                pending.extend(gi1_thunks(s - 1))
            for j in range(32):
                if j == 6 and 0 <= s - 3 < nsub:
                    # conv chunk s-3 reads y1 of chunk s-2 steps 0..4, whose
                    # writes are emitted at j=0..4 of this tick
                    pending.extend(conv_thunks(s - 3))
                # drive both layers' steps stage-interleaved so the per-engine
                # FIFO order is [PE0,PE1][sig0,sig1][dve0,dve1][tanh0,tanh1]
                # [tail0,tail1] instead of serializing the two chains
                gens = []
                if s < nsub:
                    gens.append(l_step(0, 32 * s + j, stage0, WhT0, pC0, t0pool, y0, h16_0))
                if 0 <= s - 2 < nsub:
                    gens.append(l_step(1, 32 * (s - 2) + j, stage1, WhT1, pC1, t1pool, y1, h16_1))
                for _ in range(5):
                    for g in gens:
                        next(g, None)
                nrun = -(-len(pending) // (32 - j))
                for _ in range(min(nrun, len(pending))):
                    pending.pop(0)()

        # ---- epilogue: relu+bias pool, output linear
        pooled = wpool.tile([128, 48], F32, tag="pooled")
        for fi, fs in enumerate(FS):
            for mt in range(2):
                ci = fi * 2 + mt
                macc = maccs[(fi, mt)]
                if t_pad < T:
                    nc.vector.tensor_scalar_max(out=macc, in0=macc, scalar1=0.0)
                nc.scalar.activation(pooled[:, ci * 8:(ci + 1) * 8], macc, AF.Relu,
                                     bias=bconv[:, ci:ci + 1])
        ps_o_t = pApool.tile([128, 256], F32, tag="psA", name="ps_o_t")
        ps_o = ps_o_t[0:1, 0:8]
        for ci in range(6):
            nc.tensor.matmul(ps_o, WoT[:, ci:ci + 1], pooled[:, ci * 8:(ci + 1) * 8],
                             start=(ci == 0), stop=(ci == 5))
        ov = wpool.tile([1, BL], F32, tag="ov")
        nc.vector.tensor_scalar(out=ov, in0=ps_o, scalar1=bo_sb[0:1, 0:1], scalar2=None, op0=ALU.add)
        nc.sync.dma_start(out=out_dram, in_=ov)
        if dump:
            nc.sync.dma_start(out=y0d_out, in_=y0buf)
            nc.sync.dma_start(out=y1d_out, in_=y1buf)
    nc.compile()
    return nc


def build_neff2(t_pad):
    nc = bacc.Bacc("TRN2", target_bir_lowering=False, debug=False, num_devices=NC)
    TB = t_pad * BL
    NCH = t_pad // 64
    NIT2 = t_pad // UNROLL
    nembT_in = nc.dram_tensor("nembT", [128, 6 * TB], BF16, kind="ExternalInput").ap()
    mask_in = nc.dram_tensor("maskf", [128, TB], BF16, kind="ExternalInput").ap()
    masku_in = nc.dram_tensor("masku", [128, TB], mybir.dt.uint8, kind="ExternalInput").ap()
    Wih0T_in = nc.dram_tensor("Wih0T", [6, 128, 1536], BF16, kind="ExternalInput").ap()
    WhT0_in = nc.dram_tensor("WhT0", [4, 128, 1536], BF16, kind="ExternalInput").ap()
    Wih1T_in = nc.dram_tensor("Wih1T", [4, 128, 1536], BF16, kind="ExternalInput").ap()
    WhT1_in = nc.dram_tensor("WhT1", [4, 128, 1536], BF16, kind="ExternalInput").ap()
    bias0_in = nc.dram_tensor("bias0", [128, 12], F32, kind="ExternalInput").ap()
    bias1_in = nc.dram_tensor("bias1", [128, 12], F32, kind="ExternalInput").ap()
    Wconv_in = nc.dram_tensor("Wconv", [128, 12 * 4 * 256], BF16, kind="ExternalInput").ap()
    bconv_in = nc.dram_tensor("bconv", [128, 6], F32, kind="ExternalInput").ap()
    WoT_in = nc.dram_tensor("WoT", [128, 6], F32, kind="ExternalInput").ap()
    bo_in = nc.dram_tensor("bo", [1, 1], F32, kind="ExternalInput").ap()
    out_dram = nc.dram_tensor("out", [1, BL], F32, kind="ExternalOutput").ap()

    TPAD = t_pad + 16

    with TileContext(nc) as tc, ExitStack() as ctx:
        wpool = ctx.enter_context(tc.tile_pool(name="w2", bufs=1))
        dpool = ctx.enter_context(tc.tile_pool(name="dram2", bufs=1, space="DRAM"))
        gi0d = dpool.tile([128, NIT2 * UNROLL * 96], BF16, tag="gi0d")
        gi1d = dpool.tile([128, NIT2 * UNROLL * 96], BF16, tag="gi1d")

        def load_w(name, src, n, dtype=BF16):
            out = []
            for k in range(n):
                wt = wpool.tile([128, 1536], dtype, tag=f"{name}{k}")
                nc.sync.dma_start(out=wt, in_=src[k])
                out.append(wt)
            return out

        Wih0T = load_w("wih0", Wih0T_in, 6)
        WhT0 = load_w("wh0", WhT0_in, 4)
        Wih1T = load_w("wih1", Wih1T_in, 4)
        WhT1 = load_w("wh1", WhT1_in, 4)
        bias0 = wpool.tile([128, 12], F32, tag="bias0")
        nc.sync.dma_start(out=bias0, in_=bias0_in)
        bias1 = wpool.tile([128, 12], F32, tag="bias1")
        nc.sync.dma_start(out=bias1, in_=bias1_in)
        maskf = wpool.tile([128, TB], BF16, tag="maskf")
        nc.sync.dma_start(out=maskf, in_=mask_in)
        masku = wpool.tile([128, TB], mybir.dt.uint8, tag="masku")
        nc.sync.dma_start(out=masku, in_=masku_in)

        # --- gi0 = Wih0 @ nembT + bias0  (nembT streamed per chunk)
        with tc.tile_pool(name="nemb2", bufs=2) as npool, tc.tile_pool(name="st2", bufs=2) as stpool, tc.tile_pool(name="psg0", bufs=2, space="PSUM") as ppool:
            for nch in range(NCH):
                net = npool.tile([128, 6, 512], BF16, tag="net")
                nc.sync.dma_start(out=net, in_=nembT_in.rearrange("p (k c) -> p k c", k=6)[:, :, nch * 512:(nch + 1) * 512])
                stage = stpool.tile([128, 6144], BF16, tag="stage0")
                st4 = stage.rearrange("p (i j c b) -> p i j c b", i=I4, j=UNROLL, c=12)
                for m in range(12):
                    ps = ppool.tile([128, 512], F32, tag="ps_gi0")
                    for k in range(6):
                        nc.tensor.matmul(ps, Wih0T[k][:, m * 128:(m + 1) * 128],
                                         net[:, k, :],
                                         start=(k == 0), stop=(k == 5))
                    nc.vector.tensor_scalar(
                        out=st4[:, :, :, m, :],
                        in0=ps.rearrange("p (i j b) -> p i j b", i=I4, j=UNROLL),
                        scalar1=bias0[:, m:m + 1], scalar2=None, op0=ALU.add)
                nc.sync.dma_start(out=gi0d[:, nch * 6144:(nch + 1) * 6144], in_=stage)

        with tc.tile_pool(name="y0p", bufs=1) as y0pool:
            y0buf = y0pool.tile([128, 4 * TB], BF16, tag="y0buf")
            # --- L0 scan
            with ExitStack() as c0:
                emit_layer_scan(nc, tc, c0, "L0", WhT0, gi0d, maskf, masku, y0buf, TB, NIT2)

            # --- gi1 = Wih1 @ y0 + bias1
            y04 = y0buf.rearrange("p (c q) -> p c q", c=4)
            with tc.tile_pool(name="st3", bufs=2) as stpool, tc.tile_pool(name="psg1", bufs=2, space="PSUM") as ppool:
                for nch in range(NCH):
                    stage = stpool.tile([128, 6144], BF16, tag="stage1")
                    st4 = stage.rearrange("p (i j c b) -> p i j c b", i=I4, j=UNROLL, c=12)
                    for m in range(12):
                        ps = ppool.tile([128, 512], F32, tag="ps_gi1")
                        for k in range(4):
                            nc.tensor.matmul(ps, Wih1T[k][:, m * 128:(m + 1) * 128],
                                             y04[:, k, nch * 512:(nch + 1) * 512],
                                             start=(k == 0), stop=(k == 3))
                        nc.vector.tensor_scalar(
                            out=st4[:, :, :, m, :],
                            in0=ps.rearrange("p (i j b) -> p i j b", i=I4, j=UNROLL),
                            scalar1=bias1[:, m:m + 1], scalar2=None, op0=ALU.add)
                    nc.sync.dma_start(out=gi1d[:, nch * 6144:(nch + 1) * 6144], in_=stage)

        # --- L1 scan (padded y buffer for conv reads)
        y1buf = wpool.tile([128, 4 * TPAD * BL], BF16, tag="y1buf")
        nc.vector.memset(y1buf, 0.0)
        with ExitStack() as c1:
            emit_layer_scan(nc, tc, c1, "L1", WhT1, gi1d, maskf, masku, y1buf, TPAD * BL, NIT2)

        # --- convs + maxpool + relu + output linear
        Wconv_t = wpool.tile([128, 12 * 4 * 256], BF16, tag="Wconv")
        nc.sync.dma_start(out=Wconv_t, in_=Wconv_in)
        Wconv = Wconv_t.rearrange("p (d k c) -> p d k c", d=12, k=4)
        bconv = wpool.tile([128, 6], F32, tag="bconv")
        nc.sync.dma_start(out=bconv, in_=bconv_in)
        WoT = wpool.tile([128, 6], F32, tag="WoT")
        nc.sync.dma_start(out=WoT, in_=WoT_in)
        cpool = ctx.enter_context(tc.tile_pool(name="cv", bufs=2))
        ppool = ctx.enter_context(tc.tile_pool(name="pscv", bufs=2, space="PSUM"))
        pooled = wpool.tile([128, 48], F32, tag="pooled")
        y14 = y1buf.rearrange("p (c q) -> p c q", c=4)
        dt_base = {3: 0, 4: 3, 5: 7}
        for fi, fs in enumerate(FS):
            for mt in range(2):
                ci = fi * 2 + mt
                macc = cpool.tile([128, 8], F32, tag="macc")
                nc.vector.memset(macc, -1e30)
                for nch in range(NCH):
                    ps = ppool.tile([128, 512], F32, tag="ps_cv")
                    first = True
                    for dt in range(fs):
                        for k in range(4):
                            nc.tensor.matmul(
                                ps, Wconv[:, dt_base[fs] + dt, k, mt * 128:(mt + 1) * 128],
                                y14[:, k, nch * 512 + dt * 8: nch * 512 + dt * 8 + 512],
                                start=first, stop=(dt == fs - 1 and k == 3))
                            first = False
                    nvalid = 64 if nch < NCH - 1 else 65 - fs
                    cm = cpool.tile([128, 8], F32, tag="cm")
                    nc.vector.tensor_reduce(
                        out=cm, in_=ps.rearrange("p (t b) -> p b t", t=64)[:, :, 0:nvalid],
                        axis=mybir.AxisListType.X, op=ALU.max)
                    nc.vector.tensor_max(out=macc, in0=macc, in1=cm)
                if t_pad < T:
                    # windows beyond t_pad read all-zero y -> conv value exactly 0
                    nc.vector.tensor_scalar_max(out=macc, in0=macc, scalar1=0.0)
                nc.scalar.activation(pooled[:, ci * 8:(ci + 1) * 8], macc, AF.Relu,
                                     bias=bconv[:, ci:ci + 1])
        ps_o_t = ppool.tile([128, 8], F32, tag="ps_o")
        ps_o = ps_o_t[0:1, :]
        for ci in range(6):
            nc.tensor.matmul(ps_o, WoT[:, ci:ci + 1], pooled[:, ci * 8:(ci + 1) * 8],
                             start=(ci == 0), stop=(ci == 5))
        bo_sb = wpool.tile([1, 1], F32, tag="bo_sb")
        nc.sync.dma_start(out=bo_sb, in_=bo_in)
        ov = wpool.tile([1, BL], F32, tag="ov")
        nc.vector.tensor_scalar(out=ov, in0=ps_o, scalar1=bo_sb[0:1, 0:1], scalar2=None, op0=ALU.add)
        nc.sync.dma_start(out=out_dram, in_=ov)
    nc.compile()
    return nc


def _make_runner(nc, n_cores):
    import jax
    from jax.sharding import Mesh, PartitionSpec
    from jax.experimental.shard_map import shard_map
    import concourse.bass2jax as b2j
    b2j.install_neuronx_cc_hook()
    pname = nc.partition_id_tensor.name if nc.partition_id_tensor else None
    in_names, out_names, out_avals, zero_outs = [], [], [], []
    for alloc in nc.m.functions[0].allocations:
        if not isinstance(alloc, mybir.MemoryLocationSet):
            continue
        name = alloc.memorylocations[0].name
        if alloc.kind == "ExternalInput":
            if name != pname:
                in_names.append(name)
        elif alloc.kind == "ExternalOutput":
            out_names.append(name)
            shape = tuple(alloc.tensor_shape)
            dtype = mybir.dt.np(alloc.dtype)
            out_avals.append(jax.core.ShapedArray(shape, dtype))
            zero_outs.append(np.zeros(shape, dtype))
    n_params, n_outs = len(in_names), len(out_avals)
    all_in = list(in_names) + list(out_names) + ([pname] if pname else [])
    donate = tuple(range(n_params, n_params + n_outs))

    def _body(*args):
        operands = list(args)
        if pname is not None:
            operands.append(b2j.partition_id_tensor())
        outs = b2j._bass_exec_p.bind(
            *operands, out_avals=tuple(out_avals), in_names=tuple(all_in),
            out_names=tuple(out_names), lowering_input_output_aliases=(),
            sim_require_finite=True, sim_require_nnan=True, nc=nc)
        return tuple(outs)

    mesh = Mesh(np.asarray(jax.devices()[:n_cores]), ("core",))
    fn = jax.jit(shard_map(_body, mesh=mesh,
                           in_specs=(PartitionSpec("core"),) * (n_params + n_outs),
                           out_specs=(PartitionSpec("core"),) * n_outs, check_rep=False),
                 donate_argnums=donate, keep_unused=True)

    def run(in_maps):
        import jax
        per_core = [[np.asarray(m[name]) for name in in_names] for m in in_maps]
        concat_in = [np.concatenate([per_core[c][i] for c in range(n_cores)], axis=0)
                     for i in range(n_params)]
        zeros = [np.zeros((n_cores * z.shape[0], *z.shape[1:]), z.dtype) for z in zero_outs]
        out_arrs = fn(*concat_in, *zeros)
        jax.block_until_ready(out_arrs)
        return [{name: np.asarray(out_arrs[i]).reshape(n_cores, *out_avals[i].shape)[c]
                 for i, name in enumerate(out_names)} for c in range(n_cores)]

    def bench(in_maps, iters=10, slope=True):
        """Median wall time per call with device-resident inputs (s)."""
        import jax, time
        from jax.sharding import NamedSharding, PartitionSpec
        if getattr(bench, "_key", None) is id(in_maps):
            dev_in = bench._dev_in
        else:
            per_core = [[np.asarray(m[name]) for name in in_names] for m in in_maps]
            concat_in = [np.concatenate([per_core[c][i] for c in range(n_cores)], axis=0)
                         for i in range(n_params)]
            sh = NamedSharding(mesh, PartitionSpec("core"))
            dev_in = [jax.device_put(x, sh) for x in concat_in]
            jax.block_until_ready(dev_in)
            bench._key, bench._dev_in = id(in_maps), dev_in
        zeros = [np.zeros((n_cores * z.shape[0], *z.shape[1:]), z.dtype) for z in zero_outs]
        ts = []
        for i in range(iters):
            zs = [z.copy() for z in zeros]
            t0 = time.perf_counter()
            out = fn(*dev_in, *zs)
            jax.block_until_ready(out)
            ts.append(time.perf_counter() - t0)
        ts.sort()
        sl = 0.0
        if slope:
            # async slope: queue K calls, block once; removes dispatch latency
            for K_ in (2, 6):
                zss = [[z.copy() for z in zeros] for _ in range(K_)]
                t0 = time.perf_counter()
                outs = [fn(*dev_in, *zss[k]) for k in range(K_)]
                jax.block_until_ready(outs)
                tA = (time.perf_counter() - t0) if K_ == 2 else tA
                tB = (time.perf_counter() - t0) if K_ == 6 else 0.0
            sl = (tB - tA) / 4.0
        return ts[len(ts) // 2], {"sync": ts, "slope": sl}

    run.bench = bench
    return run


# ------------------------------------------------------------- host glue ----

_cache = {}


def _get_run1():
    if "r1" not in _cache:
        _cache["r1"] = _make_runner(build_neff1(), NC)
    return _cache["r1"]


def _prep_in1(emb, Wih_c, Whh_c, bih_c, bhh_c, Ws):
    f32, f16 = np.float32, np.float16
    WihcT = _fold_gates_T(Wih_c)
    WihH = WihcT.astype(f16)
    WihL = (WihcT - WihH.astype(f32)).astype(f16)
    WTc = _fold_gates_T(Whh_c).astype(f16)
    wd = Ws[1] - Ws[0]
    wd_hi = wd.astype(f16).astype(f32)
    wd_lo = (wd - wd_hi).astype(f16)
    wdP = np.zeros((8, 128, 128), f16)
    for k in range(4):
        wdP[2 * k, :, 0] = wd_hi[k * 128:(k + 1) * 128].astype(f16)
        wdP[2 * k + 1, :, 0] = wd_lo[k * 128:(k + 1) * 128]
    biasC = np.zeros((128, 12), f32)
    bsum = bih_c + bhh_c
    for m in range(12):
        biasC[:, m] = bsum[m * 128:(m + 1) * 128] if m < 8 else bih_c[m * 128:(m + 1) * 128]
    assert np.abs(bhh_c[1024:]).max() == 0.0, "nonzero bhh_c n-gate bias unsupported"
    in1 = []
    for c in range(NC):
        es = emb[c * BL:(c + 1) * BL]                 # [8, T, E]
        embT = np.ascontiguousarray(
            es.reshape(BL, T, 6, 128).transpose(3, 2, 1, 0)).reshape(128, 6 * T * BL)
        embH = embT.astype(f16)
        embL = (embT - embH.astype(f32)).astype(f16)
        in1.append({"embH": embH, "embL": embL, "WihH": WihH, "WihL": WihL,
                    "WTc": WTc, "wdP": wdP, "biasC": biasC})
    return in1


def _get_run2(t_pad, n_full):
    key = ("r2", t_pad, n_full)
    if key not in _cache:
        _cache[key] = _make_runner(build_neff2_v2(t_pad, n_full), NC)
    return _cache[key]


def _fold_gates_T(W):
    # W: [1536, K] -> [K/128, 128, 1536] lhsT tiles (W.T folded)
    K = W.shape[1]
    return np.ascontiguousarray(W.T.reshape(K // 128, 128, 1536))


def kernel(**inputs):
    emb = np.asarray(inputs["embedded"], np.float32)
    mask = np.asarray(inputs["mask"])
    lens = mask.sum(axis=1).astype(np.int64)
    f32 = np.float32
    Wih_c, Whh_c = np.asarray(inputs["Wih_c"], f32), np.asarray(inputs["Whh_c"], f32)
    bih_c, bhh_c = np.asarray(inputs["bih_c"], f32), np.asarray(inputs["bhh_c"], f32)
    Ws, bs = np.asarray(inputs["Ws"], f32), np.asarray(inputs["bs"], f32)
    Wih0, Whh0 = np.asarray(inputs["Wih0"], f32), np.asarray(inputs["Whh0"], f32)
    bih0, bhh0 = np.asarray(inputs["bih0"], f32), np.asarray(inputs["bhh0"], f32)
    Wih1, Whh1 = np.asarray(inputs["Wih1"], f32), np.asarray(inputs["Whh1"], f32)
    bih1, bhh1 = np.asarray(inputs["bih1"], f32), np.asarray(inputs["bhh1"], f32)
    Wc = {3: np.asarray(inputs["Wc3"], f32), 4: np.asarray(inputs["Wc4"], f32),
          5: np.asarray(inputs["Wc5"], f32)}
    bc = {3: np.asarray(inputs["bc3"], f32), 4: np.asarray(inputs["bc4"], f32),
          5: np.asarray(inputs["bc5"], f32)}
    Wo, bo = np.asarray(inputs["Wo"], f32), np.asarray(inputs["bo"], f32)

    run1 = _get_run1()
    in1 = _prep_in1(emb, Wih_c, Whh_c, bih_c, bhh_c, Ws)
    _cache["in1"] = in1
    res1 = run1(in1)
    margins = np.concatenate([r["margins"].reshape(T, BL).T[None] for r in res1], 0)
    margins = margins.reshape(NC * BL, T)                   # [B, T] (b-major per core)

    # ---- host compaction (bit logic + gather, zero FLOPs)
    sel, order, valid, t_pad, n_full = _compaction(margins, bs, lens)
    run2 = _get_run2(t_pad, n_full)
    in2 = _prep_in2(emb, order, valid, t_pad,
                    Wih0, Whh0, bih0, bhh0, Wih1, Whh1, bih1, bhh1,
                    Wc, bc, Wo, bo)
    _cache["in2"], _cache["last_r2"] = in2, run2
    res2 = run2(in2)
    out = np.concatenate([r["out"].reshape(BL) for r in res2], 0)
    return out.astype(np.float32)


def _build_floor():
    nc = bacc.Bacc("TRN2", target_bir_lowering=False, debug=False, num_devices=NC)
    x_in = nc.dram_tensor("x", [128, 8], F32, kind="ExternalInput").ap()
    y_out = nc.dram_tensor("y", [128, 8], F32, kind="ExternalOutput").ap()
    with TileContext(nc) as tc, ExitStack() as ctx:
        wp = ctx.enter_context(tc.tile_pool(name="w", bufs=1))
        xt = wp.tile([128, 8], F32, tag="x")
        nc.sync.dma_start(out=xt, in_=x_in)
        nc.sync.dma_start(out=y_out, in_=xt)
    nc.compile()
    return nc


def bench_hw(rounds=5, per=9):
    """Honest device-time estimate. Alternating between different NEFFs pays a
    ~40ms program-swap per call, so each NEFF is timed in a consecutive block
    (block median ignores the first-call swap); floor blocks bracket each NEFF
    block and the per-round delta uses the adjacent floor block, cancelling
    dispatch drift. Returns ns."""
    runf = _make_runner(_build_floor(), NC)
    inf = [{"x": np.zeros((128, 8), np.float32)} for _ in range(NC)]
    runf(inf)
    d1, d2 = [], []
    for r in range(rounds):
        tf1, _ = runf.bench(inf, iters=per, slope=False)
        t1, _ = _cache["r1"].bench(_cache["in1"], iters=per, slope=False)
        tf2, _ = runf.bench(inf, iters=per, slope=False)
        t2, _ = _cache["last_r2"].bench(_cache["in2"], iters=per, slope=False)
        d1.append(t1 - min(tf1, tf2))
        d2.append(t2 - tf2)
    d1.sort(); d2.sort()
    n1 = max(0.0, d1[len(d1) // 2]) * 1e9
    n2 = max(0.0, d2[len(d2) // 2]) * 1e9
    return {"neff1_ns": n1, "neff2_ns": n2, "total_ns": n1 + n2,
            "d1_ms": [round(x * 1e3, 2) for x in d1],
            "d2_ms": [round(x * 1e3, 2) for x in d2]}


def _compaction(margins, bs, lens):
    thr = bs[0] - bs[1]
    sel = (margins > thr).astype(np.int64)
    t_idx = np.arange(T)[None, :]
    sel[:, 0] = 1
    sel[np.arange(B), lens - 1] = 1
    sel = np.where(t_idx >= lens[:, None], 0, sel)
    nsel = sel.sum(1)
    order = np.argsort(1 - sel, axis=1, kind="stable")
    valid = t_idx < nsel[:, None]
    t_pad = min(T, max(32, int(-(-int(nsel.max()) // 32) * 32)))
    n_full = min(t_pad // 32, int(nsel.min()) // 32)
    return sel, order, valid, t_pad, n_full


def _prep_in2(emb, order, valid, t_pad, Wih0, Whh0, bih0, bhh0,
              Wih1, Whh1, bih1, bhh1, Wc, bc, Wo, bo):
    f32 = np.float32
    Wih0T = _fold_gates_T(Wih0).astype(ml_dtypes.bfloat16)
    WhT0 = _fold_gates_T(Whh0).astype(ml_dtypes.bfloat16)
    Wih1T = _fold_gates_T(Wih1).astype(ml_dtypes.bfloat16)
    WhT1 = _fold_gates_T(Whh1).astype(ml_dtypes.bfloat16)
    bias0 = np.zeros((128, 12), f32)
    b0sum = bih0 + bhh0
    for m in range(12):
        bias0[:, m] = b0sum[m * 128:(m + 1) * 128] if m < 8 else bih0[m * 128:(m + 1) * 128]
    assert np.abs(bhh0[1024:]).max() == 0.0 and np.abs(bhh1[1024:]).max() == 0.0
    bias1 = np.zeros((128, 12), f32)
    b1sum = bih1 + bhh1
    for m in range(12):
        bias1[:, m] = b1sum[m * 128:(m + 1) * 128] if m < 8 else bih1[m * 128:(m + 1) * 128]
    Wconv = np.zeros((12, 4, 128, 256), f32)
    dt_base = {3: 0, 4: 3, 5: 7}
    for fs in FS:
        Wf = Wc[fs][:, 0]                                   # [NF, fs, H]
        for dt in range(fs):
            for k in range(4):
                Wconv[dt_base[fs] + dt, k] = Wf[:, dt, k * 128:(k + 1) * 128].T
    Wconv = np.ascontiguousarray(Wconv.transpose(2, 0, 1, 3)).reshape(128, -1).astype(ml_dtypes.bfloat16)
    bconv = np.zeros((128, 6), f32)
    WoT = np.zeros((128, 6), f32)
    for fi, fs in enumerate(FS):
        for mt in range(2):
            bconv[:, fi * 2 + mt] = bc[fs][mt * 128:(mt + 1) * 128]
            WoT[:, fi * 2 + mt] = Wo[0, fi * 256 + mt * 128: fi * 256 + (mt + 1) * 128]

    in2 = []
    for c in range(NC):
        bsl = slice(c * BL, (c + 1) * BL)
        new_emb = np.take_along_axis(emb[bsl], order[bsl][:, :, None], axis=1)
        new_emb = (new_emb * valid[bsl][:, :, None])[:, :t_pad]
        nembT = np.ascontiguousarray(
            new_emb.reshape(BL, t_pad, 6, 128).transpose(3, 2, 1, 0)
        ).reshape(128, 6 * t_pad * BL).astype(ml_dtypes.bfloat16)
        vs = valid[bsl][:, :t_pad]
        maskf = np.ascontiguousarray(np.broadcast_to(
            vs.T.reshape(1, t_pad * BL), (128, t_pad * BL))).astype(ml_dtypes.bfloat16)
        masku = np.ascontiguousarray(np.broadcast_to(
            vs.T.reshape(1, t_pad * BL), (128, t_pad * BL))).astype(np.uint8)
        in2.append({"nembT": nembT, "maskf": maskf, "masku": masku, "Wih0T": Wih0T, "WhT0": WhT0,
                    "Wih1T": Wih1T, "WhT1": WhT1, "bias0": bias0, "bias1": bias1,
                    "identb": np.eye(128, dtype=ml_dtypes.bfloat16),
                    "Wconv": Wconv, "bconv": bconv, "WoT": WoT,
                    "bo": bo.reshape(1, 1)})
    return in2

